# revision 8
# baseline (speedup 1.0000x reference)
"""Longformer decoder (4 layers, sliding-window causal attention) on 8 trn2 cores.

Sharding: 4096 tokens (B=2 x S=2048) split into 8 contiguous chunks of 512
(core = b*4 + chunk). Activations are kept d-major ([dim, token], dim on
partitions) so every matmul is weights-stationary with no transposes.
Attention needs a 256-token left halo of K/V per layer: layer 0 computes it
locally from the embedding gather; layers 1-3 AllGather the residual-stream
halo over 4-core groups. Final projection is vocab-sharded: after an 8-core
AllGather of the final LN output, each core computes all 4096 tokens x its
4000-vocab slice of w_out (padded to 4096). Matmuls run in float32r
(full-speed fp32 mode, ~1.5e-4 rel err).
"""
import os
import sys

import numpy as np

for _p in ("/opt/trn_rl_repo", "/root/.axon_site/_ro/trn_rl_repo"):
    if os.path.isdir(_p) and _p not in sys.path:
        sys.path.insert(0, _p)

import concourse.bass as bass
import concourse.mybir as mybir
import concourse.tile as tile
from concourse import bacc
from concourse.bass import ts, ds
from concourse.bass_utils import run_bass_kernel_spmd
from concourse.masks import make_identity

F32 = mybir.dt.float32
F32R = mybir.dt.float32r
F16 = mybir.dt.float16
I32 = mybir.dt.int32
MDT = F16 if os.environ.get("KMMDT", "f16") == "f16" else F32R
AF = mybir.ActivationFunctionType
OP = mybir.AluOpType

B, S, V, D, H, NL, MLPD = 2, 2048, 32000, 512, 8, 4, 2048
DH = D // H            # 64
HALF = 256             # attention half-window (WIN // 2)
P = 128
NCORES = 8
CHUNK = 512            # own tokens per core
W = CHUNK + HALF       # 768 = halo + own
DT = D // P            # 4 d-tiles
MT = MLPD // P         # 16 mlp tiles
VN = V // P            # 250 vocab tiles (each core does full vocab x own tokens)
NTOK = B * S           # 4096
GROUPS = [[0, 1, 2, 3], [4, 5, 6, 7]]
EXP_SHIFT = 2.0
SCALE = float(1.0 / np.sqrt(DH))

_CACHE = {}


# ================================================================ builder
def _build():
    nc = bacc.Bacc("TRN2", target_bir_lowering=False, debug=False,
                   num_devices=NCORES)

    ein = lambda n, sh, dt=F32: nc.dram_tensor(n, sh, dt, kind="ExternalInput")
    io = dict(
        wq=ein("wq", [NL, D, D], MDT), wk=ein("wk", [NL, D, D], MDT),
        wv=ein("wv", [NL, D, D], MDT), wo=ein("wo", [NL, D, D], MDT),
        w1=ein("w1", [NL, D, MLPD], MDT), w2=ein("w2", [NL, MLPD, D], MDT),
        b1=ein("b1", [NL, MLPD]), b2=ein("b2", [NL, D]),
        ln1_s=ein("ln1_s", [NL, D]), ln1_b=ein("ln1_b", [NL, D]),
        ln2_s=ein("ln2_s", [NL, D]), ln2_b=ein("ln2_b", [NL, D]),
        lnf_s=ein("lnf_s", [1, D]), lnf_b=ein("lnf_b", [1, D]),
        w_tiles=ein("w_tiles", [VN, P, DT * P], MDT), b_tiles=ein("b_tiles", [P, VN]),
        embed=ein("embed", [V, D]),
        idx_in=ein("idx_in", [P, W // P], I32),
        pe_dm=ein("pe_dm", [D, W]),
        masks=ein("masks", [2, 4, P, 256]),
        halo_offs=ein("halo_offs", [P, DT], I32),
        out=nc.dram_tensor("logits_vm", [V, CHUNK], F32, kind="ExternalOutput"),
    )
    if os.environ.get("KDEBUG") == "1":
        io["xdump"] = nc.dram_tensor("xdump", [NL + 1, D, CHUNK], F32, kind="ExternalOutput")
        io["ydump"] = nc.dram_tensor("ydump", [D, W], F32, kind="ExternalOutput")
        io["adump"] = nc.dram_tensor("adump", [D, CHUNK], F32, kind="ExternalOutput")

    with tile.TileContext(nc) as tc, nc.allow_low_precision(reason="f32r rounding"):
        _emit(nc, tc, io)
    nc.compile()
    return nc


def _emit(nc, tc, io):
    cpool = tc.alloc_tile_pool(name="const", bufs=1)
    xpool = tc.alloc_tile_pool(name="xres", bufs=1)
    ps_a = tc.alloc_tile_pool(name="ps_a", bufs=2, space="PSUM")
    ps_b = tc.alloc_tile_pool(name="ps_b", bufs=4, space="PSUM")
    ps_c = tc.alloc_tile_pool(name="ps_c", bufs=2, space="PSUM")
    drp = tc.alloc_tile_pool(name="drbounce", bufs=1, space="DRAM")

    # ------------------------------------------------ constants
    ones_f = cpool.tile([P, P], F32, tag="ones_f")
    nc.vector.memset(ones_f[:], 1.0)
    ones = cpool.tile([P, P], MDT, tag="ones")
    nc.vector.tensor_copy(out=ones[:], in_=ones_f[:])
    ones_r = cpool.tile([P, P], F32R, tag="ones_r")
    nc.vector.tensor_copy(out=ones_r[:], in_=ones_f[:])
    negb = cpool.tile([P, 1], F32, tag="negb")
    nc.vector.memset(negb[:], EXP_SHIFT)
    epsb = cpool.tile([P, 1], F32, tag="epsb")
    nc.vector.memset(epsb[:], 1e-6)
    l1s = cpool.tile([P, NL, DT], F32, tag="l1s")
    l1b = cpool.tile([P, NL, DT], F32, tag="l1b")
    l2s = cpool.tile([P, NL, DT], F32, tag="l2s")
    l2b = cpool.tile([P, NL, DT], F32, tag="l2b")
    lfs = cpool.tile([P, DT], F32, tag="lfs")
    lfb = cpool.tile([P, DT], F32, tag="lfb")
    nc.sync.dma_start(out=l1s[:], in_=io["ln1_s"].ap().rearrange("l (t p) -> p l t", p=P))
    nc.sync.dma_start(out=l1b[:], in_=io["ln1_b"].ap().rearrange("l (t p) -> p l t", p=P))
    nc.sync.dma_start(out=l2s[:], in_=io["ln2_s"].ap().rearrange("l (t p) -> p l t", p=P))
    nc.sync.dma_start(out=l2b[:], in_=io["ln2_b"].ap().rearrange("l (t p) -> p l t", p=P))
    nc.sync.dma_start(out=lfs[:], in_=io["lnf_s"].ap().rearrange("o (t p) -> p (o t)", p=P))
    nc.sync.dma_start(out=lfb[:], in_=io["lnf_b"].ap().rearrange("o (t p) -> p (o t)", p=P))
    b1t = cpool.tile([P, NL, MT], F32, tag="b1t")
    b2t = cpool.tile([P, NL, DT], F32, tag="b2t")
    bot = cpool.tile([P, VN], F32, tag="bot")
    nc.sync.dma_start(out=b1t[:], in_=io["b1"].ap().rearrange("l (m p) -> p l m", p=P))
    nc.sync.dma_start(out=b2t[:], in_=io["b2"].ap().rearrange("l (t p) -> p l t", p=P))
    nc.sync.dma_start(out=bot[:], in_=io["b_tiles"].ap())
    masks = cpool.tile([P, 2, 4, 256], F32, tag="masks")
    nc.sync.dma_start(out=masks[:], in_=io["masks"].ap().rearrange("a b p q -> p a b q"))
    hoffs = cpool.tile([P, DT], I32, tag="hoffs")
    nc.sync.dma_start(out=hoffs[:], in_=io["halo_offs"].ap())

    # residual stream (own 512 tokens, d-major) + per-layer halo
    x = xpool.tile([P, DT, CHUNK], F32, tag="x")
    xh = xpool.tile([P, DT, HALF], F32, tag="xh")

    # ------------------------------------------------ embedding
    with tc.tile_pool(name="embed", bufs=1) as epool:
        ident = epool.tile([P, P], F32, tag="ident")
        make_identity(nc, ident[:])
        pe = epool.tile([P, DT, W], F32, tag="pe")
        nc.sync.dma_start(out=pe[:], in_=io["pe_dm"].ap().rearrange("(t p) m -> p t m", p=P))
        idxt = epool.tile([P, W // P], I32, tag="idxt")
        nc.sync.dma_start(out=idxt[:], in_=io["idx_in"].ap())
        with tc.tile_pool(name="gath", bufs=2) as gpool:
            for g in range(W // P):
                gt = gpool.tile([P, D], F32, tag="gt")
                nc.gpsimd.indirect_dma_start(
                    out=gt[:], out_offset=None, in_=io["embed"].ap(),
                    in_offset=bass.IndirectOffsetOnAxis(ap=idxt[:, g:g + 1], axis=0),
                )
                for dt in range(DT):
                    pt = ps_a.tile([P, P], F32, tag="ps_a")
                    nc.tensor.transpose(pt[:], gt[:, ts(dt, P)], ident[:])
                    dst = xh[:, dt, ts(g, P)] if g < 2 else x[:, dt, ts(g - 2, P)]
                    nc.vector.tensor_add(out=dst, in0=pt[:], in1=pe[:, dt, ts(g, P)])

    def dump_x(slot):
        if "xdump" in io:
            nc.sync.dma_start(out=io["xdump"].ap()[slot].rearrange("(t p) m -> p t m", p=P),
                              in_=x[:, :, :])
    dump_x(0)

    # ------------------------------------------------ layer pools
    lp = tc.alloc_tile_pool(name="layers", bufs=1)
    tp = tc.alloc_tile_pool(name="ltrans", bufs=2)
    lp3 = tc.alloc_tile_pool(name="ltrans3", bufs=3)

    def emit_ln(srcs, y, s_of, b_of):
        """LN over d. srcs: list of (fn(dt)->AP[128,width], y_col0, width).
        y: [P, DT, w_tok] F32R out. s_of/b_of: fn(dt)->AP[P,1]."""
        srcs2 = []
        for fn, col0, width in srcs:
            for o in range(0, width, 256):
                srcs2.append((lambda dt, fn=fn, o=o: fn(dt)[:, ds(o, 256)], col0 + o, 256))
        for fn, col0, width in srcs2:
            sx = ps_a.tile([1, 512], F32, tag="ps_a")
            sxx = ps_a.tile([1, 512], F32, tag="ps_a")
            for dt in range(DT):
                xr = lp3.tile([P, 512], MDT, tag="ln_xr", bufs=2)
                xsq = lp3.tile([P, 512], MDT, tag="ln_xsq", bufs=2)
                nc.gpsimd.tensor_copy(out=xr[:, :width], in_=fn(dt))
                nc.vector.tensor_mul(out=xsq[:, :width], in0=fn(dt), in1=fn(dt))
                nc.tensor.matmul(out=sx[:, :width], lhsT=ones[:, 0:1], rhs=xr[:, :width],
                                 start=(dt == 0), stop=(dt == DT - 1))
                nc.tensor.matmul(out=sxx[:, :width], lhsT=ones[:, 0:1], rhs=xsq[:, :width],
                                 start=(dt == 0), stop=(dt == DT - 1))
            mu = lp3.tile([1, 512], F32R, tag="ln_mu", bufs=2)
            mu2 = lp3.tile([1, 512], F32, tag="ln_mu2", bufs=1)
            var = lp3.tile([1, 512], F32, tag="ln_var", bufs=1)
            sd = lp3.tile([1, 512], F32, tag="ln_sd", bufs=1)
            rstd = lp3.tile([1, 512], F32R, tag="ln_rstd", bufs=2)
            nc.vector.tensor_scalar_mul(out=mu[:, :width], in0=sx[:, :width], scalar1=1.0 / D)
            nc.vector.tensor_mul(out=mu2[:, :width], in0=mu[:, :width], in1=mu[:, :width])
            # var = sxx/D - mu^2
            nc.vector.scalar_tensor_tensor(
                out=var[:, :width], in0=sxx[:, :width], scalar=1.0 / D,
                in1=mu2[:, :width], op0=OP.mult, op1=OP.subtract)
            nc.scalar.activation(sd[:, :width], var[:, :width], AF.Sqrt, bias=epsb[0:1, :], scale=1.0)
            nc.vector.reciprocal(out=rstd[:, :width], in_=sd[:, :width])
            pmu = ps_a.tile([P, 512], F32, tag="ps_a")
            nc.tensor.matmul(out=pmu[:, :width], lhsT=ones_r[0:1, :], rhs=mu[:, :width],
                             start=True, stop=True)
            prs = ps_a.tile([P, 512], F32, tag="ps_a")
            nc.tensor.matmul(out=prs[:, :width], lhsT=ones_r[0:1, :], rhs=rstd[:, :width],
                             start=True, stop=True)
            mu_b = lp3.tile([P, 512], F32, tag="ln_mub", bufs=1)
            rs_b = lp3.tile([P, 512], F32, tag="ln_rsb", bufs=1)
            nc.vector.tensor_copy(out=mu_b[:, :width], in_=pmu[:, :width])
            nc.vector.tensor_copy(out=rs_b[:, :width], in_=prs[:, :width])
            for dt in range(DT):
                scr = lp3.tile([P, 512], F32, tag="ln_scr", bufs=2)
                nc.vector.tensor_sub(out=scr[:, :width], in0=fn(dt), in1=mu_b[:, :width])
                nc.vector.tensor_mul(out=scr[:, :width], in0=scr[:, :width], in1=rs_b[:, :width])
                nc.vector.tensor_scalar(out=y[:, dt, ds(col0, width)], in0=scr[:, :width],
                                        scalar1=s_of(dt), scalar2=b_of(dt),
                                        op0=OP.mult, op1=OP.add)

    def load_w(dram_ap, tag_r, shape3, rpool=None):
        wr = (rpool or tp).tile(shape3, MDT, tag=tag_r)
        nc.sync.dma_start(out=wr[:], in_=dram_ap)
        return wr

    # ------------------------------------------------ transformer layers
    _knl = int(os.environ.get("KNL", NL))
    _skipatt = os.environ.get("KSKIPATT") == "1"
    _skipmlp = os.environ.get("KSKIPMLP") == "1"
    _skipfin = os.environ.get("KSKIPFIN") == "1"
    for l in range(_knl):
        y = lp.tile([P, DT, W], MDT, tag="y")
        emit_ln(
            srcs=[(lambda dt: x[:, dt, :], HALF, CHUNK),
                  (lambda dt: xh[:, dt, :], 0, HALF)],
            y=y, s_of=lambda dt: l1s[:, l % NL, dt:dt + 1], b_of=lambda dt: l1b[:, l % NL, dt:dt + 1])

        # --- projections (weights stationary, d-major out)
        wq_r = load_w(io["wq"].ap()[l % NL].rearrange("(t p) m -> p t m", p=P), "wr", [P, DT, D])
        qr = lp.tile([P, DT, CHUNK], MDT, tag="qr")
        for do in range(DT):
            pq = ps_a.tile([P, CHUNK], F32, tag="ps_a")
            for dt in range(DT):
                nc.tensor.matmul(out=pq[:], lhsT=wq_r[:, dt, ts(do, P)],
                                 rhs=y[:, dt, ds(HALF, CHUNK)],
                                 start=(dt == 0), stop=(dt == DT - 1))
            nc.vector.tensor_copy(out=qr[:, do, :], in_=pq[:])

        wk_r = load_w(io["wk"].ap()[l % NL].rearrange("(t p) m -> p t m", p=P), "wr", [P, DT, D])
        kr = lp.tile([P, DT, W], MDT, tag="kr")
        for do in range(DT):
            for c0, cw in ((HALF, CHUNK), (0, HALF)):
                pk = ps_a.tile([P, CHUNK], F32, tag="ps_a")
                for dt in range(DT):
                    nc.tensor.matmul(out=pk[:, :cw], lhsT=wk_r[:, dt, ts(do, P)],
                                     rhs=y[:, dt, ds(c0, cw)],
                                     start=(dt == 0), stop=(dt == DT - 1))
                nc.vector.tensor_copy(out=kr[:, do, ds(c0, cw)], in_=pk[:, :cw])

        wv_r = load_w(io["wv"].ap()[l % NL].rearrange("(t p) m -> p t m", p=P), "wr", [P, DT, D])
        vt = [lp.tile([P, H * (DH + 1)], MDT, tag=f"vt{t}", name=f"vt{t}") for t in range(W // P)]
        for t in range(W // P):
            pv = ps_a.tile([P, D], F32, tag="ps_a")
            for dt in range(DT):
                nc.tensor.matmul(out=pv[:], lhsT=y[:, dt, ts(t, P)], rhs=wv_r[:, dt, :],
                                 start=(dt == 0), stop=(dt == DT - 1))
            vtv = vt[t][:].rearrange("p (h c) -> p h c", c=DH + 1)
            nc.vector.tensor_copy(out=vtv[:, :, 0:DH],
                                  in_=pv[:].rearrange("p (h c) -> p h c", c=DH))
            nc.vector.tensor_copy(out=vtv[:, :, DH:DH + 1], in_=ones[:, 0:H])

        # --- sliding-window attention
        attr = lp.tile([P, DT, CHUNK], MDT, tag="attr")
        for h in (range(0) if _skipatt else range(H)):
            r0 = (h % 2) * DH
            dto = h // 2
            for qB in range(2):
                pa = ps_c.tile([DH + 1, 256], F32, tag="ps_c")
                for j in range(4):
                    kt = qB * 2 + j
                    pscore = ps_b.tile([P, 256], F32, tag="ps_b")
                    nc.tensor.matmul(
                        out=pscore[:],
                        lhsT=kr[ds(r0, DH), dto, ds(qB * 256 + j * P, P)],
                        rhs=qr[ds(r0, DH), dto, ds(qB * 256, 256)],
                        start=True, stop=True)
                    ej = lp3.tile([P, 256], MDT, tag="ej", bufs=4)
                    nc.scalar.activation(ej[:], pscore[:], AF.Exp, bias=negb[:], scale=SCALE)
                    nc.vector.tensor_mul(out=ej[:], in0=ej[:], in1=masks[:, qB, j, :])
                    nc.tensor.matmul(out=pa[:], lhsT=vt[kt][:, ds(h * (DH + 1), DH + 1)],
                                     rhs=ej[:], start=(j == 0), stop=(j == 3))
                rr = lp3.tile([1, 256], F32R, tag="rr")
                nc.vector.reciprocal(out=rr[:], in_=pa[DH:DH + 1, :])
                pbc = ps_c.tile([DH, 256], F32, tag="ps_c")
                nc.tensor.matmul(out=pbc[:], lhsT=ones_r[0:1, 0:DH], rhs=rr[:],
                                 start=True, stop=True)
                bcs = lp3.tile([DH, 256], MDT, tag="bcs")
                nc.vector.tensor_copy(out=bcs[:], in_=pbc[:])
                nc.vector.tensor_mul(out=attr[ds(r0, DH), dto, ds(qB * 256, 256)],
                                     in0=pa[0:DH, :], in1=bcs[:])
        if _skipatt:
            for dt in range(DT):
                nc.vector.tensor_copy(out=attr[:, dt, :], in_=qr[:, dt, :])

        if l == 0 and "ydump" in io:
            yd = lp3.tile([P, DT, W], F32, tag="ydump_t", bufs=1)
            nc.vector.tensor_copy(out=yd[:], in_=y[:])
            nc.sync.dma_start(out=io["ydump"].ap().rearrange("(t p) m -> p t m", p=P), in_=yd[:])
        if l == 0 and "adump" in io:
            ad = lp3.tile([P, DT, CHUNK], F32, tag="adump_t", bufs=1)
            nc.vector.tensor_copy(out=ad[:], in_=attr[:])
            nc.sync.dma_start(out=io["adump"].ap().rearrange("(t p) m -> p t m", p=P), in_=ad[:])

        # --- output projection + residual
        wo_r = load_w(io["wo"].ap()[l % NL].rearrange("(t p) m -> p t m", p=P), "wr", [P, DT, D])
        for do in range(DT):
            po = ps_a.tile([P, CHUNK], F32, tag="ps_a")
            for dt in range(DT):
                nc.tensor.matmul(out=po[:], lhsT=wo_r[:, dt, ts(do, P)],
                                 rhs=attr[:, dt, :], start=(dt == 0), stop=(dt == DT - 1))
            nc.vector.tensor_add(out=x[:, do, :], in0=x[:, do, :], in1=po[:])

        # --- LN2 + MLP
        y2 = lp.tile([P, DT, CHUNK], MDT, tag="y2")
        emit_ln(srcs=[(lambda dt: x[:, dt, :], 0, CHUNK)], y=y2,
                s_of=lambda dt: l2s[:, l % NL, dt:dt + 1], b_of=lambda dt: l2b[:, l % NL, dt:dt + 1])

        pb = [ps_b.tile([P, CHUNK], F32, tag="ps_b", name=f"pb{i}") for i in range(DT)]
        w1r = lp.tile([P, DT, MLPD], MDT, tag="w1r")
        nc.sync.dma_start(out=w1r[:], in_=io["w1"].ap()[l % NL].rearrange("(t p) m -> p t m", p=P))
        w2r = lp.tile([P, MT, D], MDT, tag="w2r")
        nc.sync.dma_start(out=w2r[:], in_=io["w2"].ap()[l % NL].rearrange("(t p) m -> p t m", p=P))

        def emit_mlp2(m, hm):
            for do in range(DT):
                nc.tensor.matmul(out=pb[do][:], lhsT=w2r[:, m, ts(do, P)],
                                 rhs=hm[:], start=(m == 0), stop=(m == MT - 1))

        hist = []
        for m in (range(0) if _skipmlp else range(MT)):
            p1 = ps_a.tile([P, CHUNK], F32, tag="ps_a")
            for dt in range(DT):
                nc.tensor.matmul(out=p1[:], lhsT=w1r[:, dt, ts(m, P)],
                                 rhs=y2[:, dt, :],
                                 start=(dt == 0), stop=(dt == DT - 1))
            hm = lp3.tile([P, CHUNK], MDT, tag="hm", bufs=3)
            nc.scalar.activation(hm[:], p1[:], AF.Gelu_apprx_tanh,
                                 bias=b1t[:, l % NL, m:m + 1], scale=1.0)
            hist.append((m, hm))
            if len(hist) > 2:
                emit_mlp2(*hist.pop(0))
        for mm_, hh_ in hist:
            emit_mlp2(mm_, hh_)
        # residual (+b2), then send halo for next layer
        for do in (range(0) if _skipmlp else range(DT)):
            nc.vector.scalar_tensor_tensor(
                out=x[:, do, :], in0=pb[do][:],
                scalar=b2t[:, l % NL, do:do + 1], in1=x[:, do, :],
                op0=OP.add, op1=OP.add)
        if l < NL - 1:
            agin = drp.tile([D, HALF], F32, tag=f"agin{l}")
            agout = drp.tile([len(GROUPS[0]) * D, HALF], F32, tag=f"agout{l}")
            nc.sync.dma_start(out=agin[:].rearrange("(t p) m -> p t m", p=P),
                              in_=x[:, :, ds(HALF, HALF)])
            nc.gpsimd.collective_compute(
                "AllGather", OP.bypass, replica_groups=GROUPS,
                ins=[agin.opt()], outs=[agout.opt()])
        if l < NL - 1:
            for dt in range(DT):
                nc.gpsimd.indirect_dma_start(
                    out=xh[:, dt, :], out_offset=None, in_=agout[:],
                    in_offset=bass.IndirectOffsetOnAxis(ap=hoffs[:, dt:dt + 1], axis=0))
        dump_x(l + 1)

    # ------------------------------------------------ final LN + logits
    # Each core computes the FULL vocab for its own 512 tokens: no final
    # AllGather; w_out streams tile-by-tile from DRAM, prefetched by the
    # pool double-buffering.
    yf = lp.tile([P, DT, CHUNK], MDT, tag="y")
    emit_ln(srcs=[(lambda dt: x[:, dt, :], 0, CHUNK)], y=yf,
            s_of=lambda dt: lfs[:, dt:dt + 1], b_of=lambda dt: lfb[:, dt:dt + 1])

    lp3.release()
    tp.release()

    ps_c.release()
    ps_b.release()
    fps = tc.alloc_tile_pool(name="fps", bufs=4, space="PSUM")
    with tc.tile_pool(name="ftrans", bufs=4) as ftp, \
         tc.tile_pool(name="fout", bufs=4) as fop:
        for v_i in (range(0) if _skipfin else range(VN)):
            fwr = ftp.tile([P, DT, P], MDT, tag="fwr")
            nc.sync.dma_start(out=fwr[:],
                              in_=io["w_tiles"].ap()[v_i].rearrange("p (t q) -> p t q", t=DT))
            pf = fps.tile([P, CHUNK], F32, tag="fps")
            for dt in range(DT):
                nc.tensor.matmul(out=pf[:], lhsT=fwr[:, dt, :], rhs=yf[:, dt, :],
                                 start=(dt == 0), stop=(dt == DT - 1))
            ot = fop.tile([P, CHUNK], F32, tag="fot")
            nc.vector.tensor_scalar_add(out=ot[:], in0=pf[:],
                                        scalar1=bot[:, v_i:v_i + 1])
            nc.sync.dma_start(out=io["out"].ap()[ts(v_i, P), :], in_=ot[:])

    fps.release()
    lp.release()
    drp.release()
    ps_a.release()
    xpool.release()
    cpool.release()


# ================================================================ host side
def _pe_table():
    pos = np.arange(S, dtype=np.float32)[:, None]
    div = np.exp(np.arange(0, D, 2, dtype=np.float32) * -(np.log(10000.0) / D))
    pe = np.zeros((S, D), dtype=np.float32)
    pe[:, 0::2] = np.sin(pos * div)
    pe[:, 1::2] = np.cos(pos * div)
    return pe


def _in_maps(inputs):
    inp = np.asarray(inputs["inputs"]).astype(np.int32)
    ids = np.pad(inp, ((0, 0), (1, 0)))[:, :-1].astype(np.int32)
    pe = _pe_table()
    wout = np.asarray(inputs["w_out"], dtype=np.float32).astype(np.float16)
    bout = np.asarray(inputs["b_out"], dtype=np.float32)
    shared = {k: np.ascontiguousarray(np.asarray(inputs[k], dtype=np.float32))
              for k in ("embed", "b1", "b2", "ln1_s", "ln1_b", "ln2_s", "ln2_b")}
    for k in ("wq", "wk", "wv", "wo", "w1", "w2"):
        shared[k] = np.ascontiguousarray(
            np.asarray(inputs[k], dtype=np.float32).astype(np.float16))
    shared["lnf_s"] = np.asarray(inputs["lnf_s"], np.float32).reshape(1, D)
    shared["lnf_b"] = np.asarray(inputs["lnf_b"], np.float32).reshape(1, D)
    # w_tiles[v_i, p, dt*128+q] = w_out[dt*128+p, v_i*128+q]
    shared["w_tiles"] = np.ascontiguousarray(
        wout.reshape(DT, P, VN, P).transpose(2, 1, 0, 3).reshape(VN, P, DT * P))
    shared["b_tiles"] = np.ascontiguousarray(bout.reshape(VN, P).T)

    maps = []
    qi = np.arange(256)[None, :]
    ki = np.arange(P)[:, None]
    for c in range(NCORES):
        b, ch = divmod(c, NCORES // B)
        t0 = ch * CHUNK
        lo = t0 - HALF
        ids768 = np.zeros(W, np.int32)
        pe768 = np.zeros((W, D), np.float32)
        s0 = max(0, lo)
        ids768[s0 - lo:] = ids[b, s0:t0 + CHUNK]
        pe768[s0 - lo:] = pe[s0:t0 + CHUNK]
        m = np.zeros((2, 4, P, 256), np.float32)
        for qB in range(2):
            for j in range(4):
                w = 256 + qi - (j * P + ki)      # u_q - u_k
                ok = (w >= 0) & (w <= HALF)
                if ch == 0:
                    ok = ok & ((lo + qB * 256 + j * P + ki) >= 0)
                m[qB, j] = ok.astype(np.float32)
        src = ch - 1 if ch > 0 else 0
        hoffs = (src * D + np.arange(DT)[None, :] * P
                 + np.arange(P)[:, None]).astype(np.int32)
        mp = dict(shared)
        mp.update(
            idx_in=np.ascontiguousarray(ids768.reshape(W // P, P).T),
            pe_dm=np.ascontiguousarray(pe768.T),
            masks=m, halo_offs=hoffs)
        maps.append(mp)
    return maps


def kernel(**inputs):
    nc = _CACHE.get("nc")
    if nc is None:
        nc = _build()
        _CACHE["nc"] = nc
    maps = _in_maps(inputs)
    res = run_bass_kernel_spmd(nc, maps, list(range(NCORES))).results
    full = np.empty((NTOK, V), np.float32)
    for c in range(NCORES):
        full[c * CHUNK:(c + 1) * CHUNK, :] = res[c]["logits_vm"].T
    return full.reshape(B, S, V)



# revision 17
# speedup vs baseline: 1.2878x; 1.2878x over previous
"""Longformer decoder (4 layers, sliding-window causal attention) on 8 trn2 cores.

Sharding: 4096 tokens (B=2 x S=2048) split into 8 contiguous chunks of 512
(core = b*4 + chunk). Activations are kept d-major ([dim, token], dim on
partitions) so every matmul is weights-stationary with no transposes.
Attention needs a 256-token left halo of K/V per layer: layer 0 computes it
locally from the embedding gather; layers 1-3 AllGather the residual-stream
halo over 4-core groups. Final projection is vocab-sharded: after an 8-core
AllGather of the final LN output, each core computes all 4096 tokens x its
4000-vocab slice of w_out (padded to 4096). Matmuls run in float32r
(full-speed fp32 mode, ~1.5e-4 rel err).
"""
import os
import sys

import numpy as np

for _p in ("/opt/trn_rl_repo", "/root/.axon_site/_ro/trn_rl_repo"):
    if os.path.isdir(_p) and _p not in sys.path:
        sys.path.insert(0, _p)

import concourse.bass as bass
import concourse.mybir as mybir
import concourse.tile as tile
from concourse import bacc
from concourse.bass import ts, ds
from concourse.bass_utils import run_bass_kernel_spmd
from concourse.masks import make_identity

F32 = mybir.dt.float32
F32R = mybir.dt.float32r
F16 = mybir.dt.float16
I32 = mybir.dt.int32
MDT = F16 if os.environ.get("KMMDT", "f16") == "f16" else F32R
AF = mybir.ActivationFunctionType
OP = mybir.AluOpType

B, S, V, D, H, NL, MLPD = 2, 2048, 32000, 512, 8, 4, 2048
DH = D // H            # 64
HALF = 256             # attention half-window (WIN // 2)
P = 128
NCORES = 8
CHUNK = 512            # own tokens per core
W = CHUNK + HALF       # 768 = halo + own
DT = D // P            # 4 d-tiles
MT = MLPD // P         # 16 mlp tiles
VN = V // P            # 250 vocab tiles (each core does full vocab x own tokens)
GL = 5                 # vocab tiles per DMA group
VG = VN // GL          # 50 groups (batched DMA: 5KB contiguous per partition)
NTOK = B * S           # 4096
GROUPS = [[0, 1, 2, 3], [4, 5, 6, 7]]
EXP_SHIFT = 2.0
SCALE = float(1.0 / np.sqrt(DH))

_CACHE = {}


# ================================================================ builder
def _build():
    nc = bacc.Bacc("TRN2", target_bir_lowering=False, debug=False,
                   num_devices=NCORES)

    ein = lambda n, sh, dt=F32: nc.dram_tensor(n, sh, dt, kind="ExternalInput")
    io = dict(
        wq=ein("wq", [NL, D, D], MDT), wk=ein("wk", [NL, D, D], MDT),
        wv=ein("wv", [NL, D, D], MDT), wo=ein("wo", [NL, D, D], MDT),
        w1=ein("w1", [NL, D, MLPD], MDT), w2=ein("w2", [NL, MLPD, D], MDT),
        b1=ein("b1", [NL, MLPD]), b2=ein("b2", [NL, D]),
        ln1_s=ein("ln1_s", [NL, D]), ln1_b=ein("ln1_b", [NL, D]),
        ln2_s=ein("ln2_s", [NL, D]), ln2_b=ein("ln2_b", [NL, D]),
        lnf_s=ein("lnf_s", [1, D]), lnf_b=ein("lnf_b", [1, D]),
        w_tiles=ein("w_tiles", [VG, P, GL * DT * P], MDT),
        embed=ein("embed", [V, D]),
        idx_in=ein("idx_in", [P, W // P], I32),
        pe_dm=ein("pe_dm", [D, W]),
        masks=ein("masks", [2, 4, P, 256]),
        halo_offs=ein("halo_offs", [P, DT], I32),
        out=nc.dram_tensor("logits_vm", [VG, P, GL * CHUNK], F16, kind="ExternalOutput"),
    )
    if os.environ.get("KDEBUG") == "1":
        io["xdump"] = nc.dram_tensor("xdump", [NL + 1, D, CHUNK], F32, kind="ExternalOutput")
        io["ydump"] = nc.dram_tensor("ydump", [D, W], F32, kind="ExternalOutput")
        io["adump"] = nc.dram_tensor("adump", [D, CHUNK], F32, kind="ExternalOutput")

    with tile.TileContext(nc) as tc, nc.allow_low_precision(reason="f32r rounding"):
        _emit(nc, tc, io)
    nc.compile()
    return nc


def _emit(nc, tc, io):
    cpool = tc.alloc_tile_pool(name="const", bufs=1)
    xpool = tc.alloc_tile_pool(name="xres", bufs=1)
    ps_a = tc.alloc_tile_pool(name="ps_a", bufs=2, space="PSUM")
    ps_b = tc.alloc_tile_pool(name="ps_b", bufs=4, space="PSUM")
    ps_c = tc.alloc_tile_pool(name="ps_c", bufs=2, space="PSUM")
    drp = tc.alloc_tile_pool(name="drbounce", bufs=1, space="DRAM")

    # ------------------------------------------------ constants
    ones_f = cpool.tile([P, P], F32, tag="ones_f")
    nc.vector.memset(ones_f[:], 1.0)
    ones = cpool.tile([P, P], MDT, tag="ones")
    nc.vector.tensor_copy(out=ones[:], in_=ones_f[:])
    ones_r = cpool.tile([P, P], F32R, tag="ones_r")
    nc.vector.tensor_copy(out=ones_r[:], in_=ones_f[:])
    negb = cpool.tile([P, 1], F32, tag="negb")
    nc.vector.memset(negb[:], EXP_SHIFT)
    epsb = cpool.tile([P, 1], F32, tag="epsb")
    nc.vector.memset(epsb[:], 1e-6)
    l1s = cpool.tile([P, NL, DT], F32, tag="l1s")
    l1b = cpool.tile([P, NL, DT], F32, tag="l1b")
    l2s = cpool.tile([P, NL, DT], F32, tag="l2s")
    l2b = cpool.tile([P, NL, DT], F32, tag="l2b")
    lfs = cpool.tile([P, DT], F32, tag="lfs")
    lfb = cpool.tile([P, DT], F32, tag="lfb")
    nc.sync.dma_start(out=l1s[:], in_=io["ln1_s"].ap().rearrange("l (t p) -> p l t", p=P))
    nc.sync.dma_start(out=l1b[:], in_=io["ln1_b"].ap().rearrange("l (t p) -> p l t", p=P))
    nc.sync.dma_start(out=l2s[:], in_=io["ln2_s"].ap().rearrange("l (t p) -> p l t", p=P))
    nc.sync.dma_start(out=l2b[:], in_=io["ln2_b"].ap().rearrange("l (t p) -> p l t", p=P))
    nc.sync.dma_start(out=lfs[:], in_=io["lnf_s"].ap().rearrange("o (t p) -> p (o t)", p=P))
    nc.sync.dma_start(out=lfb[:], in_=io["lnf_b"].ap().rearrange("o (t p) -> p (o t)", p=P))
    b1t = cpool.tile([P, NL, MT], F32, tag="b1t")
    b2t = cpool.tile([P, NL, DT], F32, tag="b2t")
    nc.sync.dma_start(out=b1t[:], in_=io["b1"].ap().rearrange("l (m p) -> p l m", p=P))
    nc.sync.dma_start(out=b2t[:], in_=io["b2"].ap().rearrange("l (t p) -> p l t", p=P))
    masks = cpool.tile([P, 2, 4, 256], F32, tag="masks")
    nc.sync.dma_start(out=masks[:], in_=io["masks"].ap().rearrange("a b p q -> p a b q"))
    hoffs = cpool.tile([P, DT], I32, tag="hoffs")
    nc.sync.dma_start(out=hoffs[:], in_=io["halo_offs"].ap())

    # residual stream (own 512 tokens, d-major) + per-layer halo
    x = xpool.tile([P, DT, CHUNK], F32, tag="x")
    xh = xpool.tile([P, DT, HALF], F32, tag="xh")

    # ------------------------------------------------ embedding
    with tc.tile_pool(name="embed", bufs=1) as epool:
        ident = epool.tile([P, P], F32, tag="ident")
        make_identity(nc, ident[:])
        pe = epool.tile([P, DT, W], F32, tag="pe")
        nc.sync.dma_start(out=pe[:], in_=io["pe_dm"].ap().rearrange("(t p) m -> p t m", p=P))
        idxt = epool.tile([P, W // P], I32, tag="idxt")
        nc.sync.dma_start(out=idxt[:], in_=io["idx_in"].ap())
        with tc.tile_pool(name="gath", bufs=2) as gpool:
            for g in range(W // P):
                gt = gpool.tile([P, D], F32, tag="gt")
                nc.gpsimd.indirect_dma_start(
                    out=gt[:], out_offset=None, in_=io["embed"].ap(),
                    in_offset=bass.IndirectOffsetOnAxis(ap=idxt[:, g:g + 1], axis=0),
                )
                for dt in range(DT):
                    pt = ps_a.tile([P, P], F32, tag="ps_a")
                    nc.tensor.transpose(pt[:], gt[:, ts(dt, P)], ident[:])
                    dst = xh[:, dt, ts(g, P)] if g < 2 else x[:, dt, ts(g - 2, P)]
                    nc.vector.tensor_add(out=dst, in0=pt[:], in1=pe[:, dt, ts(g, P)])

    def dump_x(slot):
        if "xdump" in io:
            nc.sync.dma_start(out=io["xdump"].ap()[slot].rearrange("(t p) m -> p t m", p=P),
                              in_=x[:, :, :])
    dump_x(0)

    # ------------------------------------------------ layer pools
    lp = tc.alloc_tile_pool(name="layers", bufs=1)
    tp = tc.alloc_tile_pool(name="ltrans", bufs=2)
    lp3 = tc.alloc_tile_pool(name="ltrans3", bufs=3)

    def emit_ln(srcs, y, s_of, b_of):
        """LN over d. srcs: list of (fn(dt)->AP[128,width], y_col0, width).
        y: [P, DT, w_tok] F32R out. s_of/b_of: fn(dt)->AP[P,1]."""
        srcs2 = []
        for fn, col0, width in srcs:
            for o in range(0, width, 256):
                srcs2.append((lambda dt, fn=fn, o=o: fn(dt)[:, ds(o, 256)], col0 + o, 256))
        for fn, col0, width in srcs2:
            sx = ps_a.tile([1, 512], F32, tag="ps_a")
            sxx = ps_a.tile([1, 512], F32, tag="ps_a")
            for dt in range(DT):
                xr = lp3.tile([P, 512], MDT, tag="ln_xr", bufs=2)
                xsq = lp3.tile([P, 512], MDT, tag="ln_xsq", bufs=2)
                nc.gpsimd.tensor_copy(out=xr[:, :width], in_=fn(dt))
                nc.vector.tensor_mul(out=xsq[:, :width], in0=fn(dt), in1=fn(dt))
                nc.tensor.matmul(out=sx[:, :width], lhsT=ones[:, 0:1], rhs=xr[:, :width],
                                 start=(dt == 0), stop=(dt == DT - 1))
                nc.tensor.matmul(out=sxx[:, :width], lhsT=ones[:, 0:1], rhs=xsq[:, :width],
                                 start=(dt == 0), stop=(dt == DT - 1))
            mu = lp3.tile([1, 512], F32R, tag="ln_mu", bufs=2)
            mu2 = lp3.tile([1, 512], F32, tag="ln_mu2", bufs=1)
            var = lp3.tile([1, 512], F32, tag="ln_var", bufs=1)
            sd = lp3.tile([1, 512], F32, tag="ln_sd", bufs=1)
            rstd = lp3.tile([1, 512], F32R, tag="ln_rstd", bufs=2)
            nc.vector.tensor_scalar_mul(out=mu[:, :width], in0=sx[:, :width], scalar1=1.0 / D)
            nc.vector.tensor_mul(out=mu2[:, :width], in0=mu[:, :width], in1=mu[:, :width])
            # var = sxx/D - mu^2
            nc.vector.scalar_tensor_tensor(
                out=var[:, :width], in0=sxx[:, :width], scalar=1.0 / D,
                in1=mu2[:, :width], op0=OP.mult, op1=OP.subtract)
            nc.scalar.activation(sd[:, :width], var[:, :width], AF.Sqrt, bias=epsb[0:1, :], scale=1.0)
            nc.vector.reciprocal(out=rstd[:, :width], in_=sd[:, :width])
            pmu = ps_a.tile([P, 512], F32, tag="ps_a")
            nc.tensor.matmul(out=pmu[:, :width], lhsT=ones_r[0:1, :], rhs=mu[:, :width],
                             start=True, stop=True)
            prs = ps_a.tile([P, 512], F32, tag="ps_a")
            nc.tensor.matmul(out=prs[:, :width], lhsT=ones_r[0:1, :], rhs=rstd[:, :width],
                             start=True, stop=True)
            mu_b = lp3.tile([P, 512], F32, tag="ln_mub", bufs=1)
            rs_b = lp3.tile([P, 512], F32, tag="ln_rsb", bufs=1)
            nc.vector.tensor_copy(out=mu_b[:, :width], in_=pmu[:, :width])
            nc.vector.tensor_copy(out=rs_b[:, :width], in_=prs[:, :width])
            for dt in range(DT):
                scr = lp3.tile([P, 512], F32, tag="ln_scr", bufs=2)
                nc.vector.tensor_sub(out=scr[:, :width], in0=fn(dt), in1=mu_b[:, :width])
                nc.vector.tensor_mul(out=scr[:, :width], in0=scr[:, :width], in1=rs_b[:, :width])
                nc.vector.tensor_scalar(out=y[:, dt, ds(col0, width)], in0=scr[:, :width],
                                        scalar1=s_of(dt), scalar2=b_of(dt),
                                        op0=OP.mult, op1=OP.add)

    def load_w(dram_ap, tag_r, shape3, rpool=None):
        wr = (rpool or tp).tile(shape3, MDT, tag=tag_r)
        nc.sync.dma_start(out=wr[:], in_=dram_ap)
        return wr

    # ------------------------------------------------ transformer layers
    _knl = int(os.environ.get("KNL", NL))
    _skipatt = os.environ.get("KSKIPATT") == "1"
    _skipmlp = os.environ.get("KSKIPMLP") == "1"
    _skipfin = os.environ.get("KSKIPFIN") == "1"
    for l in range(_knl):
        y = lp.tile([P, DT, W], MDT, tag="y")
        emit_ln(
            srcs=[(lambda dt: x[:, dt, :], HALF, CHUNK),
                  (lambda dt: xh[:, dt, :], 0, HALF)],
            y=y, s_of=lambda dt: l1s[:, l % NL, dt:dt + 1], b_of=lambda dt: l1b[:, l % NL, dt:dt + 1])

        # --- projections (weights stationary, d-major out)
        wq_r = load_w(io["wq"].ap()[l % NL].rearrange("(t p) m -> p t m", p=P), "wr", [P, DT, D])
        qr = lp.tile([P, DT, CHUNK], MDT, tag="qr")
        for do in range(DT):
            pq = ps_a.tile([P, CHUNK], F32, tag="ps_a")
            for dt in range(DT):
                nc.tensor.matmul(out=pq[:], lhsT=wq_r[:, dt, ts(do, P)],
                                 rhs=y[:, dt, ds(HALF, CHUNK)],
                                 start=(dt == 0), stop=(dt == DT - 1))
            nc.vector.tensor_copy(out=qr[:, do, :], in_=pq[:])

        wk_r = load_w(io["wk"].ap()[l % NL].rearrange("(t p) m -> p t m", p=P), "wr", [P, DT, D])
        kr = lp.tile([P, DT, W], MDT, tag="kr")
        for do in range(DT):
            for c0, cw in ((HALF, CHUNK), (0, HALF)):
                pk = ps_a.tile([P, CHUNK], F32, tag="ps_a")
                for dt in range(DT):
                    nc.tensor.matmul(out=pk[:, :cw], lhsT=wk_r[:, dt, ts(do, P)],
                                     rhs=y[:, dt, ds(c0, cw)],
                                     start=(dt == 0), stop=(dt == DT - 1))
                nc.vector.tensor_copy(out=kr[:, do, ds(c0, cw)], in_=pk[:, :cw])

        wv_r = load_w(io["wv"].ap()[l % NL].rearrange("(t p) m -> p t m", p=P), "wr", [P, DT, D])
        vt = [lp.tile([P, H * (DH + 1)], MDT, tag=f"vt{t}", name=f"vt{t}") for t in range(W // P)]
        for t in range(W // P):
            pv = ps_a.tile([P, D], F32, tag="ps_a")
            for dt in range(DT):
                nc.tensor.matmul(out=pv[:], lhsT=y[:, dt, ts(t, P)], rhs=wv_r[:, dt, :],
                                 start=(dt == 0), stop=(dt == DT - 1))
            vtv = vt[t][:].rearrange("p (h c) -> p h c", c=DH + 1)
            nc.vector.tensor_copy(out=vtv[:, :, 0:DH],
                                  in_=pv[:].rearrange("p (h c) -> p h c", c=DH))
            nc.vector.tensor_copy(out=vtv[:, :, DH:DH + 1], in_=ones[:, 0:H])

        # --- sliding-window attention
        attr = lp.tile([P, DT, CHUNK], MDT, tag="attr")
        for h in (range(0) if _skipatt else range(H)):
            r0 = (h % 2) * DH
            dto = h // 2
            for qB in range(2):
                pa = ps_c.tile([DH + 1, 256], F32, tag="ps_c")
                for j in range(4):
                    kt = qB * 2 + j
                    pscore = ps_b.tile([P, 256], F32, tag="ps_b")
                    nc.tensor.matmul(
                        out=pscore[:],
                        lhsT=kr[ds(r0, DH), dto, ds(qB * 256 + j * P, P)],
                        rhs=qr[ds(r0, DH), dto, ds(qB * 256, 256)],
                        start=True, stop=True)
                    ej = lp3.tile([P, 256], MDT, tag="ej", bufs=4)
                    nc.scalar.activation(ej[:], pscore[:], AF.Exp, bias=negb[:], scale=SCALE)
                    nc.vector.tensor_mul(out=ej[:], in0=ej[:], in1=masks[:, qB, j, :])
                    nc.tensor.matmul(out=pa[:], lhsT=vt[kt][:, ds(h * (DH + 1), DH + 1)],
                                     rhs=ej[:], start=(j == 0), stop=(j == 3))
                rr = lp3.tile([1, 256], F32R, tag="rr")
                nc.vector.reciprocal(out=rr[:], in_=pa[DH:DH + 1, :])
                pbc = ps_c.tile([DH, 256], F32, tag="ps_c")
                nc.tensor.matmul(out=pbc[:], lhsT=ones_r[0:1, 0:DH], rhs=rr[:],
                                 start=True, stop=True)
                bcs = lp3.tile([DH, 256], MDT, tag="bcs")
                nc.vector.tensor_copy(out=bcs[:], in_=pbc[:])
                nc.vector.tensor_mul(out=attr[ds(r0, DH), dto, ds(qB * 256, 256)],
                                     in0=pa[0:DH, :], in1=bcs[:])
        if _skipatt:
            for dt in range(DT):
                nc.vector.tensor_copy(out=attr[:, dt, :], in_=qr[:, dt, :])

        if l == 0 and "ydump" in io:
            yd = lp3.tile([P, DT, W], F32, tag="ydump_t", bufs=1)
            nc.vector.tensor_copy(out=yd[:], in_=y[:])
            nc.sync.dma_start(out=io["ydump"].ap().rearrange("(t p) m -> p t m", p=P), in_=yd[:])
        if l == 0 and "adump" in io:
            ad = lp3.tile([P, DT, CHUNK], F32, tag="adump_t", bufs=1)
            nc.vector.tensor_copy(out=ad[:], in_=attr[:])
            nc.sync.dma_start(out=io["adump"].ap().rearrange("(t p) m -> p t m", p=P), in_=ad[:])

        # --- output projection + residual
        wo_r = load_w(io["wo"].ap()[l % NL].rearrange("(t p) m -> p t m", p=P), "wr", [P, DT, D])
        for do in range(DT):
            po = ps_a.tile([P, CHUNK], F32, tag="ps_a")
            for dt in range(DT):
                nc.tensor.matmul(out=po[:], lhsT=wo_r[:, dt, ts(do, P)],
                                 rhs=attr[:, dt, :], start=(dt == 0), stop=(dt == DT - 1))
            nc.vector.tensor_add(out=x[:, do, :], in0=x[:, do, :], in1=po[:])

        # --- LN2 + MLP
        y2 = lp.tile([P, DT, CHUNK], MDT, tag="y2")
        emit_ln(srcs=[(lambda dt: x[:, dt, :], 0, CHUNK)], y=y2,
                s_of=lambda dt: l2s[:, l % NL, dt:dt + 1], b_of=lambda dt: l2b[:, l % NL, dt:dt + 1])

        pb = [ps_b.tile([P, CHUNK], F32, tag="ps_b", name=f"pb{i}") for i in range(DT)]
        w1r = lp.tile([P, DT, MLPD], MDT, tag="w1r")
        nc.sync.dma_start(out=w1r[:], in_=io["w1"].ap()[l % NL].rearrange("(t p) m -> p t m", p=P))
        w2r = lp.tile([P, MT, D], MDT, tag="w2r")
        nc.sync.dma_start(out=w2r[:], in_=io["w2"].ap()[l % NL].rearrange("(t p) m -> p t m", p=P))

        def emit_mlp2(m, hm):
            for do in range(DT):
                nc.tensor.matmul(out=pb[do][:], lhsT=w2r[:, m, ts(do, P)],
                                 rhs=hm[:], start=(m == 0), stop=(m == MT - 1))

        hist = []
        for m in (range(0) if _skipmlp else range(MT)):
            p1 = ps_a.tile([P, CHUNK], F32, tag="ps_a")
            for dt in range(DT):
                nc.tensor.matmul(out=p1[:], lhsT=w1r[:, dt, ts(m, P)],
                                 rhs=y2[:, dt, :],
                                 start=(dt == 0), stop=(dt == DT - 1))
            hm = lp3.tile([P, CHUNK], MDT, tag="hm", bufs=3)
            nc.scalar.activation(hm[:], p1[:], AF.Gelu_apprx_tanh,
                                 bias=b1t[:, l % NL, m:m + 1], scale=1.0)
            hist.append((m, hm))
            if len(hist) > 2:
                emit_mlp2(*hist.pop(0))
        for mm_, hh_ in hist:
            emit_mlp2(mm_, hh_)
        # residual (+b2), then send halo for next layer
        for do in (range(0) if _skipmlp else range(DT)):
            nc.vector.scalar_tensor_tensor(
                out=x[:, do, :], in0=pb[do][:],
                scalar=b2t[:, l % NL, do:do + 1], in1=x[:, do, :],
                op0=OP.add, op1=OP.add)
        if l < NL - 1:
            agin = drp.tile([D, HALF], F32, tag=f"agin{l}")
            agout = drp.tile([len(GROUPS[0]) * D, HALF], F32, tag=f"agout{l}")
            nc.sync.dma_start(out=agin[:].rearrange("(t p) m -> p t m", p=P),
                              in_=x[:, :, ds(HALF, HALF)])
            nc.gpsimd.collective_compute(
                "AllGather", OP.bypass, replica_groups=GROUPS,
                ins=[agin.opt()], outs=[agout.opt()])
        if l < NL - 1:
            for dt in range(DT):
                nc.gpsimd.indirect_dma_start(
                    out=xh[:, dt, :], out_offset=None, in_=agout[:],
                    in_offset=bass.IndirectOffsetOnAxis(ap=hoffs[:, dt:dt + 1], axis=0))
        dump_x(l + 1)

    # ------------------------------------------------ final LN + logits
    # Each core computes the FULL vocab for its own 512 tokens: no final
    # AllGather; w_out streams tile-by-tile from DRAM, prefetched by the
    # pool double-buffering.
    yf = lp.tile([P, DT, CHUNK], MDT, tag="y")
    emit_ln(srcs=[(lambda dt: x[:, dt, :], 0, CHUNK)], y=yf,
            s_of=lambda dt: lfs[:, dt:dt + 1], b_of=lambda dt: lfb[:, dt:dt + 1])

    lp3.release()
    tp.release()

    ps_c.release()
    ps_b.release()
    fps = tc.alloc_tile_pool(name="fps", bufs=4, space="PSUM")
    # psum->sbuf copies round-robin across vector/scalar/gpsimd so no single
    # engine bottlenecks; bias is added host-side (pure linear post-step).
    with tc.tile_pool(name="ftrans", bufs=3) as ftp, \
         tc.tile_pool(name="fout", bufs=3) as fop:
        for g in (range(0) if _skipfin else range(VG)):
            fwr = ftp.tile([P, GL, DT, P], MDT, tag="fwr")
            nc.sync.dma_start(out=fwr[:],
                              in_=io["w_tiles"].ap()[g]
                              .rearrange("p (j t q) -> p j t q", j=GL, t=DT))
            ot = fop.tile([P, GL, CHUNK], F16, tag="fot")
            for j in range(GL):
                pf = fps.tile([P, CHUNK], F32, tag="fps")
                for dt in range(DT):
                    nc.tensor.matmul(out=pf[:], lhsT=fwr[:, j, dt, :], rhs=yf[:, dt, :],
                                     start=(dt == 0), stop=(dt == DT - 1))
                if (g * GL + j) % 2 == 0:
                    nc.vector.tensor_copy(out=ot[:, j, :], in_=pf[:])
                else:
                    nc.scalar.activation(ot[:, j, :], pf[:], AF.Copy)
            nc.sync.dma_start(out=io["out"].ap()[g],
                              in_=ot[:].rearrange("p j m -> p (j m)"))

    fps.release()
    lp.release()
    drp.release()
    ps_a.release()
    xpool.release()
    cpool.release()


# ================================================================ host side
def _pe_table():
    pos = np.arange(S, dtype=np.float32)[:, None]
    div = np.exp(np.arange(0, D, 2, dtype=np.float32) * -(np.log(10000.0) / D))
    pe = np.zeros((S, D), dtype=np.float32)
    pe[:, 0::2] = np.sin(pos * div)
    pe[:, 1::2] = np.cos(pos * div)
    return pe


def _in_maps(inputs):
    inp = np.asarray(inputs["inputs"]).astype(np.int32)
    ids = np.pad(inp, ((0, 0), (1, 0)))[:, :-1].astype(np.int32)
    pe = _pe_table()
    wout = np.asarray(inputs["w_out"], dtype=np.float32).astype(np.float16)
    bout = np.asarray(inputs["b_out"], dtype=np.float32)
    shared = {k: np.ascontiguousarray(np.asarray(inputs[k], dtype=np.float32))
              for k in ("embed", "b1", "b2", "ln1_s", "ln1_b", "ln2_s", "ln2_b")}
    for k in ("wq", "wk", "wv", "wo", "w1", "w2"):
        shared[k] = np.ascontiguousarray(
            np.asarray(inputs[k], dtype=np.float32).astype(np.float16))
    shared["lnf_s"] = np.asarray(inputs["lnf_s"], np.float32).reshape(1, D)
    shared["lnf_b"] = np.asarray(inputs["lnf_b"], np.float32).reshape(1, D)
    # w_tiles[g, p, ((j*DT+dt)*P)+q] = w_out[dt*128+p, (g*GL+j)*128+q]
    shared["w_tiles"] = np.ascontiguousarray(
        wout.reshape(DT, P, VG, GL, P).transpose(2, 1, 3, 0, 4)
        .reshape(VG, P, GL * DT * P))

    maps = []
    qi = np.arange(256)[None, :]
    ki = np.arange(P)[:, None]
    for c in range(NCORES):
        b, ch = divmod(c, NCORES // B)
        t0 = ch * CHUNK
        lo = t0 - HALF
        ids768 = np.zeros(W, np.int32)
        pe768 = np.zeros((W, D), np.float32)
        s0 = max(0, lo)
        ids768[s0 - lo:] = ids[b, s0:t0 + CHUNK]
        pe768[s0 - lo:] = pe[s0:t0 + CHUNK]
        m = np.zeros((2, 4, P, 256), np.float32)
        for qB in range(2):
            for j in range(4):
                w = 256 + qi - (j * P + ki)      # u_q - u_k
                ok = (w >= 0) & (w <= HALF)
                if ch == 0:
                    ok = ok & ((lo + qB * 256 + j * P + ki) >= 0)
                m[qB, j] = ok.astype(np.float32)
        src = ch - 1 if ch > 0 else 0
        hoffs = (src * D + np.arange(DT)[None, :] * P
                 + np.arange(P)[:, None]).astype(np.int32)
        mp = dict(shared)
        mp.update(
            idx_in=np.ascontiguousarray(ids768.reshape(W // P, P).T),
            pe_dm=np.ascontiguousarray(pe768.T),
            masks=m, halo_offs=hoffs)
        maps.append(mp)
    return maps


def kernel(**inputs):
    nc = _CACHE.get("nc")
    if nc is None:
        nc = _build()
        _CACHE["nc"] = nc
    maps = _in_maps(inputs)
    res = run_bass_kernel_spmd(nc, maps, list(range(NCORES))).results
    bout = np.asarray(inputs["b_out"], dtype=np.float32)
    full = np.empty((NTOK, V), np.float32)
    for c in range(NCORES):
        lv = (res[c]["logits_vm"].reshape(VG, P, GL, CHUNK)
              .transpose(0, 2, 1, 3).reshape(V, CHUNK))
        full[c * CHUNK:(c + 1) * CHUNK, :] = lv.T.astype(np.float32) + bout[None, :]
    return full.reshape(B, S, V)



# revision 25
# speedup vs baseline: 1.3737x; 1.0668x over previous
"""Longformer decoder (4 layers, sliding-window causal attention) on 8 trn2 cores.

Sharding: 4096 tokens (B=2 x S=2048) split into 8 contiguous chunks of 512
(core = b*4 + chunk). Activations are kept d-major ([dim, token], dim on
partitions) so every matmul is weights-stationary with no transposes.
Attention needs a 256-token left halo of K/V per layer: layer 0 computes it
locally from the embedding gather; layers 1-3 AllGather the residual-stream
halo over 4-core groups, overlapped with the next layer's halo-independent
work (LN1/Q/K/V over own tokens). Sliding-window masking is additive (-3000)
and applied inside PSUM via an identity-matmul accumulate, so the scalar
engine's exp produces masked probabilities directly. Softmax division is
deferred: unnormalized attention output and per-(head,q) denominators are
collected, one batched reciprocal per q-block computes 1/den for all heads,
and an indicator-matrix matmul broadcasts it back over the d-major layout.
The final projection is token-sharded: each core computes the FULL vocab for
its own 512 tokens (no final AllGather); w_out streams from DRAM in 5-tile
groups, and the output bias is added host-side.
"""
import os
import sys

import numpy as np

for _p in ("/opt/trn_rl_repo", "/root/.axon_site/_ro/trn_rl_repo"):
    if os.path.isdir(_p) and _p not in sys.path:
        sys.path.insert(0, _p)

import concourse.bass as bass
import concourse.mybir as mybir
import concourse.tile as tile
from concourse import bacc
from concourse.bass import ts, ds
from concourse.bass_utils import run_bass_kernel_spmd
from concourse.masks import make_identity

F32 = mybir.dt.float32
F32R = mybir.dt.float32r
F16 = mybir.dt.float16
I32 = mybir.dt.int32
MDT = F16 if os.environ.get("KMMDT", "f16") == "f16" else F32R
AF = mybir.ActivationFunctionType
OP = mybir.AluOpType

B, S, V, D, H, NL, MLPD = 2, 2048, 32000, 512, 8, 4, 2048
DH = D // H            # 64
HALF = 256             # attention half-window (WIN // 2)
P = 128
NCORES = 8
CHUNK = 512            # own tokens per core
W = CHUNK + HALF       # 768 = halo + own
DT = D // P            # 4 d-tiles
MT = MLPD // P         # 16 mlp tiles
VN = V // P            # 250 vocab tiles (each core does full vocab x own tokens)
GL = 5                 # vocab tiles per DMA group
VG = VN // GL          # 50 groups (batched DMA: 5KB contiguous per partition)
NTOK = B * S           # 4096
GROUPS = [[0, 1, 2, 3], [4, 5, 6, 7]]
EXP_SHIFT = 2.0
MASK_BIAS = -3000.0    # additive mask; exp(SCALE*(s+MASK_BIAS)+EXP_SHIFT) == 0
SCALE = float(1.0 / np.sqrt(DH))

_CACHE = {}


# ================================================================ builder
def _build():
    nc = bacc.Bacc("TRN2", target_bir_lowering=False, debug=False,
                   num_devices=NCORES)

    ein = lambda n, sh, dt=F32: nc.dram_tensor(n, sh, dt, kind="ExternalInput")
    io = dict(
        wq=ein("wq", [NL, D, D], MDT), wk=ein("wk", [NL, D, D], MDT),
        wv=ein("wv", [NL, D, D], MDT), wo=ein("wo", [NL, D, D], MDT),
        w1=ein("w1", [NL, D, MLPD], MDT), w2=ein("w2", [NL, MLPD, D], MDT),
        b1=ein("b1", [NL, MLPD]), b2=ein("b2", [NL, D]),
        ln1_s=ein("ln1_s", [NL, D]), ln1_b=ein("ln1_b", [NL, D]),
        ln2_s=ein("ln2_s", [NL, D]), ln2_b=ein("ln2_b", [NL, D]),
        lnf_s=ein("lnf_s", [1, D]), lnf_b=ein("lnf_b", [1, D]),
        w_tiles=ein("w_tiles", [VG, P, GL * DT * P], MDT),
        embed=ein("embed", [V, D]),
        idx_in=ein("idx_in", [P, W // P], I32),
        pe_dm=ein("pe_dm", [D, W]),
        masks=ein("masks", [2, 4, P, 256], MDT),
        halo_offs=ein("halo_offs", [P, DT], I32),
        out=nc.dram_tensor("logits_vm", [VG, P, GL * CHUNK], F16, kind="ExternalOutput"),
    )
    if os.environ.get("KDEBUG") == "1":
        io["d_y"] = nc.dram_tensor("d_y", [D, CHUNK], F16, kind="ExternalOutput")
        io["d_yh"] = nc.dram_tensor("d_yh", [D, HALF], F16, kind="ExternalOutput")
        io["d_attru"] = nc.dram_tensor("d_attru", [D, CHUNK], F32, kind="ExternalOutput")
        io["d_attr"] = nc.dram_tensor("d_attr", [D, CHUNK], F16, kind="ExternalOutput")
        io["d_rf"] = nc.dram_tensor("d_rf", [16, 256], F32, kind="ExternalOutput")
        io["d_x1"] = nc.dram_tensor("d_x1", [D, CHUNK], F32, kind="ExternalOutput")

    with tile.TileContext(nc) as tc, nc.allow_low_precision(reason="f32r rounding"):
        _emit(nc, tc, io)
    nc.compile()
    return nc


def _emit(nc, tc, io):
    cpool = tc.alloc_tile_pool(name="const", bufs=1)
    xpool = tc.alloc_tile_pool(name="xres", bufs=1)
    ps_a = tc.alloc_tile_pool(name="ps_a", bufs=2, space="PSUM")
    ps_b = tc.alloc_tile_pool(name="ps_b", bufs=4, space="PSUM")
    ps_c = tc.alloc_tile_pool(name="ps_c", bufs=2, space="PSUM")
    drp = tc.alloc_tile_pool(name="drbounce", bufs=1, space="DRAM")

    # ------------------------------------------------ constants
    ones_f = cpool.tile([P, P], F32, tag="ones_f")
    nc.vector.memset(ones_f[:], 1.0)
    ones = cpool.tile([P, P], MDT, tag="ones")
    nc.vector.tensor_copy(out=ones[:], in_=ones_f[:])
    ones_r = cpool.tile([P, P], F32R, tag="ones_r")
    nc.vector.tensor_copy(out=ones_r[:], in_=ones_f[:])
    identm = cpool.tile([P, P], MDT, tag="identm")
    make_identity(nc, identm[:])
    negb = cpool.tile([P, 1], F32, tag="negb")
    nc.vector.memset(negb[:], EXP_SHIFT)
    epsb = cpool.tile([P, 1], F32, tag="epsb")
    nc.vector.memset(epsb[:], 1e-6)
    l1s = cpool.tile([P, NL, DT], F32, tag="l1s")
    l1b = cpool.tile([P, NL, DT], F32, tag="l1b")
    l2s = cpool.tile([P, NL, DT], F32, tag="l2s")
    l2b = cpool.tile([P, NL, DT], F32, tag="l2b")
    lfs = cpool.tile([P, DT], F32, tag="lfs")
    lfb = cpool.tile([P, DT], F32, tag="lfb")
    nc.sync.dma_start(out=l1s[:], in_=io["ln1_s"].ap().rearrange("l (t p) -> p l t", p=P))
    nc.sync.dma_start(out=l1b[:], in_=io["ln1_b"].ap().rearrange("l (t p) -> p l t", p=P))
    nc.sync.dma_start(out=l2s[:], in_=io["ln2_s"].ap().rearrange("l (t p) -> p l t", p=P))
    nc.sync.dma_start(out=l2b[:], in_=io["ln2_b"].ap().rearrange("l (t p) -> p l t", p=P))
    nc.sync.dma_start(out=lfs[:], in_=io["lnf_s"].ap().rearrange("o (t p) -> p (o t)", p=P))
    nc.sync.dma_start(out=lfb[:], in_=io["lnf_b"].ap().rearrange("o (t p) -> p (o t)", p=P))
    b1t = cpool.tile([P, NL, MT], F32, tag="b1t")
    b2t = cpool.tile([P, NL, DT], F32, tag="b2t")
    nc.sync.dma_start(out=b1t[:], in_=io["b1"].ap().rearrange("l (m p) -> p l m", p=P))
    nc.sync.dma_start(out=b2t[:], in_=io["b2"].ap().rearrange("l (t p) -> p l t", p=P))
    masks = cpool.tile([P, 2, 4, 256], MDT, tag="masks")
    nc.sync.dma_start(out=masks[:], in_=io["masks"].ap().rearrange("a b p q -> p a b q"))
    hoffs = cpool.tile([P, DT], I32, tag="hoffs")
    nc.sync.dma_start(out=hoffs[:], in_=io["halo_offs"].ap())

    # residual stream (own 512 tokens, d-major, f32r so LN sum-matmuls can
    # consume it directly at full PE rate) + per-layer halo
    x = xpool.tile([P, DT, CHUNK], F32R, tag="x")
    xh = xpool.tile([P, DT, HALF], F32R, tag="xh")

    # ------------------------------------------------ embedding
    with tc.tile_pool(name="embed", bufs=1) as epool:
        ident = epool.tile([P, P], F32, tag="ident")
        make_identity(nc, ident[:])
        pe = epool.tile([P, DT, W], F32, tag="pe")
        nc.sync.dma_start(out=pe[:], in_=io["pe_dm"].ap().rearrange("(t p) m -> p t m", p=P))
        idxt = epool.tile([P, W // P], I32, tag="idxt")
        nc.sync.dma_start(out=idxt[:], in_=io["idx_in"].ap())
        with tc.tile_pool(name="gath", bufs=2) as gpool:
            for g in range(W // P):
                gt = gpool.tile([P, D], F32, tag="gt")
                nc.gpsimd.indirect_dma_start(
                    out=gt[:], out_offset=None, in_=io["embed"].ap(),
                    in_offset=bass.IndirectOffsetOnAxis(ap=idxt[:, g:g + 1], axis=0),
                )
                for dt in range(DT):
                    pt = ps_a.tile([P, P], F32, tag="ps_a")
                    nc.tensor.transpose(pt[:], gt[:, ts(dt, P)], ident[:])
                    dst = xh[:, dt, ts(g, P)] if g < 2 else x[:, dt, ts(g - 2, P)]
                    nc.vector.tensor_add(out=dst, in0=pt[:], in1=pe[:, dt, ts(g, P)])

    # ------------------------------------------------ layer pools
    lp = tc.alloc_tile_pool(name="layers", bufs=1)
    tp = tc.alloc_tile_pool(name="ltrans", bufs=2)
    lp3 = tc.alloc_tile_pool(name="ltrans3", bufs=3)
    vtp = tc.alloc_tile_pool(name="vtpool", bufs=1)

    # V tiles with a trailing ones column per head: PV matmul row DH
    # accumulates the softmax denominator for free. Ones written once.
    vt = [vtp.tile([P, H * (DH + 1)], MDT, tag=f"vt{t}", name=f"vt{t}")
          for t in range(W // P)]
    for t in range(W // P):
        vtv = vt[t][:].rearrange("p (h c) -> p h c", c=DH + 1)
        nc.vector.tensor_copy(out=vtv[:, :, DH:DH + 1], in_=ones[:, 0:H])

    def emit_ln(blocks, s_of, b_of):
        """LN over d (partition axis x DT). blocks: list of
        (src_fn(dt)->AP[128,width], dst_fn(dt)->AP[128,width], width<=512)."""
        for fn, dst, width in blocks:
            sx = ps_a.tile([1, 512], F32, tag="ps_a")
            sxx = ps_a.tile([1, 512], F32, tag="ps_a")
            for dt in range(DT):
                xsq = lp3.tile([P, 512], MDT, tag="ln_xsq", bufs=2)
                nc.vector.tensor_mul(out=xsq[:, :width], in0=fn(dt), in1=fn(dt))
                nc.tensor.matmul(out=sx[:, :width], lhsT=ones_r[:, 0:1], rhs=fn(dt),
                                 start=(dt == 0), stop=(dt == DT - 1))
                nc.tensor.matmul(out=sxx[:, :width], lhsT=ones[:, 0:1], rhs=xsq[:, :width],
                                 start=(dt == 0), stop=(dt == DT - 1))
            mu = lp3.tile([1, 512], MDT, tag="ln_mu", bufs=2)
            nc.vector.tensor_scalar_mul(out=mu[:, :width], in0=sx[:, :width], scalar1=1.0 / D)
            mu2 = lp3.tile([1, 512], F32, tag="ln_mu2", bufs=1)
            nc.vector.tensor_mul(out=mu2[:, :width], in0=mu[:, :width], in1=mu[:, :width])
            var = lp3.tile([1, 512], F32, tag="ln_var", bufs=1)
            # var = sxx/D - mu^2
            nc.vector.scalar_tensor_tensor(
                out=var[:, :width], in0=sxx[:, :width], scalar=1.0 / D,
                in1=mu2[:, :width], op0=OP.mult, op1=OP.subtract)
            sd = lp3.tile([1, 512], F32, tag="ln_sd", bufs=1)
            nc.scalar.activation(sd[:, :width], var[:, :width], AF.Sqrt, bias=epsb[0:1, :], scale=1.0)
            rt = lp3.tile([1, 512], F32, tag="ln_rt", bufs=1)
            nc.vector.reciprocal_approx_fast(out=rt[:, :width], in_=sd[:, :width])
            rstd = lp3.tile([1, 512], MDT, tag="ln_rstd", bufs=2)
            nc.vector.tensor_copy(out=rstd[:, :width], in_=rt[:, :width])
            pmu = ps_c.tile([P, 512], F32, tag="ps_c")
            nc.tensor.matmul(out=pmu[:, :width], lhsT=ones[0:1, :], rhs=mu[:, :width],
                             start=True, stop=True)
            prs = ps_c.tile([P, 512], F32, tag="ps_c")
            nc.tensor.matmul(out=prs[:, :width], lhsT=ones[0:1, :], rhs=rstd[:, :width],
                             start=True, stop=True)
            for dt in range(DT):
                scr = lp3.tile([P, 512], F32, tag="ln_scr", bufs=2)
                nc.vector.tensor_sub(out=scr[:, :width], in0=fn(dt), in1=pmu[:, :width])
                nc.vector.tensor_mul(out=scr[:, :width], in0=scr[:, :width], in1=prs[:, :width])
                nc.vector.tensor_scalar(out=dst(dt), in0=scr[:, :width],
                                        scalar1=s_of(dt), scalar2=b_of(dt),
                                        op0=OP.mult, op1=OP.add)

    def load_w(dram_ap, tag_r, shape3, rpool=None):
        wr = (rpool or tp).tile(shape3, MDT, tag=tag_r)
        nc.sync.dma_start(out=wr[:], in_=dram_ap)
        return wr

    # ------------------------------------------------ transformer layers
    _knl = int(os.environ.get("KNL", NL))
    for l in range(_knl):
        li = l % NL
        s1 = lambda dt: l1s[:, li, dt:dt + 1]
        b1_ = lambda dt: l1b[:, li, dt:dt + 1]
        y = lp.tile([P, DT, CHUNK], MDT, tag="y")
        yh = lp.tile([P, DT, HALF], MDT, tag="yh")
        # LN1 over own tokens first: independent of the halo AllGather.
        emit_ln([(lambda dt: x[:, dt, :], lambda dt: y[:, dt, :], CHUNK)], s1, b1_)

        # --- projections (weights stationary, d-major out), own tokens
        wq_r = load_w(io["wq"].ap()[li].rearrange("(t p) m -> p t m", p=P), "wr", [P, DT, D])
        qr = lp.tile([P, DT, CHUNK], MDT, tag="qr")
        for do in range(DT):
            pq = ps_a.tile([P, CHUNK], F32, tag="ps_a")
            for dt in range(DT):
                nc.tensor.matmul(out=pq[:], lhsT=wq_r[:, dt, ts(do, P)],
                                 rhs=y[:, dt, :], start=(dt == 0), stop=(dt == DT - 1))
            nc.vector.tensor_copy(out=qr[:, do, :], in_=pq[:])

        wk_r = load_w(io["wk"].ap()[li].rearrange("(t p) m -> p t m", p=P), "wr", [P, DT, D])
        kro = lp.tile([P, DT, CHUNK], MDT, tag="kro")
        for do in range(DT):
            pk = ps_a.tile([P, CHUNK], F32, tag="ps_a")
            for dt in range(DT):
                nc.tensor.matmul(out=pk[:], lhsT=wk_r[:, dt, ts(do, P)],
                                 rhs=y[:, dt, :], start=(dt == 0), stop=(dt == DT - 1))
            nc.vector.tensor_copy(out=kro[:, do, :], in_=pk[:])

        wv_r = load_w(io["wv"].ap()[li].rearrange("(t p) m -> p t m", p=P), "wr", [P, DT, D])
        for t in range(2, W // P):
            pv = ps_a.tile([P, D], F32, tag="ps_a")
            for dt in range(DT):
                nc.tensor.matmul(out=pv[:], lhsT=y[:, dt, ts(t - 2, P)], rhs=wv_r[:, dt, :],
                                 start=(dt == 0), stop=(dt == DT - 1))
            vtv = vt[t][:].rearrange("p (h c) -> p h c", c=DH + 1)
            nc.vector.tensor_copy(out=vtv[:, :, 0:DH],
                                  in_=pv[:].rearrange("p (h c) -> p h c", c=DH))

        # --- halo-dependent work (waits on previous layer's AllGather)
        yh_ = yh  # LN1 over halo tokens
        emit_ln([(lambda dt: xh[:, dt, :], lambda dt: yh_[:, dt, :], HALF)], s1, b1_)
        krh = lp.tile([P, DT, HALF], MDT, tag="krh")
        for do in range(DT):
            pk = ps_a.tile([P, CHUNK], F32, tag="ps_a")
            for dt in range(DT):
                nc.tensor.matmul(out=pk[:, :HALF], lhsT=wk_r[:, dt, ts(do, P)],
                                 rhs=yh[:, dt, :], start=(dt == 0), stop=(dt == DT - 1))
            nc.vector.tensor_copy(out=krh[:, do, :], in_=pk[:, :HALF])
        for t in range(2):
            pv = ps_a.tile([P, D], F32, tag="ps_a")
            for dt in range(DT):
                nc.tensor.matmul(out=pv[:], lhsT=yh[:, dt, ts(t, P)], rhs=wv_r[:, dt, :],
                                 start=(dt == 0), stop=(dt == DT - 1))
            vtv = vt[t][:].rearrange("p (h c) -> p h c", c=DH + 1)
            nc.vector.tensor_copy(out=vtv[:, :, 0:DH],
                                  in_=pv[:].rearrange("p (h c) -> p h c", c=DH))

        # --- sliding-window attention, deferred softmax division
        attru = lp.tile([P, DT, CHUNK], F32, tag="attru")
        attr = lp.tile([P, DT, CHUNK], MDT, tag="attr")

        def kslice(kt, r0, dto):
            if kt < 2:
                return krh[ds(r0, DH), dto, ts(kt, P)]
            return kro[ds(r0, DH), dto, ts(kt - 2, P)]

        for qB in range(2):
            for h in range(H):
                r0 = (h % 2) * DH
                dto = h // 2
                pa = ps_c.tile([DH + 1, 256], F32, tag="ps_c")
                for jp in range(2):
                    pscore = ps_b.tile([P, 512], F32, tag="ps_b")
                    for jj in range(2):
                        j = jp * 2 + jj
                        kt = qB * 2 + j
                        nc.tensor.matmul(out=pscore[:, ts(jj, 256)],
                                         lhsT=kslice(kt, r0, dto),
                                         rhs=qr[ds(r0, DH), dto, ds(qB * 256, 256)],
                                         start=True, stop=False)
                        nc.tensor.matmul(out=pscore[:, ts(jj, 256)],
                                         lhsT=identm[:], rhs=masks[:, qB, j, :],
                                         start=False, stop=True)
                    ej = lp3.tile([P, 512], MDT, tag="ej", bufs=3)
                    nc.scalar.activation(ej[:], pscore[:], AF.Exp, bias=negb[:], scale=SCALE)
                    for jj in range(2):
                        j = jp * 2 + jj
                        kt = qB * 2 + j
                        nc.tensor.matmul(out=pa[:], lhsT=vt[kt][:, ds(h * (DH + 1), DH + 1)],
                                         rhs=ej[:, ts(jj, 256)], start=(j == 0), stop=(j == 3))
                nc.vector.tensor_copy(out=attru[ds(r0, DH), dto, ds(qB * 256, 256)],
                                      in_=pa[0:DH, :])
                dent = lp3.tile([1, 256], F32, tag="dent", bufs=2)
                nc.vector.tensor_copy(out=dent[:], in_=pa[DH:DH + 1, :])
                rf = lp3.tile([1, 256], F32, tag="rf", bufs=2)
                nc.vector.reciprocal_approx_fast(out=rf[:], in_=dent[:])
                if l == 0 and "d_rf" in io:
                    nc.sync.dma_start(out=io["d_rf"].ap()[qB * 8 + h:qB * 8 + h + 1, :], in_=rf[:])
                psc = ps_c.tile([DH, 256], F32, tag="ps_c")
                nc.tensor.matmul(out=psc[:], lhsT=ones_f[0:1, 0:DH], rhs=rf[:],
                                 start=True, stop=True)
                nc.vector.tensor_mul(out=attr[ds(r0, DH), dto, ds(qB * 256, 256)],
                                     in0=attru[ds(r0, DH), dto, ds(qB * 256, 256)],
                                     in1=psc[:])

        if l == 0 and "d_y" in io:
            nc.sync.dma_start(out=io["d_y"].ap().rearrange("(t p) m -> p t m", p=P), in_=y[:])
            nc.sync.dma_start(out=io["d_yh"].ap().rearrange("(t p) m -> p t m", p=P), in_=yh[:])
            nc.sync.dma_start(out=io["d_attru"].ap().rearrange("(t p) m -> p t m", p=P), in_=attru[:])
            nc.sync.dma_start(out=io["d_attr"].ap().rearrange("(t p) m -> p t m", p=P), in_=attr[:])

        # --- output projection + residual
        wo_r = load_w(io["wo"].ap()[li].rearrange("(t p) m -> p t m", p=P), "wr", [P, DT, D])
        for do in range(DT):
            po = ps_a.tile([P, CHUNK], F32, tag="ps_a")
            for dt in range(DT):
                nc.tensor.matmul(out=po[:], lhsT=wo_r[:, dt, ts(do, P)],
                                 rhs=attr[:, dt, :], start=(dt == 0), stop=(dt == DT - 1))
            nc.vector.tensor_add(out=x[:, do, :], in0=x[:, do, :], in1=po[:])

        # --- LN2 + MLP
        y2 = lp.tile([P, DT, CHUNK], MDT, tag="y2")
        emit_ln([(lambda dt: x[:, dt, :], lambda dt: y2[:, dt, :], CHUNK)],
                lambda dt: l2s[:, li, dt:dt + 1], lambda dt: l2b[:, li, dt:dt + 1])

        pb = [ps_b.tile([P, CHUNK], F32, tag="ps_b", name=f"pb{i}") for i in range(DT)]
        w1r = lp.tile([P, DT, MLPD], MDT, tag="w1r")
        nc.sync.dma_start(out=w1r[:], in_=io["w1"].ap()[li].rearrange("(t p) m -> p t m", p=P))
        w2r = lp.tile([P, MT, D], MDT, tag="w2r")
        nc.sync.dma_start(out=w2r[:], in_=io["w2"].ap()[li].rearrange("(t p) m -> p t m", p=P))

        def emit_mlp2(m, hm):
            for do in range(DT):
                nc.tensor.matmul(out=pb[do][:], lhsT=w2r[:, m, ts(do, P)],
                                 rhs=hm[:], start=(m == 0), stop=(m == MT - 1))

        hist = []
        for m in range(MT):
            p1 = ps_a.tile([P, CHUNK], F32, tag="ps_a")
            for dt in range(DT):
                nc.tensor.matmul(out=p1[:], lhsT=w1r[:, dt, ts(m, P)],
                                 rhs=y2[:, dt, :], start=(dt == 0), stop=(dt == DT - 1))
            hm = lp3.tile([P, CHUNK], MDT, tag="hm", bufs=3)
            nc.scalar.activation(hm[:], p1[:], AF.Gelu_apprx_tanh,
                                 bias=b1t[:, li, m:m + 1], scale=1.0)
            hist.append((m, hm))
            if len(hist) > 2:
                emit_mlp2(*hist.pop(0))
        for mm_, hh_ in hist:
            emit_mlp2(mm_, hh_)
        # residual (+b2), then send halo for next layer
        for do in range(DT):
            nc.vector.scalar_tensor_tensor(
                out=x[:, do, :], in0=pb[do][:],
                scalar=b2t[:, li, do:do + 1], in1=x[:, do, :],
                op0=OP.add, op1=OP.add)
        if l == 0 and "d_x1" in io:
            xd = lp3.tile([P, DT, CHUNK], F32, tag="xdump", bufs=1)
            nc.vector.tensor_copy(out=xd[:], in_=x[:])
            nc.sync.dma_start(out=io["d_x1"].ap().rearrange("(t p) m -> p t m", p=P), in_=xd[:])
        if l < NL - 1:
            agin = drp.tile([D, HALF], F32R, tag=f"agin{l}")
            agout = drp.tile([len(GROUPS[0]) * D, HALF], F32R, tag=f"agout{l}")
            nc.sync.dma_start(out=agin[:].rearrange("(t p) m -> p t m", p=P),
                              in_=x[:, :, ds(HALF, HALF)])
            nc.gpsimd.collective_compute(
                "AllGather", OP.bypass, replica_groups=GROUPS,
                ins=[agin.opt()], outs=[agout.opt()])
            for dt in range(DT):
                nc.gpsimd.indirect_dma_start(
                    out=xh[:, dt, :], out_offset=None, in_=agout[:],
                    in_offset=bass.IndirectOffsetOnAxis(ap=hoffs[:, dt:dt + 1], axis=0))

    # ------------------------------------------------ final LN + logits
    # Each core computes the FULL vocab for its own 512 tokens: no final
    # AllGather; w_out streams tile-by-tile from DRAM, prefetched by the
    # pool double-buffering. Output bias is added host-side.
    yf = lp.tile([P, DT, CHUNK], MDT, tag="y")
    emit_ln([(lambda dt: x[:, dt, :], lambda dt: yf[:, dt, :], CHUNK)],
            lambda dt: lfs[:, dt:dt + 1], lambda dt: lfb[:, dt:dt + 1])

    vtp.release()
    lp3.release()
    tp.release()

    ps_c.release()
    ps_b.release()
    fps = tc.alloc_tile_pool(name="fps", bufs=4, space="PSUM")
    with tc.tile_pool(name="ftrans", bufs=3) as ftp, \
         tc.tile_pool(name="fout", bufs=3) as fop:
        for g in range(VG):
            fwr = ftp.tile([P, GL, DT, P], MDT, tag="fwr")
            nc.sync.dma_start(out=fwr[:],
                              in_=io["w_tiles"].ap()[g]
                              .rearrange("p (j t q) -> p j t q", j=GL, t=DT))
            ot = fop.tile([P, GL, CHUNK], F16, tag="fot")
            for j in range(GL):
                pf = fps.tile([P, CHUNK], F32, tag="fps")
                for dt in range(DT):
                    nc.tensor.matmul(out=pf[:], lhsT=fwr[:, j, dt, :], rhs=yf[:, dt, :],
                                     start=(dt == 0), stop=(dt == DT - 1))
                if (g * GL + j) % 2 == 0:
                    nc.vector.tensor_copy(out=ot[:, j, :], in_=pf[:])
                else:
                    nc.scalar.activation(ot[:, j, :], pf[:], AF.Copy)
            nc.sync.dma_start(out=io["out"].ap()[g],
                              in_=ot[:].rearrange("p j m -> p (j m)"))

    fps.release()
    lp.release()
    drp.release()
    ps_a.release()
    xpool.release()
    cpool.release()


# ================================================================ host side
def _pe_table():
    pos = np.arange(S, dtype=np.float32)[:, None]
    div = np.exp(np.arange(0, D, 2, dtype=np.float32) * -(np.log(10000.0) / D))
    pe = np.zeros((S, D), dtype=np.float32)
    pe[:, 0::2] = np.sin(pos * div)
    pe[:, 1::2] = np.cos(pos * div)
    return pe


def _in_maps(inputs):
    inp = np.asarray(inputs["inputs"]).astype(np.int32)
    ids = np.pad(inp, ((0, 0), (1, 0)))[:, :-1].astype(np.int32)
    pe = _pe_table()
    wout = np.asarray(inputs["w_out"], dtype=np.float32).astype(np.float16)
    shared = {k: np.ascontiguousarray(np.asarray(inputs[k], dtype=np.float32))
              for k in ("embed", "b1", "b2", "ln1_s", "ln1_b", "ln2_s", "ln2_b")}
    for k in ("wq", "wk", "wv", "wo", "w1", "w2"):
        shared[k] = np.ascontiguousarray(
            np.asarray(inputs[k], dtype=np.float32).astype(np.float16))
    shared["lnf_s"] = np.asarray(inputs["lnf_s"], np.float32).reshape(1, D)
    shared["lnf_b"] = np.asarray(inputs["lnf_b"], np.float32).reshape(1, D)
    # w_tiles[g, p, ((j*DT+dt)*P)+q] = w_out[dt*128+p, (g*GL+j)*128+q]
    shared["w_tiles"] = np.ascontiguousarray(
        wout.reshape(DT, P, VG, GL, P).transpose(2, 1, 3, 0, 4)
        .reshape(VG, P, GL * DT * P))

    maps = []
    qi = np.arange(256)[None, :]
    ki = np.arange(P)[:, None]
    for c in range(NCORES):
        b, ch = divmod(c, NCORES // B)
        t0 = ch * CHUNK
        lo = t0 - HALF
        ids768 = np.zeros(W, np.int32)
        pe768 = np.zeros((W, D), np.float32)
        s0 = max(0, lo)
        ids768[s0 - lo:] = ids[b, s0:t0 + CHUNK]
        pe768[s0 - lo:] = pe[s0:t0 + CHUNK]
        m = np.zeros((2, 4, P, 256), np.float16)
        for qB in range(2):
            for j in range(4):
                w = 256 + qi - (j * P + ki)      # u_q - u_k
                ok = (w >= 0) & (w <= HALF)
                if ch == 0:
                    ok = ok & ((lo + qB * 256 + j * P + ki) >= 0)
                m[qB, j] = np.where(ok, 0.0, MASK_BIAS).astype(np.float16)
        src = ch - 1 if ch > 0 else 0
        hoffs = (src * D + np.arange(DT)[None, :] * P
                 + np.arange(P)[:, None]).astype(np.int32)
        mp = dict(shared)
        mp.update(
            idx_in=np.ascontiguousarray(ids768.reshape(W // P, P).T),
            pe_dm=np.ascontiguousarray(pe768.T),
            masks=m, halo_offs=hoffs)
        maps.append(mp)
    return maps


def kernel(**inputs):
    nc = _CACHE.get("nc")
    if nc is None:
        nc = _build()
        _CACHE["nc"] = nc
    maps = _in_maps(inputs)
    res = run_bass_kernel_spmd(nc, maps, list(range(NCORES))).results
    bout = np.asarray(inputs["b_out"], dtype=np.float32)
    full = np.empty((NTOK, V), np.float32)
    for c in range(NCORES):
        lv = (res[c]["logits_vm"].reshape(VG, P, GL, CHUNK)
              .transpose(0, 2, 1, 3).reshape(V, CHUNK))
        full[c * CHUNK:(c + 1) * CHUNK, :] = lv.T.astype(np.float32) + bout[None, :]
    return full.reshape(B, S, V)


# revision 31
# speedup vs baseline: 1.4141x; 1.0294x over previous
"""Longformer decoder (4 layers, sliding-window causal attention) on 8 trn2 cores.

Sharding: 4096 tokens (B=2 x S=2048) split into 8 contiguous chunks of 512
(core = b*4 + chunk). Activations are kept d-major ([dim, token], dim on
partitions) so every matmul is weights-stationary with no transposes.
Attention needs a 256-token left halo of K/V per layer: layer 0 computes it
locally from the embedding gather; layers 1-3 AllGather the residual-stream
halo over 4-core groups, overlapped with the next layer's halo-independent
work (LN1/Q/K/V over own tokens). Sliding-window masking is additive (-3000)
and applied inside PSUM via an identity-matmul accumulate, so the scalar
engine's exp produces masked probabilities directly. Softmax division is
deferred: unnormalized attention output and per-(head,q) denominators are
collected, one batched reciprocal per q-block computes 1/den for all heads,
and an indicator-matrix matmul broadcasts it back over the d-major layout.
The final projection is token-sharded: each core computes the FULL vocab for
its own 512 tokens (no final AllGather); w_out streams from DRAM in 5-tile
groups, and the output bias is added host-side.
"""
import os
import sys

import numpy as np

for _p in ("/opt/trn_rl_repo", "/root/.axon_site/_ro/trn_rl_repo"):
    if os.path.isdir(_p) and _p not in sys.path:
        sys.path.insert(0, _p)

import concourse.bass as bass
import concourse.mybir as mybir
import concourse.tile as tile
from concourse import bacc
from concourse.bass import ts, ds
from concourse.bass_utils import run_bass_kernel_spmd
from concourse.masks import make_identity

F32 = mybir.dt.float32
F32R = mybir.dt.float32r
F16 = mybir.dt.float16
I32 = mybir.dt.int32
MDT = F16 if os.environ.get("KMMDT", "f16") == "f16" else F32R
AF = mybir.ActivationFunctionType
OP = mybir.AluOpType

B, S, V, D, H, NL, MLPD = 2, 2048, 32000, 512, 8, 4, 2048
DH = D // H            # 64
HALF = 256             # attention half-window (WIN // 2)
P = 128
NCORES = 8
CHUNK = 512            # own tokens per core
W = CHUNK + HALF       # 768 = halo + own
DT = D // P            # 4 d-tiles
MT = MLPD // P         # 16 mlp tiles
VN = V // P            # 250 vocab tiles (each core does full vocab x own tokens)
GL = 5                 # vocab tiles per DMA group
VG = VN // GL          # 50 groups (batched DMA: 5KB contiguous per partition)
NTOK = B * S           # 4096
GROUPS = [[0, 1, 2, 3], [4, 5, 6, 7]]
EXP_SHIFT = 2.0
MASK_BIAS = -3000.0    # additive mask; exp(SCALE*(s+MASK_BIAS)+EXP_SHIFT) == 0
SCALE = float(1.0 / np.sqrt(DH))

_CACHE = {}


# ================================================================ builder
def _build():
    nc = bacc.Bacc("TRN2", target_bir_lowering=False, debug=False,
                   num_devices=NCORES)

    ein = lambda n, sh, dt=F32: nc.dram_tensor(n, sh, dt, kind="ExternalInput")
    io = dict(
        wq=ein("wq", [NL, D, D], MDT), wk=ein("wk", [NL, D, D], MDT),
        wv=ein("wv", [NL, D, D], MDT), wo=ein("wo", [NL, D, D], MDT),
        w1=ein("w1", [NL, D, MLPD], MDT), w2=ein("w2", [NL, MLPD, D], MDT),
        b1=ein("b1", [NL, MLPD]), b2=ein("b2", [NL, D]),
        ln1_s=ein("ln1_s", [NL, D]), ln1_b=ein("ln1_b", [NL, D]),
        ln2_s=ein("ln2_s", [NL, D]), ln2_b=ein("ln2_b", [NL, D]),
        lnf_s=ein("lnf_s", [1, D]), lnf_b=ein("lnf_b", [1, D]),
        w_tiles=ein("w_tiles", [VG, P, GL * DT * P], MDT),
        embed=ein("embed", [V, D]),
        idx_in=ein("idx_in", [P, W // P], I32),
        pe_dm=ein("pe_dm", [D, W]),
        masks=ein("masks", [2, 4, P, 256], MDT),
        halo_offs=ein("halo_offs", [P, DT], I32),
        out=nc.dram_tensor("logits_vm", [VG, P, GL * CHUNK], F16, kind="ExternalOutput"),
    )
    if os.environ.get("KDEBUG") == "1":
        io["d_y"] = nc.dram_tensor("d_y", [D, CHUNK], F16, kind="ExternalOutput")
        io["d_yh"] = nc.dram_tensor("d_yh", [D, HALF], F16, kind="ExternalOutput")
        io["d_attru"] = nc.dram_tensor("d_attru", [D, CHUNK], F32, kind="ExternalOutput")
        io["d_attr"] = nc.dram_tensor("d_attr", [D, CHUNK], F16, kind="ExternalOutput")
        io["d_rf"] = nc.dram_tensor("d_rf", [16, 256], F32, kind="ExternalOutput")
        io["d_x1"] = nc.dram_tensor("d_x1", [D, CHUNK], F32, kind="ExternalOutput")
        io["d_xh"] = nc.dram_tensor("d_xh", [D, HALF], F32, kind="ExternalOutput")
        io["d_xhp"] = nc.dram_tensor("d_xhp", [D, HALF], F32, kind="ExternalOutput")
        io["d_y2h"] = nc.dram_tensor("d_y2h", [D, HALF], F16, kind="ExternalOutput")

    with tile.TileContext(nc) as tc, nc.allow_low_precision(reason="f32r rounding"):
        _emit(nc, tc, io)
    nc.compile()
    return nc


def _emit(nc, tc, io):
    cpool = tc.alloc_tile_pool(name="const", bufs=1)
    xpool = tc.alloc_tile_pool(name="xres", bufs=1)
    ps_a = tc.alloc_tile_pool(name="ps_a", bufs=2, space="PSUM")
    ps_b = tc.alloc_tile_pool(name="ps_b", bufs=4, space="PSUM")
    ps_c = tc.alloc_tile_pool(name="ps_c", bufs=2, space="PSUM")
    drp = tc.alloc_tile_pool(name="drbounce", bufs=1, space="DRAM")

    # ------------------------------------------------ constants
    ones_f = cpool.tile([P, P], F32, tag="ones_f")
    nc.vector.memset(ones_f[:], 1.0)
    ones = cpool.tile([P, P], MDT, tag="ones")
    nc.vector.tensor_copy(out=ones[:], in_=ones_f[:])
    ones_r = cpool.tile([P, P], F32R, tag="ones_r")
    nc.vector.tensor_copy(out=ones_r[:], in_=ones_f[:])
    identm = cpool.tile([P, P], MDT, tag="identm")
    make_identity(nc, identm[:])
    negb = cpool.tile([P, 1], F32, tag="negb")
    nc.vector.memset(negb[:], EXP_SHIFT)
    epsb = cpool.tile([P, 1], F32, tag="epsb")
    nc.vector.memset(epsb[:], 1e-6)
    l1s = cpool.tile([P, NL, DT], F32, tag="l1s")
    l1b = cpool.tile([P, NL, DT], F32, tag="l1b")
    l2s = cpool.tile([P, NL, DT], F32, tag="l2s")
    l2b = cpool.tile([P, NL, DT], F32, tag="l2b")
    lfs = cpool.tile([P, DT], F32, tag="lfs")
    lfb = cpool.tile([P, DT], F32, tag="lfb")
    nc.sync.dma_start(out=l1s[:], in_=io["ln1_s"].ap().rearrange("l (t p) -> p l t", p=P))
    nc.sync.dma_start(out=l1b[:], in_=io["ln1_b"].ap().rearrange("l (t p) -> p l t", p=P))
    nc.sync.dma_start(out=l2s[:], in_=io["ln2_s"].ap().rearrange("l (t p) -> p l t", p=P))
    nc.sync.dma_start(out=l2b[:], in_=io["ln2_b"].ap().rearrange("l (t p) -> p l t", p=P))
    nc.sync.dma_start(out=lfs[:], in_=io["lnf_s"].ap().rearrange("o (t p) -> p (o t)", p=P))
    nc.sync.dma_start(out=lfb[:], in_=io["lnf_b"].ap().rearrange("o (t p) -> p (o t)", p=P))
    b1t = cpool.tile([P, NL, MT], F32, tag="b1t")
    b2t = cpool.tile([P, NL, DT], F32, tag="b2t")
    nc.sync.dma_start(out=b1t[:], in_=io["b1"].ap().rearrange("l (m p) -> p l m", p=P))
    nc.sync.dma_start(out=b2t[:], in_=io["b2"].ap().rearrange("l (t p) -> p l t", p=P))
    masks = cpool.tile([P, 2, 4, 256], MDT, tag="masks")
    nc.sync.dma_start(out=masks[:], in_=io["masks"].ap().rearrange("a b p q -> p a b q"))
    hoffs = cpool.tile([P, DT], I32, tag="hoffs")
    nc.sync.dma_start(out=hoffs[:], in_=io["halo_offs"].ap())

    # residual stream (own 512 tokens, d-major, f32r so LN sum-matmuls can
    # consume it directly at full PE rate) + per-layer halo
    x = xpool.tile([P, DT, CHUNK], F32R, tag="x")
    xh = xpool.tile([P, DT, HALF], F32R, tag="xh")

    # ------------------------------------------------ embedding
    with tc.tile_pool(name="embed", bufs=1) as epool:
        ident = epool.tile([P, P], F32, tag="ident")
        make_identity(nc, ident[:])
        pe = epool.tile([P, DT, W], F32, tag="pe")
        nc.sync.dma_start(out=pe[:], in_=io["pe_dm"].ap().rearrange("(t p) m -> p t m", p=P))
        idxt = epool.tile([P, W // P], I32, tag="idxt")
        nc.sync.dma_start(out=idxt[:], in_=io["idx_in"].ap())
        with tc.tile_pool(name="gath", bufs=2) as gpool:
            for g in range(W // P):
                gt = gpool.tile([P, D], F32, tag="gt")
                nc.gpsimd.indirect_dma_start(
                    out=gt[:], out_offset=None, in_=io["embed"].ap(),
                    in_offset=bass.IndirectOffsetOnAxis(ap=idxt[:, g:g + 1], axis=0),
                )
                for dt in range(DT):
                    pt = ps_a.tile([P, P], F32, tag="ps_a")
                    nc.tensor.transpose(pt[:], gt[:, ts(dt, P)], ident[:])
                    dst = xh[:, dt, ts(g, P)] if g < 2 else x[:, dt, ts(g - 2, P)]
                    nc.vector.tensor_add(out=dst, in0=pt[:], in1=pe[:, dt, ts(g, P)])

    # ------------------------------------------------ layer pools
    lp = tc.alloc_tile_pool(name="layers", bufs=1)
    tp = tc.alloc_tile_pool(name="ltrans", bufs=2)
    lp3 = tc.alloc_tile_pool(name="ltrans3", bufs=3)
    vtp = tc.alloc_tile_pool(name="vtpool", bufs=1)

    # V tiles with a trailing ones column per head: PV matmul row DH
    # accumulates the softmax denominator for free. Ones written once.
    vt = [vtp.tile([P, H * (DH + 1)], MDT, tag=f"vt{t}", name=f"vt{t}")
          for t in range(W // P)]
    for t in range(W // P):
        vtv = vt[t][:].rearrange("p (h c) -> p h c", c=DH + 1)
        nc.vector.tensor_copy(out=vtv[:, :, DH:DH + 1], in_=ones[:, 0:H])

    def emit_ln(blocks, s_of, b_of, xdt=F32R):
        """LN over d (partition axis x DT). blocks: list of
        (src_fn(dt)->AP[128,width], dst_fn(dt)->AP[128,width], width<=512)."""
        ones_x = ones_r if xdt == F32R else ones
        for fn, dst, width in blocks:
            sx = ps_a.tile([1, 512], F32, tag="ps_a")
            sxx = ps_a.tile([1, 512], F32, tag="ps_a")
            for dt in range(DT):
                xsq = lp3.tile([P, 512], MDT, tag="ln_xsq", bufs=2)
                nc.vector.tensor_mul(out=xsq[:, :width], in0=fn(dt), in1=fn(dt))
                nc.tensor.matmul(out=sx[:, :width], lhsT=ones_x[:, 0:1], rhs=fn(dt),
                                 start=(dt == 0), stop=(dt == DT - 1))
                nc.tensor.matmul(out=sxx[:, :width], lhsT=ones[:, 0:1], rhs=xsq[:, :width],
                                 start=(dt == 0), stop=(dt == DT - 1))
            mu = lp3.tile([1, 512], MDT, tag="ln_mu", bufs=2)
            nc.vector.tensor_scalar_mul(out=mu[:, :width], in0=sx[:, :width], scalar1=1.0 / D)
            mu2 = lp3.tile([1, 512], F32, tag="ln_mu2", bufs=1)
            nc.vector.tensor_mul(out=mu2[:, :width], in0=mu[:, :width], in1=mu[:, :width])
            var = lp3.tile([1, 512], F32, tag="ln_var", bufs=1)
            # var = sxx/D - mu^2
            nc.vector.scalar_tensor_tensor(
                out=var[:, :width], in0=sxx[:, :width], scalar=1.0 / D,
                in1=mu2[:, :width], op0=OP.mult, op1=OP.subtract)
            sd = lp3.tile([1, 512], F32, tag="ln_sd", bufs=1)
            nc.scalar.activation(sd[:, :width], var[:, :width], AF.Sqrt, bias=epsb[0:1, :], scale=1.0)
            rt = lp3.tile([1, 512], F32, tag="ln_rt", bufs=1)
            nc.vector.reciprocal_approx_fast(out=rt[:, :width], in_=sd[:, :width])
            rstd = lp3.tile([1, 512], MDT, tag="ln_rstd", bufs=2)
            nc.vector.tensor_copy(out=rstd[:, :width], in_=rt[:, :width])
            pmu = ps_c.tile([P, 512], F32, tag="ps_c")
            nc.tensor.matmul(out=pmu[:, :width], lhsT=ones[0:1, :], rhs=mu[:, :width],
                             start=True, stop=True)
            prs = ps_c.tile([P, 512], F32, tag="ps_c")
            nc.tensor.matmul(out=prs[:, :width], lhsT=ones[0:1, :], rhs=rstd[:, :width],
                             start=True, stop=True)
            for dt in range(DT):
                scr = lp3.tile([P, 512], F32, tag="ln_scr", bufs=2)
                nc.vector.tensor_sub(out=scr[:, :width], in0=fn(dt), in1=pmu[:, :width])
                nc.vector.tensor_mul(out=scr[:, :width], in0=scr[:, :width], in1=prs[:, :width])
                nc.vector.tensor_scalar(out=dst(dt), in0=scr[:, :width],
                                        scalar1=s_of(dt), scalar2=b_of(dt),
                                        op0=OP.mult, op1=OP.add)

    def load_w(dram_ap, tag_r, shape3, rpool=None):
        wr = (rpool or tp).tile(shape3, MDT, tag=tag_r)
        nc.sync.dma_start(out=wr[:], in_=dram_ap)
        return wr

    # ------------------------------------------------ transformer layers
    _knl = int(os.environ.get("KNL", NL))
    xh_pre = xpool.tile([P, DT, HALF], F32R, tag="xh_pre")
    wmlp = {}
    for l in range(_knl):
        li = l % NL
        lpv = (l - 1) % NL
        s1 = lambda dt: l1s[:, li, dt:dt + 1]
        b1_ = lambda dt: l1b[:, li, dt:dt + 1]
        y = lp.tile([P, DT, CHUNK], MDT, tag="y")
        yh = lp.tile([P, DT, HALF], MDT, tag="yh")
        krh = lp.tile([P, DT, HALF], MDT, tag="krh")
        # LN1 over own tokens first: independent of the halo AllGather.
        emit_ln([(lambda dt: x[:, dt, :], lambda dt: y[:, dt, :], CHUNK)], s1, b1_)

        # --- projections (weights stationary, d-major out), own tokens
        wq_r = load_w(io["wq"].ap()[li].rearrange("(t p) m -> p t m", p=P), "wr", [P, DT, D])
        qr = lp.tile([P, DT, CHUNK], MDT, tag="qr")
        for do in range(DT):
            pq = ps_a.tile([P, CHUNK], F32, tag="ps_a")
            for dt in range(DT):
                nc.tensor.matmul(out=pq[:], lhsT=wq_r[:, dt, ts(do, P)],
                                 rhs=y[:, dt, :], start=(dt == 0), stop=(dt == DT - 1))
            nc.vector.tensor_copy(out=qr[:, do, :], in_=pq[:])

        wk_r = load_w(io["wk"].ap()[li].rearrange("(t p) m -> p t m", p=P), "wr", [P, DT, D])
        kro = lp.tile([P, DT, CHUNK], MDT, tag="kro")
        for do in range(DT):
            pk = ps_a.tile([P, CHUNK], F32, tag="ps_a")
            for dt in range(DT):
                nc.tensor.matmul(out=pk[:], lhsT=wk_r[:, dt, ts(do, P)],
                                 rhs=y[:, dt, :], start=(dt == 0), stop=(dt == DT - 1))
            nc.vector.tensor_copy(out=kro[:, do, :], in_=pk[:])

        wv_r = load_w(io["wv"].ap()[li].rearrange("(t p) m -> p t m", p=P), "wr", [P, DT, D])
        for t in range(2, W // P):
            pv = ps_a.tile([P, D], F32, tag="ps_a")
            for dt in range(DT):
                nc.tensor.matmul(out=pv[:], lhsT=y[:, dt, ts(t - 2, P)], rhs=wv_r[:, dt, :],
                                 start=(dt == 0), stop=(dt == DT - 1))
            vtv = vt[t][:].rearrange("p (h c) -> p h c", c=DH + 1)
            nc.vector.tensor_copy(out=vtv[:, :, 0:DH],
                                  in_=pv[:].rearrange("p (h c) -> p h c", c=DH))

        # --- sliding-window attention, deferred softmax division
        attru = lp.tile([P, DT, CHUNK], F32, tag="attru")
        attr = lp.tile([P, DT, CHUNK], MDT, tag="attr")

        def kslice(kt, r0, dto):
            if kt < 2:
                return krh[ds(r0, DH), dto, ts(kt, P)]
            return kro[ds(r0, DH), dto, ts(kt - 2, P)]

        def attn_qblock(qB):
            for h in range(H):
                r0 = (h % 2) * DH
                dto = h // 2
                pa = ps_c.tile([DH + 1, 256], F32, tag="ps_c")
                for jp in range(2):
                    pscore = ps_b.tile([P, 512], F32, tag="ps_b")
                    for jj in range(2):
                        j = jp * 2 + jj
                        kt = qB * 2 + j
                        nc.tensor.matmul(out=pscore[:, ts(jj, 256)],
                                         lhsT=kslice(kt, r0, dto),
                                         rhs=qr[ds(r0, DH), dto, ds(qB * 256, 256)],
                                         start=True, stop=False)
                        nc.tensor.matmul(out=pscore[:, ts(jj, 256)],
                                         lhsT=identm[:], rhs=masks[:, qB, j, :],
                                         start=False, stop=True)
                    ej = lp3.tile([P, 512], MDT, tag="ej", bufs=3)
                    nc.scalar.activation(ej[:], pscore[:], AF.Exp, bias=negb[:], scale=SCALE)
                    for jj in range(2):
                        j = jp * 2 + jj
                        kt = qB * 2 + j
                        nc.tensor.matmul(out=pa[:], lhsT=vt[kt][:, ds(h * (DH + 1), DH + 1)],
                                         rhs=ej[:, ts(jj, 256)], start=(j == 0), stop=(j == 3))
                nc.vector.tensor_copy(out=attru[ds(r0, DH), dto, ds(qB * 256, 256)],
                                      in_=pa[0:DH, :])
                dent = lp3.tile([1, 256], F32, tag="dent", bufs=2)
                nc.vector.tensor_copy(out=dent[:], in_=pa[DH:DH + 1, :])
                rf = lp3.tile([1, 256], F32, tag="rf", bufs=2)
                nc.vector.reciprocal_approx_fast(out=rf[:], in_=dent[:])
                rfh = lp3.tile([1, 256], MDT, tag="rfh", bufs=2)
                nc.vector.tensor_copy(out=rfh[:], in_=rf[:])
                if l == 0 and "d_rf" in io:
                    nc.sync.dma_start(out=io["d_rf"].ap()[qB * 8 + h:qB * 8 + h + 1, :], in_=rf[:])
                psc = ps_c.tile([DH, 256], F32, tag="ps_c")
                nc.tensor.matmul(out=psc[:], lhsT=ones[0:1, 0:DH], rhs=rfh[:],
                                 start=True, stop=True)
                nc.vector.tensor_mul(out=attr[ds(r0, DH), dto, ds(qB * 256, 256)],
                                     in0=attru[ds(r0, DH), dto, ds(qB * 256, 256)],
                                     in1=psc[:])

        # q-block 1 attends only to own keys: runs while the halo is in flight
        attn_qblock(1)

        # --- halo recompute (layers >= 1): the AllGather shipped the
        # PRE-MLP residual halo; apply the previous layer's LN2+MLP here.
        if l >= 1:
            if l == 1 and "d_xhp" in io:
                xhpd = lp3.tile([P, DT, HALF], F32, tag="xhpd", bufs=1)
                nc.vector.tensor_copy(out=xhpd[:], in_=xh_pre[:])
                nc.sync.dma_start(out=io["d_xhp"].ap().rearrange("(t p) m -> p t m", p=P), in_=xhpd[:])
            y2h = lp.tile([P, DT, HALF], MDT, tag="y2h")
            emit_ln([(lambda dt: xh_pre[:, dt, :], lambda dt: y2h[:, dt, :], HALF)],
                    lambda dt: l2s[:, lpv, dt:dt + 1], lambda dt: l2b[:, lpv, dt:dt + 1])
            if l == 1 and "d_y2h" in io:
                nc.sync.dma_start(out=io["d_y2h"].ap().rearrange("(t p) m -> p t m", p=P), in_=y2h[:])
            w1p, w2p = wmlp[(l - 1) % 2]
            pbh = [ps_b.tile([P, CHUNK], F32, tag="ps_b", name=f"pbh{i}") for i in range(DT)]

            def emit_mlp2h(m, hmh):
                for do in range(DT):
                    nc.tensor.matmul(out=pbh[do][:, :HALF],
                                     lhsT=w2p[:, m, ts(do, P)], rhs=hmh[:],
                                     start=(m == 0), stop=(m == MT - 1))

            histh = []
            for m in range(MT):
                p1h = ps_a.tile([P, CHUNK], F32, tag="ps_a")
                for dt in range(DT):
                    nc.tensor.matmul(out=p1h[:, :HALF], lhsT=w1p[:, dt, ts(m, P)],
                                     rhs=y2h[:, dt, :], start=(dt == 0), stop=(dt == DT - 1))
                hmh = lp3.tile([P, HALF], MDT, tag="hmh", bufs=3)
                nc.scalar.activation(hmh[:], p1h[:, :HALF], AF.Gelu_apprx_tanh,
                                     bias=b1t[:, lpv, m:m + 1], scale=1.0)
                histh.append((m, hmh))
                if len(histh) > 2:
                    emit_mlp2h(*histh.pop(0))
            for mm_, hh_ in histh:
                emit_mlp2h(mm_, hh_)
            for do in range(DT):
                nc.vector.scalar_tensor_tensor(
                    out=xh[:, do, :], in0=pbh[do][:, :HALF],
                    scalar=b2t[:, lpv, do:do + 1], in1=xh_pre[:, do, :],
                    op0=OP.add, op1=OP.add)

        if l == 1 and "d_xh" in io:
            xhd = lp3.tile([P, DT, HALF], F32, tag="xhd", bufs=1)
            nc.vector.tensor_copy(out=xhd[:], in_=xh[:])
            nc.sync.dma_start(out=io["d_xh"].ap().rearrange("(t p) m -> p t m", p=P), in_=xhd[:])

        # --- LN1 over halo + K/V halo
        emit_ln([(lambda dt: xh[:, dt, :], lambda dt: yh[:, dt, :], HALF)], s1, b1_)
        for do in range(DT):
            pk = ps_a.tile([P, CHUNK], F32, tag="ps_a")
            for dt in range(DT):
                nc.tensor.matmul(out=pk[:, :HALF], lhsT=wk_r[:, dt, ts(do, P)],
                                 rhs=yh[:, dt, :], start=(dt == 0), stop=(dt == DT - 1))
            nc.vector.tensor_copy(out=krh[:, do, :], in_=pk[:, :HALF])
        for t in range(2):
            pv = ps_a.tile([P, D], F32, tag="ps_a")
            for dt in range(DT):
                nc.tensor.matmul(out=pv[:], lhsT=yh[:, dt, ts(t, P)], rhs=wv_r[:, dt, :],
                                 start=(dt == 0), stop=(dt == DT - 1))
            vtv = vt[t][:].rearrange("p (h c) -> p h c", c=DH + 1)
            nc.vector.tensor_copy(out=vtv[:, :, 0:DH],
                                  in_=pv[:].rearrange("p (h c) -> p h c", c=DH))

        attn_qblock(0)

        if l == 0 and "d_y" in io:
            nc.sync.dma_start(out=io["d_y"].ap().rearrange("(t p) m -> p t m", p=P), in_=y[:])
            nc.sync.dma_start(out=io["d_yh"].ap().rearrange("(t p) m -> p t m", p=P), in_=yh[:])
            nc.sync.dma_start(out=io["d_attru"].ap().rearrange("(t p) m -> p t m", p=P), in_=attru[:])
            nc.sync.dma_start(out=io["d_attr"].ap().rearrange("(t p) m -> p t m", p=P), in_=attr[:])

        # --- output projection + residual
        wo_r = load_w(io["wo"].ap()[li].rearrange("(t p) m -> p t m", p=P), "wr", [P, DT, D])
        for do in range(DT):
            po = ps_a.tile([P, CHUNK], F32, tag="ps_a")
            for dt in range(DT):
                nc.tensor.matmul(out=po[:], lhsT=wo_r[:, dt, ts(do, P)],
                                 rhs=attr[:, dt, :], start=(dt == 0), stop=(dt == DT - 1))
            nc.vector.tensor_add(out=x[:, do, :], in0=x[:, do, :], in1=po[:])

        # --- ship the PRE-MLP halo now; receiver recomputes its MLP.
        # This hides the AllGather under our MLP + the next layer's
        # halo-independent work.
        if l < NL - 1:
            xhs = lp.tile([P, DT, HALF], F32R, tag="xhs")
            nc.vector.tensor_copy(out=xhs[:], in_=x[:, :, ds(HALF, HALF)])
            agin = drp.tile([D, HALF], F32R, tag=f"agin{l}")
            agout = drp.tile([len(GROUPS[0]) * D, HALF], F32R, tag=f"agout{l}")
            nc.sync.dma_start(out=agin[:].rearrange("(t p) m -> p t m", p=P),
                              in_=xhs[:])
            nc.gpsimd.collective_compute(
                "AllGather", OP.bypass, replica_groups=GROUPS,
                ins=[agin.opt()], outs=[agout.opt()])
            for dt in range(DT):
                nc.gpsimd.indirect_dma_start(
                    out=xh_pre[:, dt, :], out_offset=None, in_=agout[:],
                    in_offset=bass.IndirectOffsetOnAxis(ap=hoffs[:, dt:dt + 1], axis=0))

        # --- LN2 + MLP
        y2 = lp.tile([P, DT, CHUNK], MDT, tag="y2")
        emit_ln([(lambda dt: x[:, dt, :], lambda dt: y2[:, dt, :], CHUNK)],
                lambda dt: l2s[:, li, dt:dt + 1], lambda dt: l2b[:, li, dt:dt + 1])

        pb = [ps_b.tile([P, CHUNK], F32, tag="ps_b", name=f"pb{i}") for i in range(DT)]
        w1r = lp.tile([P, DT, MLPD], MDT, tag=f"w1r{l % 2}")
        nc.sync.dma_start(out=w1r[:], in_=io["w1"].ap()[li].rearrange("(t p) m -> p t m", p=P))
        w2r = lp.tile([P, MT, D], MDT, tag=f"w2r{l % 2}")
        nc.sync.dma_start(out=w2r[:], in_=io["w2"].ap()[li].rearrange("(t p) m -> p t m", p=P))
        wmlp[l % 2] = (w1r, w2r)

        def emit_mlp2(m, hm):
            for do in range(DT):
                nc.tensor.matmul(out=pb[do][:], lhsT=w2r[:, m, ts(do, P)],
                                 rhs=hm[:], start=(m == 0), stop=(m == MT - 1))

        hist = []
        for m in range(MT):
            p1 = ps_a.tile([P, CHUNK], F32, tag="ps_a")
            for dt in range(DT):
                nc.tensor.matmul(out=p1[:], lhsT=w1r[:, dt, ts(m, P)],
                                 rhs=y2[:, dt, :], start=(dt == 0), stop=(dt == DT - 1))
            hm = lp3.tile([P, CHUNK], MDT, tag="hm", bufs=3)
            nc.scalar.activation(hm[:], p1[:], AF.Gelu_apprx_tanh,
                                 bias=b1t[:, li, m:m + 1], scale=1.0)
            hist.append((m, hm))
            if len(hist) > 2:
                emit_mlp2(*hist.pop(0))
        for mm_, hh_ in hist:
            emit_mlp2(mm_, hh_)
        # residual (+b2)
        for do in range(DT):
            nc.vector.scalar_tensor_tensor(
                out=x[:, do, :], in0=pb[do][:],
                scalar=b2t[:, li, do:do + 1], in1=x[:, do, :],
                op0=OP.add, op1=OP.add)
        if l == 0 and "d_x1" in io:
            xd = lp3.tile([P, DT, CHUNK], F32, tag="xdump", bufs=1)
            nc.vector.tensor_copy(out=xd[:], in_=x[:])
            nc.sync.dma_start(out=io["d_x1"].ap().rearrange("(t p) m -> p t m", p=P), in_=xd[:])

    # ------------------------------------------------ final LN + logits
    # Each core computes the FULL vocab for its own 512 tokens: no final
    # AllGather; w_out streams tile-by-tile from DRAM, prefetched by the
    # pool double-buffering. Output bias is added host-side.
    yf = lp.tile([P, DT, CHUNK], MDT, tag="y")
    emit_ln([(lambda dt: x[:, dt, :], lambda dt: yf[:, dt, :], CHUNK)],
            lambda dt: lfs[:, dt:dt + 1], lambda dt: lfb[:, dt:dt + 1])

    vtp.release()
    lp3.release()
    tp.release()

    ps_c.release()
    ps_b.release()
    fps = tc.alloc_tile_pool(name="fps", bufs=4, space="PSUM")
    with tc.tile_pool(name="ftrans", bufs=3) as ftp, \
         tc.tile_pool(name="fout", bufs=3) as fop:
        for g in range(VG):
            fwr = ftp.tile([P, GL, DT, P], MDT, tag="fwr")
            nc.sync.dma_start(out=fwr[:],
                              in_=io["w_tiles"].ap()[g]
                              .rearrange("p (j t q) -> p j t q", j=GL, t=DT))
            ot = fop.tile([P, GL, CHUNK], F16, tag="fot")
            for j in range(GL):
                pf = fps.tile([P, CHUNK], F32, tag="fps")
                for dt in range(DT):
                    nc.tensor.matmul(out=pf[:], lhsT=fwr[:, j, dt, :], rhs=yf[:, dt, :],
                                     start=(dt == 0), stop=(dt == DT - 1))
                if (g * GL + j) % 2 == 0:
                    nc.vector.tensor_copy(out=ot[:, j, :], in_=pf[:])
                else:
                    nc.scalar.activation(ot[:, j, :], pf[:], AF.Copy)
            nc.sync.dma_start(out=io["out"].ap()[g],
                              in_=ot[:].rearrange("p j m -> p (j m)"))

    fps.release()
    lp.release()
    drp.release()
    ps_a.release()
    xpool.release()
    cpool.release()


# ================================================================ host side
def _pe_table():
    pos = np.arange(S, dtype=np.float32)[:, None]
    div = np.exp(np.arange(0, D, 2, dtype=np.float32) * -(np.log(10000.0) / D))
    pe = np.zeros((S, D), dtype=np.float32)
    pe[:, 0::2] = np.sin(pos * div)
    pe[:, 1::2] = np.cos(pos * div)
    return pe


def _in_maps(inputs):
    inp = np.asarray(inputs["inputs"]).astype(np.int32)
    ids = np.pad(inp, ((0, 0), (1, 0)))[:, :-1].astype(np.int32)
    pe = _pe_table()
    wout = np.asarray(inputs["w_out"], dtype=np.float32).astype(np.float16)
    shared = {k: np.ascontiguousarray(np.asarray(inputs[k], dtype=np.float32))
              for k in ("embed", "b1", "b2", "ln1_s", "ln1_b", "ln2_s", "ln2_b")}
    for k in ("wq", "wk", "wv", "wo", "w1", "w2"):
        shared[k] = np.ascontiguousarray(
            np.asarray(inputs[k], dtype=np.float32).astype(np.float16))
    shared["lnf_s"] = np.asarray(inputs["lnf_s"], np.float32).reshape(1, D)
    shared["lnf_b"] = np.asarray(inputs["lnf_b"], np.float32).reshape(1, D)
    # w_tiles[g, p, ((j*DT+dt)*P)+q] = w_out[dt*128+p, (g*GL+j)*128+q]
    shared["w_tiles"] = np.ascontiguousarray(
        wout.reshape(DT, P, VG, GL, P).transpose(2, 1, 3, 0, 4)
        .reshape(VG, P, GL * DT * P))

    maps = []
    qi = np.arange(256)[None, :]
    ki = np.arange(P)[:, None]
    for c in range(NCORES):
        b, ch = divmod(c, NCORES // B)
        t0 = ch * CHUNK
        lo = t0 - HALF
        ids768 = np.zeros(W, np.int32)
        pe768 = np.zeros((W, D), np.float32)
        s0 = max(0, lo)
        ids768[s0 - lo:] = ids[b, s0:t0 + CHUNK]
        pe768[s0 - lo:] = pe[s0:t0 + CHUNK]
        m = np.zeros((2, 4, P, 256), np.float16)
        for qB in range(2):
            for j in range(4):
                w = 256 + qi - (j * P + ki)      # u_q - u_k
                ok = (w >= 0) & (w <= HALF)
                if ch == 0:
                    ok = ok & ((lo + qB * 256 + j * P + ki) >= 0)
                m[qB, j] = np.where(ok, 0.0, MASK_BIAS).astype(np.float16)
        src = ch - 1 if ch > 0 else 0
        hoffs = (src * D + np.arange(DT)[None, :] * P
                 + np.arange(P)[:, None]).astype(np.int32)
        mp = dict(shared)
        mp.update(
            idx_in=np.ascontiguousarray(ids768.reshape(W // P, P).T),
            pe_dm=np.ascontiguousarray(pe768.T),
            masks=m, halo_offs=hoffs)
        maps.append(mp)
    return maps


def kernel(**inputs):
    nc = _CACHE.get("nc")
    if nc is None:
        nc = _build()
        _CACHE["nc"] = nc
    maps = _in_maps(inputs)
    res = run_bass_kernel_spmd(nc, maps, list(range(NCORES))).results
    bout = np.asarray(inputs["b_out"], dtype=np.float32)
    full = np.empty((NTOK, V), np.float32)
    for c in range(NCORES):
        lv = (res[c]["logits_vm"].reshape(VG, P, GL, CHUNK)
              .transpose(0, 2, 1, 3).reshape(V, CHUNK))
        full[c * CHUNK:(c + 1) * CHUNK, :] = lv.T.astype(np.float32) + bout[None, :]
    return full.reshape(B, S, V)


# revision 34
# speedup vs baseline: 1.4358x; 1.0154x over previous
"""Longformer decoder (4 layers, sliding-window causal attention) on 8 trn2 cores.

Sharding: 4096 tokens (B=2 x S=2048) split into 8 contiguous chunks of 512
(core = b*4 + chunk). Activations are kept d-major ([dim, token], dim on
partitions) so every matmul is weights-stationary with no transposes.
Attention needs a 256-token left halo of K/V per layer: layer 0 computes it
locally from the embedding gather; layers 1-3 AllGather the residual-stream
halo over 4-core groups, overlapped with the next layer's halo-independent
work (LN1/Q/K/V over own tokens). Sliding-window masking is additive (-3000)
and applied inside PSUM via an identity-matmul accumulate, so the scalar
engine's exp produces masked probabilities directly. Softmax division is
deferred: unnormalized attention output and per-(head,q) denominators are
collected, one batched reciprocal per q-block computes 1/den for all heads,
and an indicator-matrix matmul broadcasts it back over the d-major layout.
The final projection is token-sharded: each core computes the FULL vocab for
its own 512 tokens (no final AllGather); w_out streams from DRAM in 5-tile
groups, and the output bias is added host-side.
"""
import os
import sys

import numpy as np

for _p in ("/opt/trn_rl_repo", "/root/.axon_site/_ro/trn_rl_repo"):
    if os.path.isdir(_p) and _p not in sys.path:
        sys.path.insert(0, _p)

import concourse.bass as bass
import concourse.mybir as mybir
import concourse.tile as tile
from concourse import bacc
from concourse.bass import ts, ds
from concourse.bass_utils import run_bass_kernel_spmd
from concourse.masks import make_identity

F32 = mybir.dt.float32
F32R = mybir.dt.float32r
F16 = mybir.dt.float16
I32 = mybir.dt.int32
MDT = F16 if os.environ.get("KMMDT", "f16") == "f16" else F32R
AF = mybir.ActivationFunctionType
OP = mybir.AluOpType

B, S, V, D, H, NL, MLPD = 2, 2048, 32000, 512, 8, 4, 2048
DH = D // H            # 64
HALF = 256             # attention half-window (WIN // 2)
P = 128
NCORES = 8
CHUNK = 512            # own tokens per core
W = CHUNK + HALF       # 768 = halo + own
DT = D // P            # 4 d-tiles
MT = MLPD // P         # 16 mlp tiles
VN = V // P            # 250 vocab tiles (each core does full vocab x own tokens)
GL = 5                 # vocab tiles per DMA group
VG = VN // GL          # 50 groups (batched DMA: 5KB contiguous per partition)
NTOK = B * S           # 4096
GROUPS = [[0, 1, 2, 3], [4, 5, 6, 7]]
EXP_SHIFT = 2.0
MASK_BIAS = -3000.0    # additive mask; exp(SCALE*(s+MASK_BIAS)+EXP_SHIFT) == 0
SCALE = float(1.0 / np.sqrt(DH))

_CACHE = {}


# ================================================================ builder
def _build():
    nc = bacc.Bacc("TRN2", target_bir_lowering=False, debug=False,
                   num_devices=NCORES)

    ein = lambda n, sh, dt=F32: nc.dram_tensor(n, sh, dt, kind="ExternalInput")
    io = dict(
        wq=ein("wq", [NL, D, D], MDT), wk=ein("wk", [NL, D, D], MDT),
        wv=ein("wv", [NL, D, D], MDT), wo=ein("wo", [NL, D, D], MDT),
        w1=ein("w1", [NL, D, MLPD], MDT), w2=ein("w2", [NL, MLPD, D], MDT),
        b1=ein("b1", [NL, MLPD]), b2=ein("b2", [NL, D]),
        ln1_s=ein("ln1_s", [NL, D]), ln1_b=ein("ln1_b", [NL, D]),
        ln2_s=ein("ln2_s", [NL, D]), ln2_b=ein("ln2_b", [NL, D]),
        lnf_s=ein("lnf_s", [1, D]), lnf_b=ein("lnf_b", [1, D]),
        w_tiles=ein("w_tiles", [VG, P, GL * DT * P], MDT),
        embed=ein("embed", [V, D]),
        idx_in=ein("idx_in", [P, W // P], I32),
        pe_dm=ein("pe_dm", [D, W]),
        masks=ein("masks", [2, 4, P, 256], MDT),
        halo_offs=ein("halo_offs", [P, DT], I32),
        out=nc.dram_tensor("logits_vm", [VG, P, GL * CHUNK], F16, kind="ExternalOutput"),
    )
    if os.environ.get("KDEBUG") == "1":
        io["d_y"] = nc.dram_tensor("d_y", [D, CHUNK], F16, kind="ExternalOutput")
        io["d_yh"] = nc.dram_tensor("d_yh", [D, HALF], F16, kind="ExternalOutput")
        io["d_attr"] = nc.dram_tensor("d_attr", [D, CHUNK], F16, kind="ExternalOutput")
        io["d_rf"] = nc.dram_tensor("d_rf", [16, 256], F32, kind="ExternalOutput")
        io["d_x1"] = nc.dram_tensor("d_x1", [D, CHUNK], F32, kind="ExternalOutput")
        io["d_xh"] = nc.dram_tensor("d_xh", [D, HALF], F32, kind="ExternalOutput")
        io["d_xhp"] = nc.dram_tensor("d_xhp", [D, HALF], F32, kind="ExternalOutput")
        io["d_y2h"] = nc.dram_tensor("d_y2h", [D, HALF], F16, kind="ExternalOutput")

    with tile.TileContext(nc) as tc, nc.allow_low_precision(reason="f32r rounding"):
        _emit(nc, tc, io)
    nc.compile()
    return nc


def _emit(nc, tc, io):
    cpool = tc.alloc_tile_pool(name="const", bufs=1)
    xpool = tc.alloc_tile_pool(name="xres", bufs=1)
    ps_a = tc.alloc_tile_pool(name="ps_a", bufs=2, space="PSUM")
    ps_b = tc.alloc_tile_pool(name="ps_b", bufs=4, space="PSUM")
    ps_c = tc.alloc_tile_pool(name="ps_c", bufs=2, space="PSUM")
    drp = tc.alloc_tile_pool(name="drbounce", bufs=1, space="DRAM")

    # ------------------------------------------------ constants
    ones_f = cpool.tile([P, P], F32, tag="ones_f")
    nc.vector.memset(ones_f[:], 1.0)
    ones = cpool.tile([P, P], MDT, tag="ones")
    nc.vector.tensor_copy(out=ones[:], in_=ones_f[:])
    ones_r = cpool.tile([P, P], F32R, tag="ones_r")
    nc.vector.tensor_copy(out=ones_r[:], in_=ones_f[:])
    identm = cpool.tile([P, P], MDT, tag="identm")
    make_identity(nc, identm[:])
    negb = cpool.tile([P, 1], F32, tag="negb")
    nc.vector.memset(negb[:], EXP_SHIFT)
    epsb = cpool.tile([P, 1], F32, tag="epsb")
    nc.vector.memset(epsb[:], 1e-6)
    l1s = cpool.tile([P, NL, DT], F32, tag="l1s")
    l1b = cpool.tile([P, NL, DT], F32, tag="l1b")
    l2s = cpool.tile([P, NL, DT], F32, tag="l2s")
    l2b = cpool.tile([P, NL, DT], F32, tag="l2b")
    lfs = cpool.tile([P, DT], F32, tag="lfs")
    lfb = cpool.tile([P, DT], F32, tag="lfb")
    nc.sync.dma_start(out=l1s[:], in_=io["ln1_s"].ap().rearrange("l (t p) -> p l t", p=P))
    nc.sync.dma_start(out=l1b[:], in_=io["ln1_b"].ap().rearrange("l (t p) -> p l t", p=P))
    nc.sync.dma_start(out=l2s[:], in_=io["ln2_s"].ap().rearrange("l (t p) -> p l t", p=P))
    nc.sync.dma_start(out=l2b[:], in_=io["ln2_b"].ap().rearrange("l (t p) -> p l t", p=P))
    nc.sync.dma_start(out=lfs[:], in_=io["lnf_s"].ap().rearrange("o (t p) -> p (o t)", p=P))
    nc.sync.dma_start(out=lfb[:], in_=io["lnf_b"].ap().rearrange("o (t p) -> p (o t)", p=P))
    b1t = cpool.tile([P, NL, MT], F32, tag="b1t")
    b2t = cpool.tile([P, NL, DT], F32, tag="b2t")
    nc.sync.dma_start(out=b1t[:], in_=io["b1"].ap().rearrange("l (m p) -> p l m", p=P))
    nc.sync.dma_start(out=b2t[:], in_=io["b2"].ap().rearrange("l (t p) -> p l t", p=P))
    masks = cpool.tile([P, 2, 4, 256], MDT, tag="masks")
    nc.sync.dma_start(out=masks[:], in_=io["masks"].ap().rearrange("a b p q -> p a b q"))
    hoffs = cpool.tile([P, DT], I32, tag="hoffs")
    nc.sync.dma_start(out=hoffs[:], in_=io["halo_offs"].ap())

    # residual stream (own 512 tokens, d-major, f32r so LN sum-matmuls can
    # consume it directly at full PE rate) + per-layer halo
    x = xpool.tile([P, DT, CHUNK], F32R, tag="x")
    xh = xpool.tile([P, DT, HALF], F32R, tag="xh")

    # ------------------------------------------------ embedding
    with tc.tile_pool(name="embed", bufs=1) as epool:
        ident = epool.tile([P, P], F32, tag="ident")
        make_identity(nc, ident[:])
        pe = epool.tile([P, DT, W], F32, tag="pe")
        nc.sync.dma_start(out=pe[:], in_=io["pe_dm"].ap().rearrange("(t p) m -> p t m", p=P))
        idxt = epool.tile([P, W // P], I32, tag="idxt")
        nc.sync.dma_start(out=idxt[:], in_=io["idx_in"].ap())
        with tc.tile_pool(name="gath", bufs=2) as gpool:
            for g in range(W // P):
                gt = gpool.tile([P, D], F32, tag="gt")
                nc.gpsimd.indirect_dma_start(
                    out=gt[:], out_offset=None, in_=io["embed"].ap(),
                    in_offset=bass.IndirectOffsetOnAxis(ap=idxt[:, g:g + 1], axis=0),
                )
                for dt in range(DT):
                    pt = ps_a.tile([P, P], F32, tag="ps_a")
                    nc.tensor.transpose(pt[:], gt[:, ts(dt, P)], ident[:])
                    dst = xh[:, dt, ts(g, P)] if g < 2 else x[:, dt, ts(g - 2, P)]
                    nc.vector.tensor_add(out=dst, in0=pt[:], in1=pe[:, dt, ts(g, P)])

    # ------------------------------------------------ layer pools
    lp = tc.alloc_tile_pool(name="layers", bufs=1)
    tp = tc.alloc_tile_pool(name="ltrans", bufs=2)
    lp3 = tc.alloc_tile_pool(name="ltrans3", bufs=3)
    vtp = tc.alloc_tile_pool(name="vtpool", bufs=1)

    # V tiles with a trailing ones column per head: PV matmul row DH
    # accumulates the softmax denominator for free. Ones written once.
    vt = [vtp.tile([P, H * (DH + 1)], MDT, tag=f"vt{t}", name=f"vt{t}")
          for t in range(W // P)]
    for t in range(W // P):
        vtv = vt[t][:].rearrange("p (h c) -> p h c", c=DH + 1)
        nc.vector.tensor_copy(out=vtv[:, :, DH:DH + 1], in_=ones[:, 0:H])

    def emit_ln(blocks, s_of, b_of, xdt=F32R):
        """LN over d (partition axis x DT). blocks: list of
        (src_fn(dt)->AP[128,width], dst_fn(dt)->AP[128,width], width<=512)."""
        ones_x = ones_r if xdt == F32R else ones
        for fn, dst, width in blocks:
            sx = ps_a.tile([1, 512], F32, tag="ps_a")
            sxx = ps_a.tile([1, 512], F32, tag="ps_a")
            for dt in range(DT):
                xsq = lp3.tile([P, 512], MDT, tag="ln_xsq", bufs=2)
                nc.vector.tensor_mul(out=xsq[:, :width], in0=fn(dt), in1=fn(dt))
                nc.tensor.matmul(out=sx[:, :width], lhsT=ones_x[:, 0:1], rhs=fn(dt),
                                 start=(dt == 0), stop=(dt == DT - 1))
                nc.tensor.matmul(out=sxx[:, :width], lhsT=ones[:, 0:1], rhs=xsq[:, :width],
                                 start=(dt == 0), stop=(dt == DT - 1))
            mu = lp3.tile([1, 512], MDT, tag="ln_mu", bufs=2)
            nc.vector.tensor_scalar_mul(out=mu[:, :width], in0=sx[:, :width], scalar1=1.0 / D)
            mu2 = lp3.tile([1, 512], F32, tag="ln_mu2", bufs=1)
            nc.vector.tensor_mul(out=mu2[:, :width], in0=mu[:, :width], in1=mu[:, :width])
            var = lp3.tile([1, 512], F32, tag="ln_var", bufs=1)
            # var = sxx/D - mu^2
            nc.vector.scalar_tensor_tensor(
                out=var[:, :width], in0=sxx[:, :width], scalar=1.0 / D,
                in1=mu2[:, :width], op0=OP.mult, op1=OP.subtract)
            sd = lp3.tile([1, 512], F32, tag="ln_sd", bufs=1)
            nc.scalar.activation(sd[:, :width], var[:, :width], AF.Sqrt, bias=epsb[0:1, :], scale=1.0)
            rt = lp3.tile([1, 512], F32, tag="ln_rt", bufs=1)
            nc.vector.reciprocal_approx_fast(out=rt[:, :width], in_=sd[:, :width])
            rstd = lp3.tile([1, 512], MDT, tag="ln_rstd", bufs=2)
            nc.vector.tensor_copy(out=rstd[:, :width], in_=rt[:, :width])
            pmu = ps_c.tile([P, 512], F32, tag="ps_c")
            nc.tensor.matmul(out=pmu[:, :width], lhsT=ones[0:1, :], rhs=mu[:, :width],
                             start=True, stop=True)
            prs = ps_c.tile([P, 512], F32, tag="ps_c")
            nc.tensor.matmul(out=prs[:, :width], lhsT=ones[0:1, :], rhs=rstd[:, :width],
                             start=True, stop=True)
            for dt in range(DT):
                scr = lp3.tile([P, 512], F32, tag="ln_scr", bufs=2)
                nc.vector.tensor_sub(out=scr[:, :width], in0=fn(dt), in1=pmu[:, :width])
                nc.vector.tensor_mul(out=scr[:, :width], in0=scr[:, :width], in1=prs[:, :width])
                nc.vector.tensor_scalar(out=dst(dt), in0=scr[:, :width],
                                        scalar1=s_of(dt), scalar2=b_of(dt),
                                        op0=OP.mult, op1=OP.add)

    def load_w(dram_ap, tag_r, shape3, rpool=None):
        wr = (rpool or tp).tile(shape3, MDT, tag=tag_r)
        nc.sync.dma_start(out=wr[:], in_=dram_ap)
        return wr

    # ------------------------------------------------ transformer layers
    _knl = int(os.environ.get("KNL", NL))
    xh_pre = xpool.tile([P, DT, HALF], F32R, tag="xh_pre")
    wmlp = {}
    for l in range(_knl):
        li = l % NL
        lpv = (l - 1) % NL
        s1 = lambda dt: l1s[:, li, dt:dt + 1]
        b1_ = lambda dt: l1b[:, li, dt:dt + 1]
        y = lp.tile([P, DT, CHUNK], MDT, tag="y")
        yh = lp.tile([P, DT, HALF], MDT, tag="yh")
        krh = lp.tile([P, DT, HALF], MDT, tag="krh")
        # LN1 over own tokens first: independent of the halo AllGather.
        emit_ln([(lambda dt: x[:, dt, :], lambda dt: y[:, dt, :], CHUNK)], s1, b1_)

        # --- projections (weights stationary, d-major out), own tokens
        wq_r = load_w(io["wq"].ap()[li].rearrange("(t p) m -> p t m", p=P), "wr", [P, DT, D])
        qr = lp.tile([P, DT, CHUNK], MDT, tag="qr")
        for do in range(DT):
            pq = ps_a.tile([P, CHUNK], F32, tag="ps_a")
            for dt in range(DT):
                nc.tensor.matmul(out=pq[:], lhsT=wq_r[:, dt, ts(do, P)],
                                 rhs=y[:, dt, :], start=(dt == 0), stop=(dt == DT - 1))
            nc.vector.tensor_copy(out=qr[:, do, :], in_=pq[:])

        wk_r = load_w(io["wk"].ap()[li].rearrange("(t p) m -> p t m", p=P), "wr", [P, DT, D])
        kro = lp.tile([P, DT, CHUNK], MDT, tag="kro")
        for do in range(DT):
            pk = ps_a.tile([P, CHUNK], F32, tag="ps_a")
            for dt in range(DT):
                nc.tensor.matmul(out=pk[:], lhsT=wk_r[:, dt, ts(do, P)],
                                 rhs=y[:, dt, :], start=(dt == 0), stop=(dt == DT - 1))
            nc.vector.tensor_copy(out=kro[:, do, :], in_=pk[:])

        wv_r = load_w(io["wv"].ap()[li].rearrange("(t p) m -> p t m", p=P), "wr", [P, DT, D])
        for t in range(2, W // P):
            pv = ps_a.tile([P, D], F32, tag="ps_a")
            for dt in range(DT):
                nc.tensor.matmul(out=pv[:], lhsT=y[:, dt, ts(t - 2, P)], rhs=wv_r[:, dt, :],
                                 start=(dt == 0), stop=(dt == DT - 1))
            vtv = vt[t][:].rearrange("p (h c) -> p h c", c=DH + 1)
            nc.vector.tensor_copy(out=vtv[:, :, 0:DH],
                                  in_=pv[:].rearrange("p (h c) -> p h c", c=DH))

        # --- sliding-window attention, deferred softmax division
        attru = lp.tile([P, DT, CHUNK], F32, tag="attru")
        attr = lp.tile([P, DT, CHUNK], MDT, tag="attr")

        def kslice(kt, r0, dto):
            if kt < 2:
                return krh[ds(r0, DH), dto, ts(kt, P)]
            return kro[ds(r0, DH), dto, ts(kt - 2, P)]

        def attn_qblock(qB):
            for h in range(H):
                r0 = (h % 2) * DH
                dto = h // 2
                pa = ps_c.tile([DH + 1, 256], F32, tag="ps_c")
                for jp in range(2):
                    pscore = ps_b.tile([P, 512], F32, tag="ps_b")
                    for jj in range(2):
                        j = jp * 2 + jj
                        kt = qB * 2 + j
                        nc.tensor.matmul(out=pscore[:, ts(jj, 256)],
                                         lhsT=kslice(kt, r0, dto),
                                         rhs=qr[ds(r0, DH), dto, ds(qB * 256, 256)],
                                         start=True, stop=False)
                        nc.tensor.matmul(out=pscore[:, ts(jj, 256)],
                                         lhsT=identm[:], rhs=masks[:, qB, j, :],
                                         start=False, stop=True)
                    ej = lp3.tile([P, 512], MDT, tag="ej", bufs=3)
                    nc.scalar.activation(ej[:], pscore[:], AF.Exp, bias=negb[:], scale=SCALE)
                    for jj in range(2):
                        j = jp * 2 + jj
                        kt = qB * 2 + j
                        nc.tensor.matmul(out=pa[:], lhsT=vt[kt][:, ds(h * (DH + 1), DH + 1)],
                                         rhs=ej[:, ts(jj, 256)], start=(j == 0), stop=(j == 3))
                nc.vector.tensor_copy(out=attru[ds(r0, DH), dto, ds(qB * 256, 256)],
                                      in_=pa[0:DH, :])
                dent = lp3.tile([1, 256], F32, tag="dent", bufs=2)
                nc.vector.tensor_copy(out=dent[:], in_=pa[DH:DH + 1, :])
                rf = lp3.tile([1, 256], F32, tag="rf", bufs=2)
                nc.vector.reciprocal_approx_fast(out=rf[:], in_=dent[:])
                rfh = lp3.tile([1, 256], MDT, tag="rfh", bufs=2)
                nc.vector.tensor_copy(out=rfh[:], in_=rf[:])
                if l == 0 and "d_rf" in io:
                    nc.sync.dma_start(out=io["d_rf"].ap()[qB * 8 + h:qB * 8 + h + 1, :], in_=rf[:])
                psc = ps_c.tile([DH, 256], F32, tag="ps_c")
                nc.tensor.matmul(out=psc[:], lhsT=ones[0:1, 0:DH], rhs=rfh[:],
                                 start=True, stop=True)
                nc.vector.tensor_mul(out=attr[ds(r0, DH), dto, ds(qB * 256, 256)],
                                     in0=attru[ds(r0, DH), dto, ds(qB * 256, 256)],
                                     in1=psc[:])

        # q-block 1 attends only to own keys: runs while the halo is in flight
        attn_qblock(1)

        # --- halo recompute (layers >= 1): the AllGather shipped the
        # PRE-MLP residual halo; apply the previous layer's LN2+MLP here.
        if l >= 1:
            if l == 1 and "d_xhp" in io:
                xhpd = lp3.tile([P, DT, HALF], F32, tag="xhpd", bufs=1)
                nc.vector.tensor_copy(out=xhpd[:], in_=xh_pre[:])
                nc.sync.dma_start(out=io["d_xhp"].ap().rearrange("(t p) m -> p t m", p=P), in_=xhpd[:])
            y2h = lp.tile([P, DT, HALF], MDT, tag="y2h")
            emit_ln([(lambda dt: xh_pre[:, dt, :], lambda dt: y2h[:, dt, :], HALF)],
                    lambda dt: l2s[:, lpv, dt:dt + 1], lambda dt: l2b[:, lpv, dt:dt + 1])
            if l == 1 and "d_y2h" in io:
                nc.sync.dma_start(out=io["d_y2h"].ap().rearrange("(t p) m -> p t m", p=P), in_=y2h[:])
            w1p, w2p = wmlp[(l - 1) % 2]
            pbh = [ps_b.tile([P, CHUNK], F32, tag="ps_b", name=f"pbh{i}") for i in range(DT)]

            def emit_mlp2h(m, hmh):
                for do in range(DT):
                    nc.tensor.matmul(out=pbh[do][:, :HALF],
                                     lhsT=w2p[:, m, ts(do, P)], rhs=hmh[:],
                                     start=(m == 0), stop=(m == MT - 1))

            histh = []
            for m in range(MT):
                p1h = ps_a.tile([P, CHUNK], F32, tag="ps_a")
                for dt in range(DT):
                    nc.tensor.matmul(out=p1h[:, :HALF], lhsT=w1p[:, dt, ts(m, P)],
                                     rhs=y2h[:, dt, :], start=(dt == 0), stop=(dt == DT - 1))
                hmh = lp3.tile([P, HALF], MDT, tag="hmh", bufs=3)
                nc.scalar.activation(hmh[:], p1h[:, :HALF], AF.Gelu_apprx_tanh,
                                     bias=b1t[:, lpv, m:m + 1], scale=1.0)
                histh.append((m, hmh))
                if len(histh) > 2:
                    emit_mlp2h(*histh.pop(0))
            for mm_, hh_ in histh:
                emit_mlp2h(mm_, hh_)
            for do in range(DT):
                nc.vector.scalar_tensor_tensor(
                    out=xh[:, do, :], in0=pbh[do][:, :HALF],
                    scalar=b2t[:, lpv, do:do + 1], in1=xh_pre[:, do, :],
                    op0=OP.add, op1=OP.add)

        if l == 1 and "d_xh" in io:
            xhd = lp3.tile([P, DT, HALF], F32, tag="xhd", bufs=1)
            nc.vector.tensor_copy(out=xhd[:], in_=xh[:])
            nc.sync.dma_start(out=io["d_xh"].ap().rearrange("(t p) m -> p t m", p=P), in_=xhd[:])

        # --- LN1 over halo + K/V halo
        emit_ln([(lambda dt: xh[:, dt, :], lambda dt: yh[:, dt, :], HALF)], s1, b1_)
        for do in range(DT):
            pk = ps_a.tile([P, CHUNK], F32, tag="ps_a")
            for dt in range(DT):
                nc.tensor.matmul(out=pk[:, :HALF], lhsT=wk_r[:, dt, ts(do, P)],
                                 rhs=yh[:, dt, :], start=(dt == 0), stop=(dt == DT - 1))
            nc.vector.tensor_copy(out=krh[:, do, :], in_=pk[:, :HALF])
        for t in range(2):
            pv = ps_a.tile([P, D], F32, tag="ps_a")
            for dt in range(DT):
                nc.tensor.matmul(out=pv[:], lhsT=yh[:, dt, ts(t, P)], rhs=wv_r[:, dt, :],
                                 start=(dt == 0), stop=(dt == DT - 1))
            vtv = vt[t][:].rearrange("p (h c) -> p h c", c=DH + 1)
            nc.vector.tensor_copy(out=vtv[:, :, 0:DH],
                                  in_=pv[:].rearrange("p (h c) -> p h c", c=DH))

        attn_qblock(0)

        if l == 0 and "d_y" in io:
            nc.sync.dma_start(out=io["d_y"].ap().rearrange("(t p) m -> p t m", p=P), in_=y[:])
            nc.sync.dma_start(out=io["d_yh"].ap().rearrange("(t p) m -> p t m", p=P), in_=yh[:])
            nc.sync.dma_start(out=io["d_attr"].ap().rearrange("(t p) m -> p t m", p=P), in_=attr[:])

        # --- output projection + residual
        wo_r = load_w(io["wo"].ap()[li].rearrange("(t p) m -> p t m", p=P), "wr", [P, DT, D])
        for do in range(DT):
            po = ps_a.tile([P, CHUNK], F32, tag="ps_a")
            for dt in range(DT):
                nc.tensor.matmul(out=po[:], lhsT=wo_r[:, dt, ts(do, P)],
                                 rhs=attr[:, dt, :], start=(dt == 0), stop=(dt == DT - 1))
            nc.vector.tensor_add(out=x[:, do, :], in0=x[:, do, :], in1=po[:])

        # --- ship the PRE-MLP halo now; receiver recomputes its MLP.
        # This hides the AllGather under our MLP + the next layer's
        # halo-independent work.
        if l < NL - 1:
            xhs = lp.tile([P, DT, HALF], F32R, tag="xhs")
            nc.vector.tensor_copy(out=xhs[:], in_=x[:, :, ds(HALF, HALF)])
            agin = drp.tile([D, HALF], F32R, tag=f"agin{l}")
            agout = drp.tile([len(GROUPS[0]) * D, HALF], F32R, tag=f"agout{l}")
            nc.sync.dma_start(out=agin[:].rearrange("(t p) m -> p t m", p=P),
                              in_=xhs[:])
            nc.gpsimd.collective_compute(
                "AllGather", OP.bypass, replica_groups=GROUPS,
                ins=[agin.opt()], outs=[agout.opt()])
            for dt in range(DT):
                nc.gpsimd.indirect_dma_start(
                    out=xh_pre[:, dt, :], out_offset=None, in_=agout[:],
                    in_offset=bass.IndirectOffsetOnAxis(ap=hoffs[:, dt:dt + 1], axis=0))

        # --- LN2 + MLP
        y2 = lp.tile([P, DT, CHUNK], MDT, tag="y2")
        emit_ln([(lambda dt: x[:, dt, :], lambda dt: y2[:, dt, :], CHUNK)],
                lambda dt: l2s[:, li, dt:dt + 1], lambda dt: l2b[:, li, dt:dt + 1])

        pb = [ps_b.tile([P, CHUNK], F32, tag="ps_b", name=f"pb{i}") for i in range(DT)]
        w1r = lp.tile([P, DT, MLPD], MDT, tag=f"w1r{l % 2}")
        nc.sync.dma_start(out=w1r[:], in_=io["w1"].ap()[li].rearrange("(t p) m -> p t m", p=P))
        w2r = lp.tile([P, MT, D], MDT, tag=f"w2r{l % 2}")
        nc.sync.dma_start(out=w2r[:], in_=io["w2"].ap()[li].rearrange("(t p) m -> p t m", p=P))
        wmlp[l % 2] = (w1r, w2r)

        def emit_mlp2(m, hm):
            for do in range(DT):
                nc.tensor.matmul(out=pb[do][:], lhsT=w2r[:, m, ts(do, P)],
                                 rhs=hm[:], start=(m == 0), stop=(m == MT - 1))

        hist = []
        for m in range(MT):
            p1 = ps_a.tile([P, CHUNK], F32, tag="ps_a")
            for dt in range(DT):
                nc.tensor.matmul(out=p1[:], lhsT=w1r[:, dt, ts(m, P)],
                                 rhs=y2[:, dt, :], start=(dt == 0), stop=(dt == DT - 1))
            hm = lp3.tile([P, CHUNK], MDT, tag="hm", bufs=3)
            nc.scalar.activation(hm[:], p1[:], AF.Gelu_apprx_tanh,
                                 bias=b1t[:, li, m:m + 1], scale=1.0)
            hist.append((m, hm))
            if len(hist) > 2:
                emit_mlp2(*hist.pop(0))
        for mm_, hh_ in hist:
            emit_mlp2(mm_, hh_)
        # residual (+b2)
        for do in range(DT):
            nc.vector.scalar_tensor_tensor(
                out=x[:, do, :], in0=pb[do][:],
                scalar=b2t[:, li, do:do + 1], in1=x[:, do, :],
                op0=OP.add, op1=OP.add)
        if l == 0 and "d_x1" in io:
            xd = lp3.tile([P, DT, CHUNK], F32, tag="xdump", bufs=1)
            nc.vector.tensor_copy(out=xd[:], in_=x[:])
            nc.sync.dma_start(out=io["d_x1"].ap().rearrange("(t p) m -> p t m", p=P), in_=xd[:])

    # ------------------------------------------------ final LN + logits
    # Each core computes the FULL vocab for its own 512 tokens: no final
    # AllGather; w_out streams tile-by-tile from DRAM, prefetched by the
    # pool double-buffering. Output bias is added host-side.
    yf = lp.tile([P, DT, CHUNK], MDT, tag="y")
    emit_ln([(lambda dt: x[:, dt, :], lambda dt: yf[:, dt, :], CHUNK)],
            lambda dt: lfs[:, dt:dt + 1], lambda dt: lfb[:, dt:dt + 1])

    vtp.release()
    lp3.release()
    tp.release()

    ps_c.release()
    ps_b.release()
    fps = tc.alloc_tile_pool(name="fps", bufs=4, space="PSUM")
    with tc.tile_pool(name="ftrans", bufs=3) as ftp, \
         tc.tile_pool(name="fout", bufs=3) as fop:
        for g in range(VG):
            fwr = ftp.tile([P, GL, DT, P], MDT, tag="fwr")
            nc.sync.dma_start(out=fwr[:],
                              in_=io["w_tiles"].ap()[g]
                              .rearrange("p (j t q) -> p j t q", j=GL, t=DT))
            ot = fop.tile([P, GL, CHUNK], F16, tag="fot")
            for j in range(GL):
                pf = fps.tile([P, CHUNK], F32, tag="fps")
                for dt in range(DT):
                    nc.tensor.matmul(out=pf[:], lhsT=fwr[:, j, dt, :], rhs=yf[:, dt, :],
                                     start=(dt == 0), stop=(dt == DT - 1))
                if (g * GL + j) % 2 == 0:
                    nc.vector.tensor_copy(out=ot[:, j, :], in_=pf[:])
                else:
                    nc.scalar.activation(ot[:, j, :], pf[:], AF.Copy)
            nc.sync.dma_start(out=io["out"].ap()[g],
                              in_=ot[:].rearrange("p j m -> p (j m)"))

    fps.release()
    lp.release()
    drp.release()
    ps_a.release()
    xpool.release()
    cpool.release()


# ================================================================ host side
def _pe_table():
    pos = np.arange(S, dtype=np.float32)[:, None]
    div = np.exp(np.arange(0, D, 2, dtype=np.float32) * -(np.log(10000.0) / D))
    pe = np.zeros((S, D), dtype=np.float32)
    pe[:, 0::2] = np.sin(pos * div)
    pe[:, 1::2] = np.cos(pos * div)
    return pe


def _in_maps(inputs):
    inp = np.asarray(inputs["inputs"]).astype(np.int32)
    ids = np.pad(inp, ((0, 0), (1, 0)))[:, :-1].astype(np.int32)
    pe = _pe_table()
    wout = np.asarray(inputs["w_out"], dtype=np.float32).astype(np.float16)
    shared = {k: np.ascontiguousarray(np.asarray(inputs[k], dtype=np.float32))
              for k in ("embed", "b1", "b2", "ln1_s", "ln1_b", "ln2_s", "ln2_b")}
    for k in ("wq", "wk", "wv", "wo", "w1", "w2"):
        shared[k] = np.ascontiguousarray(
            np.asarray(inputs[k], dtype=np.float32).astype(np.float16))
    shared["lnf_s"] = np.asarray(inputs["lnf_s"], np.float32).reshape(1, D)
    shared["lnf_b"] = np.asarray(inputs["lnf_b"], np.float32).reshape(1, D)
    # w_tiles[g, p, ((j*DT+dt)*P)+q] = w_out[dt*128+p, (g*GL+j)*128+q]
    shared["w_tiles"] = np.ascontiguousarray(
        wout.reshape(DT, P, VG, GL, P).transpose(2, 1, 3, 0, 4)
        .reshape(VG, P, GL * DT * P))

    maps = []
    qi = np.arange(256)[None, :]
    ki = np.arange(P)[:, None]
    for c in range(NCORES):
        b, ch = divmod(c, NCORES // B)
        t0 = ch * CHUNK
        lo = t0 - HALF
        ids768 = np.zeros(W, np.int32)
        pe768 = np.zeros((W, D), np.float32)
        s0 = max(0, lo)
        ids768[s0 - lo:] = ids[b, s0:t0 + CHUNK]
        pe768[s0 - lo:] = pe[s0:t0 + CHUNK]
        m = np.zeros((2, 4, P, 256), np.float16)
        for qB in range(2):
            for j in range(4):
                w = 256 + qi - (j * P + ki)      # u_q - u_k
                ok = (w >= 0) & (w <= HALF)
                if ch == 0:
                    ok = ok & ((lo + qB * 256 + j * P + ki) >= 0)
                m[qB, j] = np.where(ok, 0.0, MASK_BIAS).astype(np.float16)
        src = ch - 1 if ch > 0 else 0
        hoffs = (src * D + np.arange(DT)[None, :] * P
                 + np.arange(P)[:, None]).astype(np.int32)
        mp = dict(shared)
        mp.update(
            idx_in=np.ascontiguousarray(ids768.reshape(W // P, P).T),
            pe_dm=np.ascontiguousarray(pe768.T),
            masks=m, halo_offs=hoffs)
        maps.append(mp)
    return maps


def kernel(**inputs):
    nc = _CACHE.get("nc")
    if nc is None:
        nc = _build()
        _CACHE["nc"] = nc
    maps = _in_maps(inputs)
    res = run_bass_kernel_spmd(nc, maps, list(range(NCORES))).results
    bout = np.asarray(inputs["b_out"], dtype=np.float32)
    full = np.empty((NTOK, V), np.float32)
    for c in range(NCORES):
        lv = (res[c]["logits_vm"].reshape(VG, P, GL, CHUNK)
              .transpose(0, 2, 1, 3).reshape(V, CHUNK))
        full[c * CHUNK:(c + 1) * CHUNK, :] = lv.T.astype(np.float32) + bout[None, :]
    return full.reshape(B, S, V)


# revision 36
# speedup vs baseline: 1.4741x; 1.0266x over previous
"""Longformer decoder (4 layers, sliding-window causal attention) on 8 trn2 cores.

Sharding: 4096 tokens (B=2 x S=2048) split into 8 contiguous chunks of 512
(core = b*4 + chunk). Activations are kept d-major ([dim, token], dim on
partitions) so every matmul is weights-stationary with no transposes.
Attention needs a 256-token left halo of K/V per layer: layer 0 computes it
locally from the embedding gather; layers 1-3 AllGather the residual-stream
halo over 4-core groups, overlapped with the next layer's halo-independent
work (LN1/Q/K/V over own tokens). Sliding-window masking is additive (-3000)
and applied inside PSUM via an identity-matmul accumulate, so the scalar
engine's exp produces masked probabilities directly. Softmax division is
deferred: unnormalized attention output and per-(head,q) denominators are
collected, one batched reciprocal per q-block computes 1/den for all heads,
and an indicator-matrix matmul broadcasts it back over the d-major layout.
The final projection is token-sharded: each core computes the FULL vocab for
its own 512 tokens (no final AllGather); w_out streams from DRAM in 5-tile
groups, and the output bias is added host-side.
"""
import os
import sys

import numpy as np

for _p in ("/opt/trn_rl_repo", "/root/.axon_site/_ro/trn_rl_repo"):
    if os.path.isdir(_p) and _p not in sys.path:
        sys.path.insert(0, _p)

import concourse.bass as bass
import concourse.mybir as mybir
import concourse.tile as tile
from concourse import bacc
from concourse.bass import ts, ds
from concourse.bass_utils import run_bass_kernel_spmd
from concourse.masks import make_identity

F32 = mybir.dt.float32
F32R = mybir.dt.float32r
F16 = mybir.dt.float16
I32 = mybir.dt.int32
MDT = F16 if os.environ.get("KMMDT", "f16") == "f16" else F32R
AF = mybir.ActivationFunctionType
OP = mybir.AluOpType

B, S, V, D, H, NL, MLPD = 2, 2048, 32000, 512, 8, 4, 2048
DH = D // H            # 64
HALF = 256             # attention half-window (WIN // 2)
P = 128
NCORES = 8
CHUNK = 512            # own tokens per core
W = CHUNK + HALF       # 768 = halo + own
DT = D // P            # 4 d-tiles
MT = MLPD // P         # 16 mlp tiles
VN = V // P            # 250 vocab tiles (each core does full vocab x own tokens)
GL = 5                 # vocab tiles per DMA group
VG = VN // GL          # 50 groups (batched DMA: 5KB contiguous per partition)
NTOK = B * S           # 4096
GROUPS = [[0, 1, 2, 3], [4, 5, 6, 7]]
EXP_SHIFT = 2.0
MASK_BIAS = -3000.0    # additive mask; exp(SCALE*(s+MASK_BIAS)+EXP_SHIFT) == 0
SCALE = float(1.0 / np.sqrt(DH))

_CACHE = {}


# ================================================================ builder
def _build():
    nc = bacc.Bacc("TRN2", target_bir_lowering=False, debug=False,
                   num_devices=NCORES)

    ein = lambda n, sh, dt=F32: nc.dram_tensor(n, sh, dt, kind="ExternalInput")
    io = dict(
        wq=ein("wq", [NL, D, D], MDT), wk=ein("wk", [NL, D, D], MDT),
        wv=ein("wv", [NL, D, D], MDT), wo=ein("wo", [NL, D, D], MDT),
        w1=ein("w1", [NL, D, MLPD], MDT), w2=ein("w2", [NL, MLPD, D], MDT),
        b1=ein("b1", [NL, MLPD]), b2=ein("b2", [NL, D]),
        ln1_s=ein("ln1_s", [NL, D]), ln1_b=ein("ln1_b", [NL, D]),
        ln2_s=ein("ln2_s", [NL, D]), ln2_b=ein("ln2_b", [NL, D]),
        lnf_s=ein("lnf_s", [1, D]), lnf_b=ein("lnf_b", [1, D]),
        w_tiles=ein("w_tiles", [VG, P, GL * DT * P], MDT),
        embed=ein("embed", [V, D]),
        idx_in=ein("idx_in", [P, W // P], I32),
        pe_dm=ein("pe_dm", [D, W]),
        masks=ein("masks", [2, 4, P, 256], MDT),
        halo_offs=ein("halo_offs", [P, DT], I32),
        out=nc.dram_tensor("logits_vm", [VG, P, GL * CHUNK], F16, kind="ExternalOutput"),
    )
    if os.environ.get("KDEBUG") == "1":
        io["d_y"] = nc.dram_tensor("d_y", [D, CHUNK], F16, kind="ExternalOutput")
        io["d_yh"] = nc.dram_tensor("d_yh", [D, HALF], F16, kind="ExternalOutput")
        io["d_attr"] = nc.dram_tensor("d_attr", [D, CHUNK], F16, kind="ExternalOutput")
        io["d_rf"] = nc.dram_tensor("d_rf", [16, 256], F32, kind="ExternalOutput")
        io["d_x1"] = nc.dram_tensor("d_x1", [D, CHUNK], F32, kind="ExternalOutput")
        io["d_xh"] = nc.dram_tensor("d_xh", [D, HALF], F32, kind="ExternalOutput")
        io["d_xhp"] = nc.dram_tensor("d_xhp", [D, HALF], F32, kind="ExternalOutput")
        io["d_y2h"] = nc.dram_tensor("d_y2h", [D, HALF], F16, kind="ExternalOutput")

    with tile.TileContext(nc) as tc, nc.allow_low_precision(reason="f32r rounding"):
        _emit(nc, tc, io)
    nc.compile()
    return nc


def _emit(nc, tc, io):
    cpool = tc.alloc_tile_pool(name="const", bufs=1)
    xpool = tc.alloc_tile_pool(name="xres", bufs=1)
    ps_a = tc.alloc_tile_pool(name="ps_a", bufs=2, space="PSUM")
    ps_b = tc.alloc_tile_pool(name="ps_b", bufs=4, space="PSUM")
    ps_c = tc.alloc_tile_pool(name="ps_c", bufs=2, space="PSUM")
    drp = tc.alloc_tile_pool(name="drbounce", bufs=1, space="DRAM")

    # ------------------------------------------------ constants
    ones_f = cpool.tile([P, P], F32, tag="ones_f")
    nc.vector.memset(ones_f[:], 1.0)
    ones = cpool.tile([P, P], MDT, tag="ones")
    nc.vector.tensor_copy(out=ones[:], in_=ones_f[:])
    ones_r = cpool.tile([P, P], F32R, tag="ones_r")
    nc.vector.tensor_copy(out=ones_r[:], in_=ones_f[:])
    identm = cpool.tile([P, P], MDT, tag="identm")
    make_identity(nc, identm[:])
    negb = cpool.tile([P, 1], F32, tag="negb")
    nc.vector.memset(negb[:], EXP_SHIFT)
    epsb = cpool.tile([P, 1], F32, tag="epsb")
    nc.vector.memset(epsb[:], 1e-6)
    l1s = cpool.tile([P, NL, DT], F32, tag="l1s")
    l1b = cpool.tile([P, NL, DT], F32, tag="l1b")
    l2s = cpool.tile([P, NL, DT], F32, tag="l2s")
    l2b = cpool.tile([P, NL, DT], F32, tag="l2b")
    lfs = cpool.tile([P, DT], F32, tag="lfs")
    lfb = cpool.tile([P, DT], F32, tag="lfb")
    nc.sync.dma_start(out=l1s[:], in_=io["ln1_s"].ap().rearrange("l (t p) -> p l t", p=P))
    nc.sync.dma_start(out=l1b[:], in_=io["ln1_b"].ap().rearrange("l (t p) -> p l t", p=P))
    nc.sync.dma_start(out=l2s[:], in_=io["ln2_s"].ap().rearrange("l (t p) -> p l t", p=P))
    nc.sync.dma_start(out=l2b[:], in_=io["ln2_b"].ap().rearrange("l (t p) -> p l t", p=P))
    nc.sync.dma_start(out=lfs[:], in_=io["lnf_s"].ap().rearrange("o (t p) -> p (o t)", p=P))
    nc.sync.dma_start(out=lfb[:], in_=io["lnf_b"].ap().rearrange("o (t p) -> p (o t)", p=P))
    b1t = cpool.tile([P, NL, MT], F32, tag="b1t")
    b2t = cpool.tile([P, NL, DT], F32, tag="b2t")
    nc.sync.dma_start(out=b1t[:], in_=io["b1"].ap().rearrange("l (m p) -> p l m", p=P))
    nc.sync.dma_start(out=b2t[:], in_=io["b2"].ap().rearrange("l (t p) -> p l t", p=P))
    masks = cpool.tile([P, 2, 4, 256], MDT, tag="masks")
    nc.sync.dma_start(out=masks[:], in_=io["masks"].ap().rearrange("a b p q -> p a b q"))
    hoffs = cpool.tile([P, DT], I32, tag="hoffs")
    nc.sync.dma_start(out=hoffs[:], in_=io["halo_offs"].ap())

    # residual stream (own 512 tokens, d-major, f32r so LN sum-matmuls can
    # consume it directly at full PE rate) + per-layer halo
    x = xpool.tile([P, DT, CHUNK], F32R, tag="x")
    xh = xpool.tile([P, DT, HALF], F32R, tag="xh")

    # ------------------------------------------------ embedding
    with tc.tile_pool(name="embed", bufs=1) as epool:
        ident = epool.tile([P, P], F32, tag="ident")
        make_identity(nc, ident[:])
        pe = epool.tile([P, DT, W], F32, tag="pe")
        nc.sync.dma_start(out=pe[:], in_=io["pe_dm"].ap().rearrange("(t p) m -> p t m", p=P))
        idxt = epool.tile([P, W // P], I32, tag="idxt")
        nc.sync.dma_start(out=idxt[:], in_=io["idx_in"].ap())
        with tc.tile_pool(name="gath", bufs=2) as gpool:
            for g in range(W // P):
                gt = gpool.tile([P, D], F32, tag="gt")
                nc.gpsimd.indirect_dma_start(
                    out=gt[:], out_offset=None, in_=io["embed"].ap(),
                    in_offset=bass.IndirectOffsetOnAxis(ap=idxt[:, g:g + 1], axis=0),
                )
                for dt in range(DT):
                    pt = ps_a.tile([P, P], F32, tag="ps_a")
                    nc.tensor.transpose(pt[:], gt[:, ts(dt, P)], ident[:])
                    dst = xh[:, dt, ts(g, P)] if g < 2 else x[:, dt, ts(g - 2, P)]
                    nc.vector.tensor_add(out=dst, in0=pt[:], in1=pe[:, dt, ts(g, P)])

    # ------------------------------------------------ layer pools
    lp = tc.alloc_tile_pool(name="layers", bufs=1)
    tp = tc.alloc_tile_pool(name="ltrans", bufs=2)
    lp3 = tc.alloc_tile_pool(name="ltrans3", bufs=3)
    vtp = tc.alloc_tile_pool(name="vtpool", bufs=1)

    # V tiles with a trailing ones column per head: PV matmul row DH
    # accumulates the softmax denominator for free. Ones written once.
    vt = [vtp.tile([P, H * (DH + 1)], MDT, tag=f"vt{t}", name=f"vt{t}")
          for t in range(W // P)]
    for t in range(W // P):
        vtv = vt[t][:].rearrange("p (h c) -> p h c", c=DH + 1)
        nc.vector.tensor_copy(out=vtv[:, :, DH:DH + 1], in_=ones[:, 0:H])

    def emit_ln(blocks):
        """LN over d (partition axis x DT), stage-pipelined across blocks.
        blocks: list of (src_fn(dt), dst_fn(dt), width, s_of, b_of, xdt)."""
        st = []
        for bi, (fn, dst, width, sof, bof, xd) in enumerate(blocks):
            spool = ps_a if bi % 2 == 0 else ps_b
            ptag = "ps_a" if bi % 2 == 0 else "ps_b"
            ones_x = ones_r if xd == F32R else ones
            sx = spool.tile([1, 512], F32, tag=ptag)
            sxx = spool.tile([1, 512], F32, tag=ptag)
            for dt in range(DT):
                xsq = lp3.tile([P, 512], MDT, tag="ln_xsq", bufs=2)
                nc.vector.tensor_mul(out=xsq[:, :width], in0=fn(dt), in1=fn(dt))
                nc.tensor.matmul(out=sx[:, :width], lhsT=ones_x[:, 0:1], rhs=fn(dt),
                                 start=(dt == 0), stop=(dt == DT - 1))
                nc.tensor.matmul(out=sxx[:, :width], lhsT=ones[:, 0:1], rhs=xsq[:, :width],
                                 start=(dt == 0), stop=(dt == DT - 1))
            st.append([sx, sxx])
        for bi, (fn, dst, width, sof, bof, xd) in enumerate(blocks):
            sx, sxx = st[bi]
            mu = lp3.tile([1, 512], MDT, tag="ln_mu", bufs=2)
            nc.vector.tensor_scalar_mul(out=mu[:, :width], in0=sx[:, :width], scalar1=1.0 / D)
            mu2 = lp3.tile([1, 512], F32, tag="ln_mu2", bufs=2)
            nc.vector.tensor_mul(out=mu2[:, :width], in0=mu[:, :width], in1=mu[:, :width])
            var = lp3.tile([1, 512], F32, tag="ln_var", bufs=2)
            # var = sxx/D - mu^2
            nc.vector.scalar_tensor_tensor(
                out=var[:, :width], in0=sxx[:, :width], scalar=1.0 / D,
                in1=mu2[:, :width], op0=OP.mult, op1=OP.subtract)
            sd = lp3.tile([1, 512], F32, tag="ln_sd", bufs=2)
            nc.scalar.activation(sd[:, :width], var[:, :width], AF.Sqrt, bias=epsb[0:1, :], scale=1.0)
            rt = lp3.tile([1, 512], F32, tag="ln_rt", bufs=2)
            nc.vector.reciprocal_approx_fast(out=rt[:, :width], in_=sd[:, :width])
            rstd = lp3.tile([1, 512], MDT, tag="ln_rstd", bufs=2)
            nc.vector.tensor_copy(out=rstd[:, :width], in_=rt[:, :width])
            st[bi] += [mu, rstd]
        for bi, (fn, dst, width, sof, bof, xd) in enumerate(blocks):
            sx, sxx, mu, rstd = st[bi]
            bpool = ps_c if bi % 2 == 0 else ps_b
            ptag = "ps_c" if bi % 2 == 0 else "ps_b"
            pmu = bpool.tile([P, 512], F32, tag=ptag)
            nc.tensor.matmul(out=pmu[:, :width], lhsT=ones[0:1, :], rhs=mu[:, :width],
                             start=True, stop=True)
            prs = bpool.tile([P, 512], F32, tag=ptag)
            nc.tensor.matmul(out=prs[:, :width], lhsT=ones[0:1, :], rhs=rstd[:, :width],
                             start=True, stop=True)
            st[bi] += [pmu, prs]
        for bi, (fn, dst, width, sof, bof, xd) in enumerate(blocks):
            sx, sxx, mu, rstd, pmu, prs = st[bi]
            for dt in range(DT):
                scr = lp3.tile([P, 512], F32, tag="ln_scr", bufs=2)
                nc.vector.tensor_sub(out=scr[:, :width], in0=fn(dt), in1=pmu[:, :width])
                nc.vector.tensor_mul(out=scr[:, :width], in0=scr[:, :width], in1=prs[:, :width])
                nc.vector.tensor_scalar(out=dst(dt), in0=scr[:, :width],
                                        scalar1=sof(dt), scalar2=bof(dt),
                                        op0=OP.mult, op1=OP.add)

    def load_w(dram_ap, tag_r, shape3, rpool=None):
        wr = (rpool or tp).tile(shape3, MDT, tag=tag_r)
        nc.sync.dma_start(out=wr[:], in_=dram_ap)
        return wr

    # ------------------------------------------------ transformer layers
    _knl = int(os.environ.get("KNL", NL))
    xh_pre = xpool.tile([P, DT, HALF], F32R, tag="xh_pre")
    wmlp = {}
    for l in range(_knl):
        li = l % NL
        lpv = (l - 1) % NL
        s1 = lambda dt: l1s[:, li, dt:dt + 1]
        b1_ = lambda dt: l1b[:, li, dt:dt + 1]
        y = lp.tile([P, DT, CHUNK], MDT, tag="y")
        yh = lp.tile([P, DT, HALF], MDT, tag="yh")
        krh = lp.tile([P, DT, HALF], MDT, tag="krh")
        # LN1 over own tokens; for layers >= 1, paired (stage-pipelined) with
        # the halo LN2 recompute so their stat chains hide each other.
        ln1_own = (lambda dt: x[:, dt, :], lambda dt: y[:, dt, :], CHUNK, s1, b1_, F32R)
        if l == 0:
            emit_ln([ln1_own])
        else:
            # --- halo recompute: the AllGather shipped the PRE-MLP residual
            # halo; apply the previous layer's LN2+MLP here (overlaps the
            # collective with the sender's MLP + this layer's own work).
            y2h = lp.tile([P, DT, HALF], MDT, tag="y2h")
            emit_ln([ln1_own,
                     (lambda dt: xh_pre[:, dt, :], lambda dt: y2h[:, dt, :], HALF,
                      lambda dt: l2s[:, lpv, dt:dt + 1], lambda dt: l2b[:, lpv, dt:dt + 1],
                      F32R)])
            w1p, w2p = wmlp[(l - 1) % 2]
            pbh = [ps_b.tile([P, CHUNK], F32, tag="ps_b", name=f"pbh{i}") for i in range(DT)]

            def emit_mlp2h(m, hmh):
                for do in range(DT):
                    nc.tensor.matmul(out=pbh[do][:, :HALF],
                                     lhsT=w2p[:, m, ts(do, P)], rhs=hmh[:],
                                     start=(m == 0), stop=(m == MT - 1))

            histh = []
            for m in range(MT):
                p1h = ps_a.tile([P, CHUNK], F32, tag="ps_a")
                for dt in range(DT):
                    nc.tensor.matmul(out=p1h[:, :HALF], lhsT=w1p[:, dt, ts(m, P)],
                                     rhs=y2h[:, dt, :], start=(dt == 0), stop=(dt == DT - 1))
                hmh = lp3.tile([P, HALF], MDT, tag="hmh", bufs=3)
                nc.scalar.activation(hmh[:], p1h[:, :HALF], AF.Gelu_apprx_tanh,
                                     bias=b1t[:, lpv, m:m + 1], scale=1.0)
                histh.append((m, hmh))
                if len(histh) > 2:
                    emit_mlp2h(*histh.pop(0))
            for mm_, hh_ in histh:
                emit_mlp2h(mm_, hh_)
            for do in range(DT):
                nc.vector.scalar_tensor_tensor(
                    out=xh[:, do, :], in0=pbh[do][:, :HALF],
                    scalar=b2t[:, lpv, do:do + 1], in1=xh_pre[:, do, :],
                    op0=OP.add, op1=OP.add)
            if l == 1 and "d_xh" in io:
                xhd = lp3.tile([P, DT, HALF], F32, tag="xhd", bufs=1)
                nc.vector.tensor_copy(out=xhd[:], in_=xh[:])
                nc.sync.dma_start(out=io["d_xh"].ap().rearrange("(t p) m -> p t m", p=P), in_=xhd[:])

        # --- projections (weights stationary, d-major out), own tokens
        wq_r = load_w(io["wq"].ap()[li].rearrange("(t p) m -> p t m", p=P), "wr", [P, DT, D])
        qr = lp.tile([P, DT, CHUNK], MDT, tag="qr")
        for do in range(DT):
            pq = ps_a.tile([P, CHUNK], F32, tag="ps_a")
            for dt in range(DT):
                nc.tensor.matmul(out=pq[:], lhsT=wq_r[:, dt, ts(do, P)],
                                 rhs=y[:, dt, :], start=(dt == 0), stop=(dt == DT - 1))
            nc.vector.tensor_copy(out=qr[:, do, :], in_=pq[:])

        wk_r = load_w(io["wk"].ap()[li].rearrange("(t p) m -> p t m", p=P), "wr", [P, DT, D])
        kro = lp.tile([P, DT, CHUNK], MDT, tag="kro")
        for do in range(DT):
            pk = ps_a.tile([P, CHUNK], F32, tag="ps_a")
            for dt in range(DT):
                nc.tensor.matmul(out=pk[:], lhsT=wk_r[:, dt, ts(do, P)],
                                 rhs=y[:, dt, :], start=(dt == 0), stop=(dt == DT - 1))
            nc.vector.tensor_copy(out=kro[:, do, :], in_=pk[:])

        wv_r = load_w(io["wv"].ap()[li].rearrange("(t p) m -> p t m", p=P), "wr", [P, DT, D])
        for t in range(2, W // P):
            pv = ps_a.tile([P, D], F32, tag="ps_a")
            for dt in range(DT):
                nc.tensor.matmul(out=pv[:], lhsT=y[:, dt, ts(t - 2, P)], rhs=wv_r[:, dt, :],
                                 start=(dt == 0), stop=(dt == DT - 1))
            vtv = vt[t][:].rearrange("p (h c) -> p h c", c=DH + 1)
            nc.vector.tensor_copy(out=vtv[:, :, 0:DH],
                                  in_=pv[:].rearrange("p (h c) -> p h c", c=DH))

        # --- LN1 over halo + K/V halo
        emit_ln([(lambda dt: xh[:, dt, :], lambda dt: yh[:, dt, :], HALF, s1, b1_, F32R)])
        for do in range(DT):
            pk = ps_a.tile([P, CHUNK], F32, tag="ps_a")
            for dt in range(DT):
                nc.tensor.matmul(out=pk[:, :HALF], lhsT=wk_r[:, dt, ts(do, P)],
                                 rhs=yh[:, dt, :], start=(dt == 0), stop=(dt == DT - 1))
            nc.vector.tensor_copy(out=krh[:, do, :], in_=pk[:, :HALF])
        for t in range(2):
            pv = ps_a.tile([P, D], F32, tag="ps_a")
            for dt in range(DT):
                nc.tensor.matmul(out=pv[:], lhsT=yh[:, dt, ts(t, P)], rhs=wv_r[:, dt, :],
                                 start=(dt == 0), stop=(dt == DT - 1))
            vtv = vt[t][:].rearrange("p (h c) -> p h c", c=DH + 1)
            nc.vector.tensor_copy(out=vtv[:, :, 0:DH],
                                  in_=pv[:].rearrange("p (h c) -> p h c", c=DH))

        # --- sliding-window attention, deferred softmax division
        attru = lp.tile([P, DT, CHUNK], F32, tag="attru")
        attr = lp.tile([P, DT, CHUNK], MDT, tag="attr")

        def kslice(kt, r0, dto):
            if kt < 2:
                return krh[ds(r0, DH), dto, ts(kt, P)]
            return kro[ds(r0, DH), dto, ts(kt - 2, P)]

        def attn_qblock(qB):
            for h in range(H):
                r0 = (h % 2) * DH
                dto = h // 2
                pa = ps_c.tile([DH + 1, 256], F32, tag="ps_c")
                for jp in range(2):
                    pscore = ps_b.tile([P, 512], F32, tag="ps_b")
                    for jj in range(2):
                        j = jp * 2 + jj
                        kt = qB * 2 + j
                        nc.tensor.matmul(out=pscore[:, ts(jj, 256)],
                                         lhsT=kslice(kt, r0, dto),
                                         rhs=qr[ds(r0, DH), dto, ds(qB * 256, 256)],
                                         start=True, stop=False)
                        nc.tensor.matmul(out=pscore[:, ts(jj, 256)],
                                         lhsT=identm[:], rhs=masks[:, qB, j, :],
                                         start=False, stop=True)
                    ej = lp3.tile([P, 512], MDT, tag="ej", bufs=3)
                    nc.scalar.activation(ej[:], pscore[:], AF.Exp, bias=negb[:], scale=SCALE)
                    for jj in range(2):
                        j = jp * 2 + jj
                        kt = qB * 2 + j
                        nc.tensor.matmul(out=pa[:], lhsT=vt[kt][:, ds(h * (DH + 1), DH + 1)],
                                         rhs=ej[:, ts(jj, 256)], start=(j == 0), stop=(j == 3))
                nc.vector.tensor_copy(out=attru[ds(r0, DH), dto, ds(qB * 256, 256)],
                                      in_=pa[0:DH, :])
                dent = lp3.tile([1, 256], F32, tag="dent", bufs=2)
                nc.vector.tensor_copy(out=dent[:], in_=pa[DH:DH + 1, :])
                rf = lp3.tile([1, 256], F32, tag="rf", bufs=2)
                nc.vector.reciprocal_approx_fast(out=rf[:], in_=dent[:])
                rfh = lp3.tile([1, 256], MDT, tag="rfh", bufs=2)
                nc.vector.tensor_copy(out=rfh[:], in_=rf[:])
                if l == 0 and "d_rf" in io:
                    nc.sync.dma_start(out=io["d_rf"].ap()[qB * 8 + h:qB * 8 + h + 1, :], in_=rf[:])
                psc = ps_a.tile([DH, 256], F32, tag="ps_a")
                nc.tensor.matmul(out=psc[:], lhsT=ones[0:1, 0:DH], rhs=rfh[:],
                                 start=True, stop=True)
                nc.vector.tensor_mul(out=attr[ds(r0, DH), dto, ds(qB * 256, 256)],
                                     in0=attru[ds(r0, DH), dto, ds(qB * 256, 256)],
                                     in1=psc[:])

        attn_qblock(1)
        attn_qblock(0)

        if l == 0 and "d_y" in io:
            nc.sync.dma_start(out=io["d_y"].ap().rearrange("(t p) m -> p t m", p=P), in_=y[:])
            nc.sync.dma_start(out=io["d_yh"].ap().rearrange("(t p) m -> p t m", p=P), in_=yh[:])
            nc.sync.dma_start(out=io["d_attr"].ap().rearrange("(t p) m -> p t m", p=P), in_=attr[:])

        # --- output projection + residual
        wo_r = load_w(io["wo"].ap()[li].rearrange("(t p) m -> p t m", p=P), "wr", [P, DT, D])
        for do in range(DT):
            po = ps_a.tile([P, CHUNK], F32, tag="ps_a")
            for dt in range(DT):
                nc.tensor.matmul(out=po[:], lhsT=wo_r[:, dt, ts(do, P)],
                                 rhs=attr[:, dt, :], start=(dt == 0), stop=(dt == DT - 1))
            nc.vector.tensor_add(out=x[:, do, :], in0=x[:, do, :], in1=po[:])

        # --- ship the PRE-MLP halo now; receiver recomputes its MLP.
        # This hides the AllGather under our MLP + the next layer's
        # halo-independent work.
        if l < NL - 1:
            xhs = lp.tile([P, DT, HALF], F32R, tag="xhs")
            nc.vector.tensor_copy(out=xhs[:], in_=x[:, :, ds(HALF, HALF)])
            agin = drp.tile([D, HALF], F32R, tag=f"agin{l}")
            agout = drp.tile([len(GROUPS[0]) * D, HALF], F32R, tag=f"agout{l}")
            nc.sync.dma_start(out=agin[:].rearrange("(t p) m -> p t m", p=P),
                              in_=xhs[:])
            nc.gpsimd.collective_compute(
                "AllGather", OP.bypass, replica_groups=GROUPS,
                ins=[agin.opt()], outs=[agout.opt()])
            for dt in range(DT):
                nc.gpsimd.indirect_dma_start(
                    out=xh_pre[:, dt, :], out_offset=None, in_=agout[:],
                    in_offset=bass.IndirectOffsetOnAxis(ap=hoffs[:, dt:dt + 1], axis=0))

        # --- LN2 + MLP
        y2 = lp.tile([P, DT, CHUNK], MDT, tag="y2")
        s2 = lambda dt: l2s[:, li, dt:dt + 1]
        b2_ = lambda dt: l2b[:, li, dt:dt + 1]
        emit_ln([(lambda dt: x[:, dt, 0:256], lambda dt: y2[:, dt, 0:256], 256, s2, b2_, F32R),
                 (lambda dt: x[:, dt, 256:512], lambda dt: y2[:, dt, 256:512], 256, s2, b2_, F32R)])

        pb = [ps_b.tile([P, CHUNK], F32, tag="ps_b", name=f"pb{i}") for i in range(DT)]
        w1r = lp.tile([P, DT, MLPD], MDT, tag=f"w1r{l % 2}")
        nc.sync.dma_start(out=w1r[:], in_=io["w1"].ap()[li].rearrange("(t p) m -> p t m", p=P))
        w2r = lp.tile([P, MT, D], MDT, tag=f"w2r{l % 2}")
        nc.sync.dma_start(out=w2r[:], in_=io["w2"].ap()[li].rearrange("(t p) m -> p t m", p=P))
        wmlp[l % 2] = (w1r, w2r)

        def emit_mlp2(m, hm):
            for do in range(DT):
                nc.tensor.matmul(out=pb[do][:], lhsT=w2r[:, m, ts(do, P)],
                                 rhs=hm[:], start=(m == 0), stop=(m == MT - 1))

        hist = []
        for m in range(MT):
            p1 = ps_a.tile([P, CHUNK], F32, tag="ps_a")
            for dt in range(DT):
                nc.tensor.matmul(out=p1[:], lhsT=w1r[:, dt, ts(m, P)],
                                 rhs=y2[:, dt, :], start=(dt == 0), stop=(dt == DT - 1))
            hm = lp3.tile([P, CHUNK], MDT, tag="hm", bufs=3)
            nc.scalar.activation(hm[:], p1[:], AF.Gelu_apprx_tanh,
                                 bias=b1t[:, li, m:m + 1], scale=1.0)
            hist.append((m, hm))
            if len(hist) > 2:
                emit_mlp2(*hist.pop(0))
        for mm_, hh_ in hist:
            emit_mlp2(mm_, hh_)
        # residual (+b2)
        for do in range(DT):
            nc.vector.scalar_tensor_tensor(
                out=x[:, do, :], in0=pb[do][:],
                scalar=b2t[:, li, do:do + 1], in1=x[:, do, :],
                op0=OP.add, op1=OP.add)
        if l == 0 and "d_x1" in io:
            xd = lp3.tile([P, DT, CHUNK], F32, tag="xdump", bufs=1)
            nc.vector.tensor_copy(out=xd[:], in_=x[:])
            nc.sync.dma_start(out=io["d_x1"].ap().rearrange("(t p) m -> p t m", p=P), in_=xd[:])

    # ------------------------------------------------ final LN + logits
    # Each core computes the FULL vocab for its own 512 tokens: no final
    # AllGather; w_out streams tile-by-tile from DRAM, prefetched by the
    # pool double-buffering. Output bias is added host-side.
    yf = lp.tile([P, DT, CHUNK], MDT, tag="y")
    fs_ = lambda dt: lfs[:, dt:dt + 1]
    fb_ = lambda dt: lfb[:, dt:dt + 1]
    emit_ln([(lambda dt: x[:, dt, 0:256], lambda dt: yf[:, dt, 0:256], 256, fs_, fb_, F32R),
             (lambda dt: x[:, dt, 256:512], lambda dt: yf[:, dt, 256:512], 256, fs_, fb_, F32R)])

    vtp.release()
    lp3.release()
    tp.release()

    ps_c.release()
    ps_b.release()
    fps = tc.alloc_tile_pool(name="fps", bufs=4, space="PSUM")
    with tc.tile_pool(name="ftrans", bufs=3) as ftp, \
         tc.tile_pool(name="fout", bufs=3) as fop:
        for g in range(VG):
            fwr = ftp.tile([P, GL, DT, P], MDT, tag="fwr")
            nc.sync.dma_start(out=fwr[:],
                              in_=io["w_tiles"].ap()[g]
                              .rearrange("p (j t q) -> p j t q", j=GL, t=DT))
            ot = fop.tile([P, GL, CHUNK], F16, tag="fot")
            for j in range(GL):
                pf = fps.tile([P, CHUNK], F32, tag="fps")
                for dt in range(DT):
                    nc.tensor.matmul(out=pf[:], lhsT=fwr[:, j, dt, :], rhs=yf[:, dt, :],
                                     start=(dt == 0), stop=(dt == DT - 1))
                if (g * GL + j) % 2 == 0:
                    nc.vector.tensor_copy(out=ot[:, j, :], in_=pf[:])
                else:
                    nc.scalar.activation(ot[:, j, :], pf[:], AF.Copy)
            nc.sync.dma_start(out=io["out"].ap()[g],
                              in_=ot[:].rearrange("p j m -> p (j m)"))

    fps.release()
    lp.release()
    drp.release()
    ps_a.release()
    xpool.release()
    cpool.release()


# ================================================================ host side
def _pe_table():
    pos = np.arange(S, dtype=np.float32)[:, None]
    div = np.exp(np.arange(0, D, 2, dtype=np.float32) * -(np.log(10000.0) / D))
    pe = np.zeros((S, D), dtype=np.float32)
    pe[:, 0::2] = np.sin(pos * div)
    pe[:, 1::2] = np.cos(pos * div)
    return pe


def _in_maps(inputs):
    inp = np.asarray(inputs["inputs"]).astype(np.int32)
    ids = np.pad(inp, ((0, 0), (1, 0)))[:, :-1].astype(np.int32)
    pe = _pe_table()
    wout = np.asarray(inputs["w_out"], dtype=np.float32).astype(np.float16)
    shared = {k: np.ascontiguousarray(np.asarray(inputs[k], dtype=np.float32))
              for k in ("embed", "b1", "b2", "ln1_s", "ln1_b", "ln2_s", "ln2_b")}
    for k in ("wq", "wk", "wv", "wo", "w1", "w2"):
        shared[k] = np.ascontiguousarray(
            np.asarray(inputs[k], dtype=np.float32).astype(np.float16))
    shared["lnf_s"] = np.asarray(inputs["lnf_s"], np.float32).reshape(1, D)
    shared["lnf_b"] = np.asarray(inputs["lnf_b"], np.float32).reshape(1, D)
    # w_tiles[g, p, ((j*DT+dt)*P)+q] = w_out[dt*128+p, (g*GL+j)*128+q]
    shared["w_tiles"] = np.ascontiguousarray(
        wout.reshape(DT, P, VG, GL, P).transpose(2, 1, 3, 0, 4)
        .reshape(VG, P, GL * DT * P))

    maps = []
    qi = np.arange(256)[None, :]
    ki = np.arange(P)[:, None]
    for c in range(NCORES):
        b, ch = divmod(c, NCORES // B)
        t0 = ch * CHUNK
        lo = t0 - HALF
        ids768 = np.zeros(W, np.int32)
        pe768 = np.zeros((W, D), np.float32)
        s0 = max(0, lo)
        ids768[s0 - lo:] = ids[b, s0:t0 + CHUNK]
        pe768[s0 - lo:] = pe[s0:t0 + CHUNK]
        m = np.zeros((2, 4, P, 256), np.float16)
        for qB in range(2):
            for j in range(4):
                w = 256 + qi - (j * P + ki)      # u_q - u_k
                ok = (w >= 0) & (w <= HALF)
                if ch == 0:
                    ok = ok & ((lo + qB * 256 + j * P + ki) >= 0)
                m[qB, j] = np.where(ok, 0.0, MASK_BIAS).astype(np.float16)
        src = ch - 1 if ch > 0 else 0
        hoffs = (src * D + np.arange(DT)[None, :] * P
                 + np.arange(P)[:, None]).astype(np.int32)
        mp = dict(shared)
        mp.update(
            idx_in=np.ascontiguousarray(ids768.reshape(W // P, P).T),
            pe_dm=np.ascontiguousarray(pe768.T),
            masks=m, halo_offs=hoffs)
        maps.append(mp)
    return maps


def kernel(**inputs):
    nc = _CACHE.get("nc")
    if nc is None:
        nc = _build()
        _CACHE["nc"] = nc
    maps = _in_maps(inputs)
    res = run_bass_kernel_spmd(nc, maps, list(range(NCORES))).results
    bout = np.asarray(inputs["b_out"], dtype=np.float32)
    full = np.empty((NTOK, V), np.float32)
    for c in range(NCORES):
        lv = (res[c]["logits_vm"].reshape(VG, P, GL, CHUNK)
              .transpose(0, 2, 1, 3).reshape(V, CHUNK))
        full[c * CHUNK:(c + 1) * CHUNK, :] = lv.T.astype(np.float32) + bout[None, :]
    return full.reshape(B, S, V)


# revision 37
# speedup vs baseline: 1.5033x; 1.0198x over previous
"""Longformer decoder (4 layers, sliding-window causal attention) on 8 trn2 cores.

Sharding: 4096 tokens (B=2 x S=2048) split into 8 contiguous chunks of 512
(core = b*4 + chunk). Activations are kept d-major ([dim, token], dim on
partitions) so every matmul is weights-stationary with no transposes.
Attention needs a 256-token left halo of K/V per layer: layer 0 computes it
locally from the embedding gather; layers 1-3 AllGather the residual-stream
halo over 4-core groups, overlapped with the next layer's halo-independent
work (LN1/Q/K/V over own tokens). Sliding-window masking is additive (-3000)
and applied inside PSUM via an identity-matmul accumulate, so the scalar
engine's exp produces masked probabilities directly. Softmax division is
deferred: unnormalized attention output and per-(head,q) denominators are
collected, one batched reciprocal per q-block computes 1/den for all heads,
and an indicator-matrix matmul broadcasts it back over the d-major layout.
The final projection is token-sharded: each core computes the FULL vocab for
its own 512 tokens (no final AllGather); w_out streams from DRAM in 5-tile
groups, and the output bias is added host-side.
"""
import os
import sys

import numpy as np

for _p in ("/opt/trn_rl_repo", "/root/.axon_site/_ro/trn_rl_repo"):
    if os.path.isdir(_p) and _p not in sys.path:
        sys.path.insert(0, _p)

import concourse.bass as bass
import concourse.mybir as mybir
import concourse.tile as tile
from concourse import bacc
from concourse.bass import ts, ds
from concourse.bass_utils import run_bass_kernel_spmd
from concourse.masks import make_identity

F32 = mybir.dt.float32
F32R = mybir.dt.float32r
F16 = mybir.dt.float16
I32 = mybir.dt.int32
MDT = F16 if os.environ.get("KMMDT", "f16") == "f16" else F32R
AF = mybir.ActivationFunctionType
OP = mybir.AluOpType

B, S, V, D, H, NL, MLPD = 2, 2048, 32000, 512, 8, 4, 2048
DH = D // H            # 64
HALF = 256             # attention half-window (WIN // 2)
P = 128
NCORES = 8
CHUNK = 512            # own tokens per core
W = CHUNK + HALF       # 768 = halo + own
DT = D // P            # 4 d-tiles
MT = MLPD // P         # 16 mlp tiles
VN = V // P            # 250 vocab tiles (each core does full vocab x own tokens)
GL = 5                 # vocab tiles per DMA group
VG = VN // GL          # 50 groups (batched DMA: 5KB contiguous per partition)
NTOK = B * S           # 4096
GROUPS = [[0, 1, 2, 3], [4, 5, 6, 7]]
EXP_SHIFT = 2.0
MASK_BIAS = -3000.0    # additive mask; exp(SCALE*(s+MASK_BIAS)+EXP_SHIFT) == 0
SCALE = float(1.0 / np.sqrt(DH))

_CACHE = {}


# ================================================================ builder
def _build():
    nc = bacc.Bacc("TRN2", target_bir_lowering=False, debug=False,
                   num_devices=NCORES)

    ein = lambda n, sh, dt=F32: nc.dram_tensor(n, sh, dt, kind="ExternalInput")
    io = dict(
        wq=ein("wq", [NL, P, DT * D], MDT), wk=ein("wk", [NL, P, DT * D], MDT),
        wv=ein("wv", [NL, P, DT * D], MDT), wo=ein("wo", [NL, P, DT * D], MDT),
        w1=ein("w1", [NL, P, DT * MLPD], MDT), w2=ein("w2", [NL, P, MT * D], MDT),
        b1=ein("b1", [P, NL * MT]), b2=ein("b2", [P, NL * DT]),
        ln1_s=ein("ln1_s", [P, NL * DT]), ln1_b=ein("ln1_b", [P, NL * DT]),
        ln2_s=ein("ln2_s", [P, NL * DT]), ln2_b=ein("ln2_b", [P, NL * DT]),
        lnf_s=ein("lnf_s", [P, DT]), lnf_b=ein("lnf_b", [P, DT]),
        w_tiles=ein("w_tiles", [VG, P, GL * DT * P], MDT),
        embed=ein("embed", [V, D]),
        idx_in=ein("idx_in", [P, W // P], I32),
        pe_dm=ein("pe_dm", [P, DT * W]),
        masks=ein("masks", [P, 2 * 4 * 256], MDT),
        halo_offs=ein("halo_offs", [P, DT], I32),
        out=nc.dram_tensor("logits_vm", [VG, P, GL * CHUNK], F16, kind="ExternalOutput"),
    )
    if os.environ.get("KDEBUG") == "1":
        io["d_y"] = nc.dram_tensor("d_y", [D, CHUNK], F16, kind="ExternalOutput")
        io["d_yh"] = nc.dram_tensor("d_yh", [D, HALF], F16, kind="ExternalOutput")
        io["d_attr"] = nc.dram_tensor("d_attr", [D, CHUNK], F16, kind="ExternalOutput")
        io["d_rf"] = nc.dram_tensor("d_rf", [16, 256], F32, kind="ExternalOutput")
        io["d_x1"] = nc.dram_tensor("d_x1", [D, CHUNK], F32, kind="ExternalOutput")
        io["d_xh"] = nc.dram_tensor("d_xh", [D, HALF], F32, kind="ExternalOutput")
        io["d_xhp"] = nc.dram_tensor("d_xhp", [D, HALF], F32, kind="ExternalOutput")
        io["d_y2h"] = nc.dram_tensor("d_y2h", [D, HALF], F16, kind="ExternalOutput")

    with tile.TileContext(nc) as tc, nc.allow_low_precision(reason="f32r rounding"):
        _emit(nc, tc, io)
    nc.compile()
    return nc


def _emit(nc, tc, io):
    cpool = tc.alloc_tile_pool(name="const", bufs=1)
    xpool = tc.alloc_tile_pool(name="xres", bufs=1)
    ps_a = tc.alloc_tile_pool(name="ps_a", bufs=2, space="PSUM")
    ps_b = tc.alloc_tile_pool(name="ps_b", bufs=4, space="PSUM")
    ps_c = tc.alloc_tile_pool(name="ps_c", bufs=2, space="PSUM")
    drp = tc.alloc_tile_pool(name="drbounce", bufs=1, space="DRAM")

    # ------------------------------------------------ constants
    ones_f = cpool.tile([P, P], F32, tag="ones_f")
    nc.vector.memset(ones_f[:], 1.0)
    ones = cpool.tile([P, P], MDT, tag="ones")
    nc.vector.tensor_copy(out=ones[:], in_=ones_f[:])
    ones_r = cpool.tile([P, P], F32R, tag="ones_r")
    nc.vector.tensor_copy(out=ones_r[:], in_=ones_f[:])
    identm = cpool.tile([P, P], MDT, tag="identm")
    make_identity(nc, identm[:])
    negb = cpool.tile([P, 1], F32, tag="negb")
    nc.vector.memset(negb[:], EXP_SHIFT)
    epsb = cpool.tile([P, 1], F32, tag="epsb")
    nc.vector.memset(epsb[:], 1e-6)
    l1s = cpool.tile([P, NL, DT], F32, tag="l1s")
    l1b = cpool.tile([P, NL, DT], F32, tag="l1b")
    l2s = cpool.tile([P, NL, DT], F32, tag="l2s")
    l2b = cpool.tile([P, NL, DT], F32, tag="l2b")
    lfs = cpool.tile([P, DT], F32, tag="lfs")
    lfb = cpool.tile([P, DT], F32, tag="lfb")
    b1t = cpool.tile([P, NL, MT], F32, tag="b1t")
    b2t = cpool.tile([P, NL, DT], F32, tag="b2t")
    masks = cpool.tile([P, 2, 4, 256], MDT, tag="masks")
    hoffs = cpool.tile([P, DT], I32, tag="hoffs")

    def load_consts():
        nc.sync.dma_start(out=l1s[:], in_=io["ln1_s"].ap().rearrange("p (l t) -> p l t", l=NL))
        nc.sync.dma_start(out=l1b[:], in_=io["ln1_b"].ap().rearrange("p (l t) -> p l t", l=NL))
        nc.sync.dma_start(out=l2s[:], in_=io["ln2_s"].ap().rearrange("p (l t) -> p l t", l=NL))
        nc.sync.dma_start(out=l2b[:], in_=io["ln2_b"].ap().rearrange("p (l t) -> p l t", l=NL))
        nc.sync.dma_start(out=lfs[:], in_=io["lnf_s"].ap())
        nc.sync.dma_start(out=lfb[:], in_=io["lnf_b"].ap())
        nc.sync.dma_start(out=b1t[:], in_=io["b1"].ap().rearrange("p (l m) -> p l m", l=NL))
        nc.sync.dma_start(out=b2t[:], in_=io["b2"].ap().rearrange("p (l t) -> p l t", l=NL))
        nc.sync.dma_start(out=masks[:], in_=io["masks"].ap().rearrange("p (a b q) -> p a b q", a=2, b=4))
        nc.sync.dma_start(out=hoffs[:], in_=io["halo_offs"].ap())

    # residual stream (own 512 tokens, d-major, f32r so LN sum-matmuls can
    # consume it directly at full PE rate) + per-layer halo
    x = xpool.tile([P, DT, CHUNK], F32R, tag="x")
    xh = xpool.tile([P, DT, HALF], F32R, tag="xh")

    # ------------------------------------------------ embedding
    with tc.tile_pool(name="embed", bufs=1) as epool:
        ident = epool.tile([P, P], F32, tag="ident")
        make_identity(nc, ident[:])
        pe = epool.tile([P, DT, W], F32, tag="pe")
        nc.sync.dma_start(out=pe[:], in_=io["pe_dm"].ap().rearrange("p (t m) -> p t m", t=DT))
        idxt = epool.tile([P, W // P], I32, tag="idxt")
        nc.sync.dma_start(out=idxt[:], in_=io["idx_in"].ap())
        with tc.tile_pool(name="gath", bufs=2) as gpool:
            for g in range(W // P):
                gt = gpool.tile([P, D], F32, tag="gt")
                nc.gpsimd.indirect_dma_start(
                    out=gt[:], out_offset=None, in_=io["embed"].ap(),
                    in_offset=bass.IndirectOffsetOnAxis(ap=idxt[:, g:g + 1], axis=0),
                )
                for dt in range(DT):
                    pt = ps_a.tile([P, P], F32, tag="ps_a")
                    nc.tensor.transpose(pt[:], gt[:, ts(dt, P)], ident[:])
                    dst = xh[:, dt, ts(g, P)] if g < 2 else x[:, dt, ts(g - 2, P)]
                    nc.vector.tensor_add(out=dst, in0=pt[:], in1=pe[:, dt, ts(g, P)])

    load_consts()

    # ------------------------------------------------ layer pools
    lp = tc.alloc_tile_pool(name="layers", bufs=1)
    tp = tc.alloc_tile_pool(name="ltrans", bufs=2)
    lp3 = tc.alloc_tile_pool(name="ltrans3", bufs=3)
    vtp = tc.alloc_tile_pool(name="vtpool", bufs=1)

    # V tiles with a trailing ones column per head: PV matmul row DH
    # accumulates the softmax denominator for free. Ones written once.
    vt = [vtp.tile([P, H * (DH + 1)], MDT, tag=f"vt{t}", name=f"vt{t}")
          for t in range(W // P)]
    for t in range(W // P):
        vtv = vt[t][:].rearrange("p (h c) -> p h c", c=DH + 1)
        nc.vector.tensor_copy(out=vtv[:, :, DH:DH + 1], in_=ones[:, 0:H])

    def emit_ln(blocks):
        """LN over d (partition axis x DT), stage-pipelined across blocks.
        blocks: list of (src_fn(dt), dst_fn(dt), width, s_of, b_of, xdt)."""
        st = []
        for bi, (fn, dst, width, sof, bof, xd) in enumerate(blocks):
            spool = ps_a if bi % 2 == 0 else ps_b
            ptag = "ps_a" if bi % 2 == 0 else "ps_b"
            ones_x = ones_r if xd == F32R else ones
            sx = spool.tile([1, 512], F32, tag=ptag)
            sxx = spool.tile([1, 512], F32, tag=ptag)
            for dt in range(DT):
                xsq = lp3.tile([P, 512], MDT, tag="ln_xsq", bufs=2)
                nc.vector.tensor_mul(out=xsq[:, :width], in0=fn(dt), in1=fn(dt))
                nc.tensor.matmul(out=sx[:, :width], lhsT=ones_x[:, 0:1], rhs=fn(dt),
                                 start=(dt == 0), stop=(dt == DT - 1))
                nc.tensor.matmul(out=sxx[:, :width], lhsT=ones[:, 0:1], rhs=xsq[:, :width],
                                 start=(dt == 0), stop=(dt == DT - 1))
            st.append([sx, sxx])
        for bi, (fn, dst, width, sof, bof, xd) in enumerate(blocks):
            sx, sxx = st[bi]
            mu = lp3.tile([1, 512], MDT, tag="ln_mu", bufs=2)
            nc.vector.tensor_scalar_mul(out=mu[:, :width], in0=sx[:, :width], scalar1=1.0 / D)
            mu2 = lp3.tile([1, 512], F32, tag="ln_mu2", bufs=2)
            nc.vector.tensor_mul(out=mu2[:, :width], in0=mu[:, :width], in1=mu[:, :width])
            var = lp3.tile([1, 512], F32, tag="ln_var", bufs=2)
            # var = sxx/D - mu^2
            nc.vector.scalar_tensor_tensor(
                out=var[:, :width], in0=sxx[:, :width], scalar=1.0 / D,
                in1=mu2[:, :width], op0=OP.mult, op1=OP.subtract)
            sd = lp3.tile([1, 512], F32, tag="ln_sd", bufs=2)
            nc.scalar.activation(sd[:, :width], var[:, :width], AF.Sqrt, bias=epsb[0:1, :], scale=1.0)
            rt = lp3.tile([1, 512], F32, tag="ln_rt", bufs=2)
            nc.vector.reciprocal_approx_fast(out=rt[:, :width], in_=sd[:, :width])
            rstd = lp3.tile([1, 512], MDT, tag="ln_rstd", bufs=2)
            nc.vector.tensor_copy(out=rstd[:, :width], in_=rt[:, :width])
            st[bi] += [mu, rstd]
        for bi, (fn, dst, width, sof, bof, xd) in enumerate(blocks):
            sx, sxx, mu, rstd = st[bi]
            bpool = ps_c if bi % 2 == 0 else ps_b
            ptag = "ps_c" if bi % 2 == 0 else "ps_b"
            pmu = bpool.tile([P, 512], F32, tag=ptag)
            nc.tensor.matmul(out=pmu[:, :width], lhsT=ones[0:1, :], rhs=mu[:, :width],
                             start=True, stop=True)
            prs = bpool.tile([P, 512], F32, tag=ptag)
            nc.tensor.matmul(out=prs[:, :width], lhsT=ones[0:1, :], rhs=rstd[:, :width],
                             start=True, stop=True)
            st[bi] += [pmu, prs]
        for bi, (fn, dst, width, sof, bof, xd) in enumerate(blocks):
            sx, sxx, mu, rstd, pmu, prs = st[bi]
            for dt in range(DT):
                scr = lp3.tile([P, 512], F32, tag="ln_scr", bufs=2)
                nc.vector.tensor_sub(out=scr[:, :width], in0=fn(dt), in1=pmu[:, :width])
                nc.vector.tensor_mul(out=scr[:, :width], in0=scr[:, :width], in1=prs[:, :width])
                nc.vector.tensor_scalar(out=dst(dt), in0=scr[:, :width],
                                        scalar1=sof(dt), scalar2=bof(dt),
                                        op0=OP.mult, op1=OP.add)

    def load_w(dram_ap, tag_r, shape3, rpool=None):
        wr = (rpool or tp).tile(shape3, MDT, tag=tag_r)
        nc.sync.dma_start(out=wr[:], in_=dram_ap)
        return wr

    # ------------------------------------------------ transformer layers
    _knl = int(os.environ.get("KNL", NL))
    xh_pre = xpool.tile([P, DT, HALF], F32R, tag="xh_pre")
    wmlp = {}
    for l in range(_knl):
        li = l % NL
        lpv = (l - 1) % NL
        s1 = lambda dt: l1s[:, li, dt:dt + 1]
        b1_ = lambda dt: l1b[:, li, dt:dt + 1]
        y = lp.tile([P, DT, CHUNK], MDT, tag="y")
        yh = lp.tile([P, DT, HALF], MDT, tag="yh")
        krh = lp.tile([P, DT, HALF], MDT, tag="krh")
        # LN1 over own tokens; for layers >= 1, paired (stage-pipelined) with
        # the halo LN2 recompute so their stat chains hide each other.
        ln1_own = (lambda dt: x[:, dt, :], lambda dt: y[:, dt, :], CHUNK, s1, b1_, F32R)
        if l == 0:
            emit_ln([ln1_own])
        else:
            # --- halo recompute: the AllGather shipped the PRE-MLP residual
            # halo; apply the previous layer's LN2+MLP here (overlaps the
            # collective with the sender's MLP + this layer's own work).
            y2h = lp.tile([P, DT, HALF], MDT, tag="y2h")
            emit_ln([ln1_own,
                     (lambda dt: xh_pre[:, dt, :], lambda dt: y2h[:, dt, :], HALF,
                      lambda dt: l2s[:, lpv, dt:dt + 1], lambda dt: l2b[:, lpv, dt:dt + 1],
                      F32R)])
            w1p, w2p = wmlp[(l - 1) % 2]
            pbh = [ps_b.tile([P, CHUNK], F32, tag="ps_b", name=f"pbh{i}") for i in range(DT)]

            def emit_mlp2h(m, hmh):
                for do in range(DT):
                    nc.tensor.matmul(out=pbh[do][:, :HALF],
                                     lhsT=w2p[:, m, ts(do, P)], rhs=hmh[:],
                                     start=(m == 0), stop=(m == MT - 1))

            histh = []
            for m in range(MT):
                p1h = ps_a.tile([P, CHUNK], F32, tag="ps_a")
                for dt in range(DT):
                    nc.tensor.matmul(out=p1h[:, :HALF], lhsT=w1p[:, dt, ts(m, P)],
                                     rhs=y2h[:, dt, :], start=(dt == 0), stop=(dt == DT - 1))
                hmh = lp3.tile([P, HALF], MDT, tag="hmh", bufs=3)
                nc.scalar.activation(hmh[:], p1h[:, :HALF], AF.Gelu_apprx_tanh,
                                     bias=b1t[:, lpv, m:m + 1], scale=1.0)
                histh.append((m, hmh))
                if len(histh) > 2:
                    emit_mlp2h(*histh.pop(0))
            for mm_, hh_ in histh:
                emit_mlp2h(mm_, hh_)
            for do in range(DT):
                nc.vector.scalar_tensor_tensor(
                    out=xh[:, do, :], in0=pbh[do][:, :HALF],
                    scalar=b2t[:, lpv, do:do + 1], in1=xh_pre[:, do, :],
                    op0=OP.add, op1=OP.add)
            if l == 1 and "d_xh" in io:
                xhd = lp3.tile([P, DT, HALF], F32, tag="xhd", bufs=1)
                nc.vector.tensor_copy(out=xhd[:], in_=xh[:])
                nc.sync.dma_start(out=io["d_xh"].ap().rearrange("(t p) m -> p t m", p=P), in_=xhd[:])

        # --- projections (weights stationary, d-major out), own tokens
        wq_r = load_w(io["wq"].ap()[li].rearrange("p (t m) -> p t m", t=DT), "wr", [P, DT, D])
        qr = lp.tile([P, DT, CHUNK], MDT, tag="qr")
        for do in range(DT):
            pq = ps_a.tile([P, CHUNK], F32, tag="ps_a")
            for dt in range(DT):
                nc.tensor.matmul(out=pq[:], lhsT=wq_r[:, dt, ts(do, P)],
                                 rhs=y[:, dt, :], start=(dt == 0), stop=(dt == DT - 1))
            nc.vector.tensor_copy(out=qr[:, do, :], in_=pq[:])

        wk_r = load_w(io["wk"].ap()[li].rearrange("p (t m) -> p t m", t=DT), "wr", [P, DT, D])
        kro = lp.tile([P, DT, CHUNK], MDT, tag="kro")
        for do in range(DT):
            pk = ps_a.tile([P, CHUNK], F32, tag="ps_a")
            for dt in range(DT):
                nc.tensor.matmul(out=pk[:], lhsT=wk_r[:, dt, ts(do, P)],
                                 rhs=y[:, dt, :], start=(dt == 0), stop=(dt == DT - 1))
            nc.vector.tensor_copy(out=kro[:, do, :], in_=pk[:])

        wv_r = load_w(io["wv"].ap()[li].rearrange("p (t m) -> p t m", t=DT), "wr", [P, DT, D])
        for t in range(2, W // P):
            pv = ps_a.tile([P, D], F32, tag="ps_a")
            for dt in range(DT):
                nc.tensor.matmul(out=pv[:], lhsT=y[:, dt, ts(t - 2, P)], rhs=wv_r[:, dt, :],
                                 start=(dt == 0), stop=(dt == DT - 1))
            vtv = vt[t][:].rearrange("p (h c) -> p h c", c=DH + 1)
            nc.vector.tensor_copy(out=vtv[:, :, 0:DH],
                                  in_=pv[:].rearrange("p (h c) -> p h c", c=DH))

        # --- LN1 over halo + K/V halo
        emit_ln([(lambda dt: xh[:, dt, :], lambda dt: yh[:, dt, :], HALF, s1, b1_, F32R)])
        for do in range(DT):
            pk = ps_a.tile([P, CHUNK], F32, tag="ps_a")
            for dt in range(DT):
                nc.tensor.matmul(out=pk[:, :HALF], lhsT=wk_r[:, dt, ts(do, P)],
                                 rhs=yh[:, dt, :], start=(dt == 0), stop=(dt == DT - 1))
            nc.vector.tensor_copy(out=krh[:, do, :], in_=pk[:, :HALF])
        for t in range(2):
            pv = ps_a.tile([P, D], F32, tag="ps_a")
            for dt in range(DT):
                nc.tensor.matmul(out=pv[:], lhsT=yh[:, dt, ts(t, P)], rhs=wv_r[:, dt, :],
                                 start=(dt == 0), stop=(dt == DT - 1))
            vtv = vt[t][:].rearrange("p (h c) -> p h c", c=DH + 1)
            nc.vector.tensor_copy(out=vtv[:, :, 0:DH],
                                  in_=pv[:].rearrange("p (h c) -> p h c", c=DH))

        # --- sliding-window attention, deferred softmax division
        attru = lp.tile([P, DT, CHUNK], F32, tag="attru")
        attr = lp.tile([P, DT, CHUNK], MDT, tag="attr")

        def kslice(kt, r0, dto):
            if kt < 2:
                return krh[ds(r0, DH), dto, ts(kt, P)]
            return kro[ds(r0, DH), dto, ts(kt - 2, P)]

        def attn_qblock(qB):
            for h in range(H):
                r0 = (h % 2) * DH
                dto = h // 2
                pa = ps_c.tile([DH + 1, 256], F32, tag="ps_c")
                for jp in range(2):
                    pscore = ps_b.tile([P, 512], F32, tag="ps_b")
                    for jj in range(2):
                        j = jp * 2 + jj
                        kt = qB * 2 + j
                        nc.tensor.matmul(out=pscore[:, ts(jj, 256)],
                                         lhsT=kslice(kt, r0, dto),
                                         rhs=qr[ds(r0, DH), dto, ds(qB * 256, 256)],
                                         start=True, stop=False)
                        nc.tensor.matmul(out=pscore[:, ts(jj, 256)],
                                         lhsT=identm[:], rhs=masks[:, qB, j, :],
                                         start=False, stop=True)
                    ej = lp3.tile([P, 512], MDT, tag="ej", bufs=3)
                    nc.scalar.activation(ej[:], pscore[:], AF.Exp, bias=negb[:], scale=SCALE)
                    for jj in range(2):
                        j = jp * 2 + jj
                        kt = qB * 2 + j
                        nc.tensor.matmul(out=pa[:], lhsT=vt[kt][:, ds(h * (DH + 1), DH + 1)],
                                         rhs=ej[:, ts(jj, 256)], start=(j == 0), stop=(j == 3))
                nc.vector.tensor_copy(out=attru[ds(r0, DH), dto, ds(qB * 256, 256)],
                                      in_=pa[0:DH, :])
                dent = lp3.tile([1, 256], F32, tag="dent", bufs=2)
                nc.vector.tensor_copy(out=dent[:], in_=pa[DH:DH + 1, :])
                rf = lp3.tile([1, 256], F32, tag="rf", bufs=2)
                nc.vector.reciprocal_approx_fast(out=rf[:], in_=dent[:])
                rfh = lp3.tile([1, 256], MDT, tag="rfh", bufs=2)
                nc.vector.tensor_copy(out=rfh[:], in_=rf[:])
                if l == 0 and "d_rf" in io:
                    nc.sync.dma_start(out=io["d_rf"].ap()[qB * 8 + h:qB * 8 + h + 1, :], in_=rf[:])
                psc = ps_a.tile([DH, 256], F32, tag="ps_a")
                nc.tensor.matmul(out=psc[:], lhsT=ones[0:1, 0:DH], rhs=rfh[:],
                                 start=True, stop=True)
                nc.vector.tensor_mul(out=attr[ds(r0, DH), dto, ds(qB * 256, 256)],
                                     in0=attru[ds(r0, DH), dto, ds(qB * 256, 256)],
                                     in1=psc[:])

        attn_qblock(1)
        attn_qblock(0)

        if l == 0 and "d_y" in io:
            nc.sync.dma_start(out=io["d_y"].ap().rearrange("(t p) m -> p t m", p=P), in_=y[:])
            nc.sync.dma_start(out=io["d_yh"].ap().rearrange("(t p) m -> p t m", p=P), in_=yh[:])
            nc.sync.dma_start(out=io["d_attr"].ap().rearrange("(t p) m -> p t m", p=P), in_=attr[:])

        # --- output projection + residual
        wo_r = load_w(io["wo"].ap()[li].rearrange("p (t m) -> p t m", t=DT), "wr", [P, DT, D])
        for do in range(DT):
            po = ps_a.tile([P, CHUNK], F32, tag="ps_a")
            for dt in range(DT):
                nc.tensor.matmul(out=po[:], lhsT=wo_r[:, dt, ts(do, P)],
                                 rhs=attr[:, dt, :], start=(dt == 0), stop=(dt == DT - 1))
            nc.vector.tensor_add(out=x[:, do, :], in0=x[:, do, :], in1=po[:])

        # --- ship the PRE-MLP halo now; receiver recomputes its MLP.
        # This hides the AllGather under our MLP + the next layer's
        # halo-independent work.
        if l < NL - 1:
            xhs = lp.tile([P, DT, HALF], F32R, tag="xhs")
            nc.vector.tensor_copy(out=xhs[:], in_=x[:, :, ds(HALF, HALF)])
            agin = drp.tile([D, HALF], F32R, tag=f"agin{l}")
            agout = drp.tile([len(GROUPS[0]) * D, HALF], F32R, tag=f"agout{l}")
            nc.sync.dma_start(out=agin[:].rearrange("(t p) m -> p t m", p=P),
                              in_=xhs[:])
            nc.gpsimd.collective_compute(
                "AllGather", OP.bypass, replica_groups=GROUPS,
                ins=[agin.opt()], outs=[agout.opt()])
            for dt in range(DT):
                nc.gpsimd.indirect_dma_start(
                    out=xh_pre[:, dt, :], out_offset=None, in_=agout[:],
                    in_offset=bass.IndirectOffsetOnAxis(ap=hoffs[:, dt:dt + 1], axis=0))

        # --- LN2 + MLP
        y2 = lp.tile([P, DT, CHUNK], MDT, tag="y2")
        s2 = lambda dt: l2s[:, li, dt:dt + 1]
        b2_ = lambda dt: l2b[:, li, dt:dt + 1]
        emit_ln([(lambda dt: x[:, dt, 0:256], lambda dt: y2[:, dt, 0:256], 256, s2, b2_, F32R),
                 (lambda dt: x[:, dt, 256:512], lambda dt: y2[:, dt, 256:512], 256, s2, b2_, F32R)])

        pb = [ps_b.tile([P, CHUNK], F32, tag="ps_b", name=f"pb{i}") for i in range(DT)]
        w1r = lp.tile([P, DT, MLPD], MDT, tag=f"w1r{l % 2}")
        nc.sync.dma_start(out=w1r[:], in_=io["w1"].ap()[li].rearrange("p (t m) -> p t m", t=DT))
        w2r = lp.tile([P, MT, D], MDT, tag=f"w2r{l % 2}")
        nc.sync.dma_start(out=w2r[:], in_=io["w2"].ap()[li].rearrange("p (t m) -> p t m", t=MT))
        wmlp[l % 2] = (w1r, w2r)

        def emit_mlp2(m, hm):
            for do in range(DT):
                nc.tensor.matmul(out=pb[do][:], lhsT=w2r[:, m, ts(do, P)],
                                 rhs=hm[:], start=(m == 0), stop=(m == MT - 1))

        hist = []
        for m in range(MT):
            p1 = ps_a.tile([P, CHUNK], F32, tag="ps_a")
            for dt in range(DT):
                nc.tensor.matmul(out=p1[:], lhsT=w1r[:, dt, ts(m, P)],
                                 rhs=y2[:, dt, :], start=(dt == 0), stop=(dt == DT - 1))
            hm = lp3.tile([P, CHUNK], MDT, tag="hm", bufs=3)
            nc.scalar.activation(hm[:], p1[:], AF.Gelu_apprx_tanh,
                                 bias=b1t[:, li, m:m + 1], scale=1.0)
            hist.append((m, hm))
            if len(hist) > 2:
                emit_mlp2(*hist.pop(0))
        for mm_, hh_ in hist:
            emit_mlp2(mm_, hh_)
        # residual (+b2)
        for do in range(DT):
            nc.vector.scalar_tensor_tensor(
                out=x[:, do, :], in0=pb[do][:],
                scalar=b2t[:, li, do:do + 1], in1=x[:, do, :],
                op0=OP.add, op1=OP.add)
        if l == 0 and "d_x1" in io:
            xd = lp3.tile([P, DT, CHUNK], F32, tag="xdump", bufs=1)
            nc.vector.tensor_copy(out=xd[:], in_=x[:])
            nc.sync.dma_start(out=io["d_x1"].ap().rearrange("(t p) m -> p t m", p=P), in_=xd[:])

    # ------------------------------------------------ final LN + logits
    # Each core computes the FULL vocab for its own 512 tokens: no final
    # AllGather; w_out streams tile-by-tile from DRAM, prefetched by the
    # pool double-buffering. Output bias is added host-side.
    yf = lp.tile([P, DT, CHUNK], MDT, tag="y")
    fs_ = lambda dt: lfs[:, dt:dt + 1]
    fb_ = lambda dt: lfb[:, dt:dt + 1]
    emit_ln([(lambda dt: x[:, dt, 0:256], lambda dt: yf[:, dt, 0:256], 256, fs_, fb_, F32R),
             (lambda dt: x[:, dt, 256:512], lambda dt: yf[:, dt, 256:512], 256, fs_, fb_, F32R)])

    vtp.release()
    lp3.release()
    tp.release()

    ps_c.release()
    ps_b.release()
    fps = tc.alloc_tile_pool(name="fps", bufs=4, space="PSUM")
    with tc.tile_pool(name="ftrans", bufs=3) as ftp, \
         tc.tile_pool(name="fout", bufs=3) as fop:
        for g in range(VG):
            fwr = ftp.tile([P, GL, DT, P], MDT, tag="fwr")
            nc.sync.dma_start(out=fwr[:],
                              in_=io["w_tiles"].ap()[g]
                              .rearrange("p (j t q) -> p j t q", j=GL, t=DT))
            ot = fop.tile([P, GL, CHUNK], F16, tag="fot")
            for j in range(GL):
                pf = fps.tile([P, CHUNK], F32, tag="fps")
                for dt in range(DT):
                    nc.tensor.matmul(out=pf[:], lhsT=fwr[:, j, dt, :], rhs=yf[:, dt, :],
                                     start=(dt == 0), stop=(dt == DT - 1))
                if (g * GL + j) % 2 == 0:
                    nc.vector.tensor_copy(out=ot[:, j, :], in_=pf[:])
                else:
                    nc.scalar.activation(ot[:, j, :], pf[:], AF.Copy)
            nc.sync.dma_start(out=io["out"].ap()[g],
                              in_=ot[:].rearrange("p j m -> p (j m)"))

    fps.release()
    lp.release()
    drp.release()
    ps_a.release()
    xpool.release()
    cpool.release()


# ================================================================ host side
def _pe_table():
    pos = np.arange(S, dtype=np.float32)[:, None]
    div = np.exp(np.arange(0, D, 2, dtype=np.float32) * -(np.log(10000.0) / D))
    pe = np.zeros((S, D), dtype=np.float32)
    pe[:, 0::2] = np.sin(pos * div)
    pe[:, 1::2] = np.cos(pos * div)
    return pe


def _in_maps(inputs):
    inp = np.asarray(inputs["inputs"]).astype(np.int32)
    ids = np.pad(inp, ((0, 0), (1, 0)))[:, :-1].astype(np.int32)
    pe = _pe_table()
    wout = np.asarray(inputs["w_out"], dtype=np.float32).astype(np.float16)
    def dmaj(a):
        # [X, (DT_, P)-rows, M] -> [X, P, DT_*M] device layout (d-major tiles)
        a = np.asarray(a)
        nl, dd, m = a.shape
        return np.ascontiguousarray(
            a.reshape(nl, dd // P, P, m).transpose(0, 2, 1, 3).reshape(nl, P, (dd // P) * m))

    def prow(a, tiles):
        # [.., tiles*P] -> [P, .. * tiles] per-partition rows
        a = np.asarray(a, np.float32).reshape(-1, tiles, P)
        return np.ascontiguousarray(a.transpose(2, 0, 1).reshape(P, -1))

    shared = {"embed": np.ascontiguousarray(np.asarray(inputs["embed"], np.float32))}
    for k in ("ln1_s", "ln1_b", "ln2_s", "ln2_b"):
        shared[k] = prow(inputs[k], DT)
    shared["b1"] = prow(inputs["b1"], MT)
    shared["b2"] = prow(inputs["b2"], DT)
    for k in ("wq", "wk", "wv", "wo", "w1"):
        shared[k] = dmaj(np.asarray(inputs[k], np.float32).astype(np.float16))
    shared["w2"] = dmaj(np.asarray(inputs["w2"], np.float32).astype(np.float16))
    shared["lnf_s"] = prow(np.asarray(inputs["lnf_s"], np.float32).reshape(1, D), DT)
    shared["lnf_b"] = prow(np.asarray(inputs["lnf_b"], np.float32).reshape(1, D), DT)
    # w_tiles[g, p, ((j*DT+dt)*P)+q] = w_out[dt*128+p, (g*GL+j)*128+q]
    shared["w_tiles"] = np.ascontiguousarray(
        wout.reshape(DT, P, VG, GL, P).transpose(2, 1, 3, 0, 4)
        .reshape(VG, P, GL * DT * P))

    maps = []
    qi = np.arange(256)[None, :]
    ki = np.arange(P)[:, None]
    for c in range(NCORES):
        b, ch = divmod(c, NCORES // B)
        t0 = ch * CHUNK
        lo = t0 - HALF
        ids768 = np.zeros(W, np.int32)
        pe768 = np.zeros((W, D), np.float32)
        s0 = max(0, lo)
        ids768[s0 - lo:] = ids[b, s0:t0 + CHUNK]
        pe768[s0 - lo:] = pe[s0:t0 + CHUNK]
        m = np.zeros((2, 4, P, 256), np.float16)
        for qB in range(2):
            for j in range(4):
                w = 256 + qi - (j * P + ki)      # u_q - u_k
                ok = (w >= 0) & (w <= HALF)
                if ch == 0:
                    ok = ok & ((lo + qB * 256 + j * P + ki) >= 0)
                m[qB, j] = np.where(ok, 0.0, MASK_BIAS).astype(np.float16)
        src = ch - 1 if ch > 0 else 0
        hoffs = (src * D + np.arange(DT)[None, :] * P
                 + np.arange(P)[:, None]).astype(np.int32)
        mp = dict(shared)
        mp.update(
            idx_in=np.ascontiguousarray(ids768.reshape(W // P, P).T),
            pe_dm=np.ascontiguousarray(
                pe768.T.reshape(DT, P, W).transpose(1, 0, 2).reshape(P, DT * W)),
            masks=np.ascontiguousarray(
                m.transpose(2, 0, 1, 3).reshape(P, 2 * 4 * 256)),
            halo_offs=hoffs)
        maps.append(mp)
    return maps


def kernel(**inputs):
    nc = _CACHE.get("nc")
    if nc is None:
        nc = _build()
        _CACHE["nc"] = nc
    maps = _in_maps(inputs)
    res = run_bass_kernel_spmd(nc, maps, list(range(NCORES))).results
    bout = np.asarray(inputs["b_out"], dtype=np.float32)
    full = np.empty((NTOK, V), np.float32)
    for c in range(NCORES):
        lv = (res[c]["logits_vm"].reshape(VG, P, GL, CHUNK)
              .transpose(0, 2, 1, 3).reshape(V, CHUNK))
        full[c * CHUNK:(c + 1) * CHUNK, :] = lv.T.astype(np.float32) + bout[None, :]
    return full.reshape(B, S, V)


# revision 39
# speedup vs baseline: 1.5922x; 1.0591x over previous
"""Longformer decoder (4 layers, sliding-window causal attention) on 8 trn2 cores.

Sharding: 4096 tokens (B=2 x S=2048) split into 8 contiguous chunks of 512
(core = b*4 + chunk). Activations are kept d-major ([dim, token], dim on
partitions) so every matmul is weights-stationary with no transposes.
Attention needs a 256-token left halo of K/V per layer: layer 0 computes it
locally from the embedding gather; layers 1-3 AllGather the residual-stream
halo over 4-core groups, overlapped with the next layer's halo-independent
work (LN1/Q/K/V over own tokens). Sliding-window masking is additive (-3000)
and applied inside PSUM via an identity-matmul accumulate, so the scalar
engine's exp produces masked probabilities directly. Softmax division is
deferred: unnormalized attention output and per-(head,q) denominators are
collected, one batched reciprocal per q-block computes 1/den for all heads,
and an indicator-matrix matmul broadcasts it back over the d-major layout.
The final projection is token-sharded: each core computes the FULL vocab for
its own 512 tokens (no final AllGather); w_out streams from DRAM in 5-tile
groups, and the output bias is added host-side.
"""
import os
import sys

import numpy as np

for _p in ("/opt/trn_rl_repo", "/root/.axon_site/_ro/trn_rl_repo"):
    if os.path.isdir(_p) and _p not in sys.path:
        sys.path.insert(0, _p)

import concourse.bass as bass
import concourse.mybir as mybir
import concourse.tile as tile
from concourse import bacc
from concourse.bass import ts, ds
from concourse.bass_utils import run_bass_kernel_spmd
from concourse.masks import make_identity

F32 = mybir.dt.float32
F32R = mybir.dt.float32r
F16 = mybir.dt.float16
I32 = mybir.dt.int32
MDT = F16 if os.environ.get("KMMDT", "f16") == "f16" else F32R
AF = mybir.ActivationFunctionType
OP = mybir.AluOpType

B, S, V, D, H, NL, MLPD = 2, 2048, 32000, 512, 8, 4, 2048
DH = D // H            # 64
HALF = 256             # attention half-window (WIN // 2)
P = 128
NCORES = 8
CHUNK = 512            # own tokens per core
W = CHUNK + HALF       # 768 = halo + own
DT = D // P            # 4 d-tiles
MT = MLPD // P         # 16 mlp tiles
VN = V // P            # 250 vocab tiles (each core does full vocab x own tokens)
GL = 5                 # vocab tiles per DMA group
VG = VN // GL          # 50 groups (batched DMA: 5KB contiguous per partition)
NTOK = B * S           # 4096
GROUPS = [[0, 1, 2, 3], [4, 5, 6, 7]]
EXP_SHIFT = 2.0
MASK_BIAS = -3000.0    # additive mask; exp(SCALE*(s+MASK_BIAS)+EXP_SHIFT) == 0
SCALE = float(1.0 / np.sqrt(DH))

_CACHE = {}


# ================================================================ builder
def _build():
    nc = bacc.Bacc("TRN2", target_bir_lowering=False, debug=False,
                   num_devices=NCORES)

    ein = lambda n, sh, dt=F32: nc.dram_tensor(n, sh, dt, kind="ExternalInput")
    io = dict(
        wq=ein("wq", [NL, P, DT * D], MDT), wk=ein("wk", [NL, P, DT * D], MDT),
        wv=ein("wv", [NL, P, DT * D], MDT), wo=ein("wo", [NL, P, DT * D], MDT),
        w1=ein("w1", [NL, P, DT * MLPD], MDT), w2=ein("w2", [NL, P, MT * D], MDT),
        b1=ein("b1", [P, NL * MT]), b2=ein("b2", [P, NL * DT]),
        ln1_s=ein("ln1_s", [P, NL * DT]), ln1_b=ein("ln1_b", [P, NL * DT]),
        ln2_s=ein("ln2_s", [P, NL * DT]), ln2_b=ein("ln2_b", [P, NL * DT]),
        lnf_s=ein("lnf_s", [P, DT]), lnf_b=ein("lnf_b", [P, DT]),
        w_tiles=ein("w_tiles", [VG, P, GL * DT * P], MDT),
        embed=ein("embed", [V, D]),
        idx_in=ein("idx_in", [P, W // P], I32),
        pe_dm=ein("pe_dm", [P, DT * W]),
        masks=ein("masks", [P, 2 * 4 * 256], MDT),
        halo_offs=ein("halo_offs", [P, DT], I32),
        out=nc.dram_tensor("logits_vm", [VG, P, GL * CHUNK], F16, kind="ExternalOutput"),
    )
    if os.environ.get("KDEBUG") == "1":
        io["d_y"] = nc.dram_tensor("d_y", [D, CHUNK], F16, kind="ExternalOutput")
        io["d_yh"] = nc.dram_tensor("d_yh", [D, HALF], F16, kind="ExternalOutput")
        io["d_attr"] = nc.dram_tensor("d_attr", [D, CHUNK], F16, kind="ExternalOutput")
        io["d_rf"] = nc.dram_tensor("d_rf", [16, 256], F32, kind="ExternalOutput")
        io["d_x1"] = nc.dram_tensor("d_x1", [D, CHUNK], F32, kind="ExternalOutput")
        io["d_xh"] = nc.dram_tensor("d_xh", [D, HALF], F32, kind="ExternalOutput")
        io["d_xhp"] = nc.dram_tensor("d_xhp", [D, HALF], F32, kind="ExternalOutput")
        io["d_y2h"] = nc.dram_tensor("d_y2h", [D, HALF], F16, kind="ExternalOutput")

    with tile.TileContext(nc) as tc, nc.allow_low_precision(reason="f32r rounding"):
        _emit(nc, tc, io)
    nc.compile()
    return nc


def _emit(nc, tc, io):
    cpool = tc.alloc_tile_pool(name="const", bufs=1)
    xpool = tc.alloc_tile_pool(name="xres", bufs=1)
    ps_a = tc.alloc_tile_pool(name="ps_a", bufs=2, space="PSUM")
    ps_b = tc.alloc_tile_pool(name="ps_b", bufs=4, space="PSUM")
    ps_c = tc.alloc_tile_pool(name="ps_c", bufs=2, space="PSUM")
    drp = tc.alloc_tile_pool(name="drbounce", bufs=1, space="DRAM")

    # ------------------------------------------------ constants
    ones_f = cpool.tile([P, P], F32, tag="ones_f")
    nc.vector.memset(ones_f[:], 1.0)
    ones = cpool.tile([P, P], MDT, tag="ones")
    nc.vector.tensor_copy(out=ones[:], in_=ones_f[:])
    ones_r = cpool.tile([P, P], F32R, tag="ones_r")
    nc.vector.tensor_copy(out=ones_r[:], in_=ones_f[:])
    identm = cpool.tile([P, P], MDT, tag="identm")
    make_identity(nc, identm[:])
    negb = cpool.tile([P, 1], F32, tag="negb")
    nc.vector.memset(negb[:], EXP_SHIFT)
    epsb = cpool.tile([P, 1], F32, tag="epsb")
    nc.vector.memset(epsb[:], 1e-6)
    l1s = cpool.tile([P, NL, DT], F32, tag="l1s")
    l1b = cpool.tile([P, NL, DT], F32, tag="l1b")
    l2s = cpool.tile([P, NL, DT], F32, tag="l2s")
    l2b = cpool.tile([P, NL, DT], F32, tag="l2b")
    lfs = cpool.tile([P, DT], F32, tag="lfs")
    lfb = cpool.tile([P, DT], F32, tag="lfb")
    b1t = cpool.tile([P, NL, MT], F32, tag="b1t")
    b2t = cpool.tile([P, NL, DT], F32, tag="b2t")
    masks = cpool.tile([P, 2, 4, 256], MDT, tag="masks")
    hoffs = cpool.tile([P, DT], I32, tag="hoffs")

    def load_consts():
        nc.sync.dma_start(out=l1s[:], in_=io["ln1_s"].ap().rearrange("p (l t) -> p l t", l=NL))
        nc.sync.dma_start(out=l1b[:], in_=io["ln1_b"].ap().rearrange("p (l t) -> p l t", l=NL))
        nc.sync.dma_start(out=l2s[:], in_=io["ln2_s"].ap().rearrange("p (l t) -> p l t", l=NL))
        nc.sync.dma_start(out=l2b[:], in_=io["ln2_b"].ap().rearrange("p (l t) -> p l t", l=NL))
        nc.sync.dma_start(out=lfs[:], in_=io["lnf_s"].ap())
        nc.sync.dma_start(out=lfb[:], in_=io["lnf_b"].ap())
        nc.sync.dma_start(out=b1t[:], in_=io["b1"].ap().rearrange("p (l m) -> p l m", l=NL))
        nc.sync.dma_start(out=b2t[:], in_=io["b2"].ap().rearrange("p (l t) -> p l t", l=NL))
        nc.sync.dma_start(out=masks[:], in_=io["masks"].ap().rearrange("p (a b q) -> p a b q", a=2, b=4))
        nc.sync.dma_start(out=hoffs[:], in_=io["halo_offs"].ap())

    # residual stream (own 512 tokens, d-major, f32r so LN sum-matmuls can
    # consume it directly at full PE rate) + per-layer halo
    x = xpool.tile([P, DT, CHUNK], F32R, tag="x")
    xh = xpool.tile([P, DT, HALF], F32R, tag="xh")

    # ------------------------------------------------ embedding
    with tc.tile_pool(name="embed", bufs=1) as epool:
        ident = epool.tile([P, P], F32, tag="ident")
        make_identity(nc, ident[:])
        pe = epool.tile([P, DT, W], F32, tag="pe")
        nc.sync.dma_start(out=pe[:], in_=io["pe_dm"].ap().rearrange("p (t m) -> p t m", t=DT))
        idxt = epool.tile([P, W // P], I32, tag="idxt")
        nc.sync.dma_start(out=idxt[:], in_=io["idx_in"].ap())
        with tc.tile_pool(name="gath", bufs=2) as gpool:
            for g in range(W // P):
                gt = gpool.tile([P, D], F32, tag="gt")
                nc.gpsimd.indirect_dma_start(
                    out=gt[:], out_offset=None, in_=io["embed"].ap(),
                    in_offset=bass.IndirectOffsetOnAxis(ap=idxt[:, g:g + 1], axis=0),
                )
                for dt in range(DT):
                    pt = ps_a.tile([P, P], F32, tag="ps_a")
                    nc.tensor.transpose(pt[:], gt[:, ts(dt, P)], ident[:])
                    dst = xh[:, dt, ts(g, P)] if g < 2 else x[:, dt, ts(g - 2, P)]
                    nc.vector.tensor_add(out=dst, in0=pt[:], in1=pe[:, dt, ts(g, P)])

    load_consts()

    # ------------------------------------------------ layer pools
    lp = tc.alloc_tile_pool(name="layers", bufs=1)
    tp = tc.alloc_tile_pool(name="ltrans", bufs=2)
    lp3 = tc.alloc_tile_pool(name="ltrans3", bufs=3)
    vtp = tc.alloc_tile_pool(name="vtpool", bufs=1)

    # V tiles with a trailing ones column per head: PV matmul row DH
    # accumulates the softmax denominator for free. Ones written once.
    vt = [vtp.tile([P, H * (DH + 1)], MDT, tag=f"vt{t}", name=f"vt{t}")
          for t in range(W // P)]
    for t in range(W // P):
        vtv = vt[t][:].rearrange("p (h c) -> p h c", c=DH + 1)
        nc.vector.tensor_copy(out=vtv[:, :, DH:DH + 1], in_=ones[:, 0:H])

    def emit_ln(blocks):
        """LN over d (partition axis x DT), stage-pipelined across blocks.
        blocks: list of (src_fn(dt), dst_fn(dt), width, s_of, b_of, xdt)."""
        st = []
        for bi, (fn, dst, width, sof, bof, xd) in enumerate(blocks):
            spool = ps_a if bi % 2 == 0 else ps_b
            ptag = "ps_a" if bi % 2 == 0 else "ps_b"
            ones_x = ones_r if xd == F32R else ones
            sx = spool.tile([1, 512], F32, tag=ptag)
            sxx = spool.tile([1, 512], F32, tag=ptag)
            for dt in range(DT):
                xsq = lp3.tile([P, 512], MDT, tag="ln_xsq", bufs=2)
                nc.vector.tensor_mul(out=xsq[:, :width], in0=fn(dt), in1=fn(dt))
                nc.tensor.matmul(out=sx[:, :width], lhsT=ones_x[:, 0:1], rhs=fn(dt),
                                 start=(dt == 0), stop=(dt == DT - 1))
                nc.tensor.matmul(out=sxx[:, :width], lhsT=ones[:, 0:1], rhs=xsq[:, :width],
                                 start=(dt == 0), stop=(dt == DT - 1))
            st.append([sx, sxx])
        for bi, (fn, dst, width, sof, bof, xd) in enumerate(blocks):
            sx, sxx = st[bi]
            mu = lp3.tile([1, 512], MDT, tag="ln_mu", bufs=2)
            nc.vector.tensor_scalar_mul(out=mu[:, :width], in0=sx[:, :width], scalar1=1.0 / D)
            mu2 = lp3.tile([1, 512], F32, tag="ln_mu2", bufs=2)
            nc.vector.tensor_mul(out=mu2[:, :width], in0=mu[:, :width], in1=mu[:, :width])
            var = lp3.tile([1, 512], F32, tag="ln_var", bufs=2)
            # var = sxx/D - mu^2
            nc.vector.scalar_tensor_tensor(
                out=var[:, :width], in0=sxx[:, :width], scalar=1.0 / D,
                in1=mu2[:, :width], op0=OP.mult, op1=OP.subtract)
            sd = lp3.tile([1, 512], F32, tag="ln_sd", bufs=2)
            nc.scalar.activation(sd[:, :width], var[:, :width], AF.Sqrt, bias=epsb[0:1, :], scale=1.0)
            rt = lp3.tile([1, 512], F32, tag="ln_rt", bufs=2)
            nc.vector.reciprocal_approx_fast(out=rt[:, :width], in_=sd[:, :width])
            rstd = lp3.tile([1, 512], MDT, tag="ln_rstd", bufs=2)
            nc.vector.tensor_copy(out=rstd[:, :width], in_=rt[:, :width])
            st[bi] += [mu, rstd]
        for bi, (fn, dst, width, sof, bof, xd) in enumerate(blocks):
            sx, sxx, mu, rstd = st[bi]
            bpool = ps_c if bi % 2 == 0 else ps_b
            ptag = "ps_c" if bi % 2 == 0 else "ps_b"
            pmu = bpool.tile([P, 512], F32, tag=ptag)
            nc.tensor.matmul(out=pmu[:, :width], lhsT=ones[0:1, :], rhs=mu[:, :width],
                             start=True, stop=True)
            prs = bpool.tile([P, 512], F32, tag=ptag)
            nc.tensor.matmul(out=prs[:, :width], lhsT=ones[0:1, :], rhs=rstd[:, :width],
                             start=True, stop=True)
            st[bi] += [pmu, prs]
        for bi, (fn, dst, width, sof, bof, xd) in enumerate(blocks):
            sx, sxx, mu, rstd, pmu, prs = st[bi]
            for dt in range(DT):
                scr = lp3.tile([P, 512], F32, tag="ln_scr", bufs=2)
                nc.vector.tensor_sub(out=scr[:, :width], in0=fn(dt), in1=pmu[:, :width])
                nc.vector.tensor_mul(out=scr[:, :width], in0=scr[:, :width], in1=prs[:, :width])
                nc.vector.tensor_scalar(out=dst(dt), in0=scr[:, :width],
                                        scalar1=sof(dt), scalar2=bof(dt),
                                        op0=OP.mult, op1=OP.add)

    def load_w(dram_ap, tag_r, shape3, rpool=None):
        wr = (rpool or tp).tile(shape3, MDT, tag=tag_r)
        nc.sync.dma_start(out=wr[:], in_=dram_ap)
        return wr

    # ------------------------------------------------ transformer layers
    _knl = int(os.environ.get("KNL", NL))
    xh_pre = xpool.tile([P, DT, HALF], F16, tag="xh_pre")
    wmlp = {}
    for l in range(_knl):
        li = l % NL
        lpv = (l - 1) % NL
        s1 = lambda dt: l1s[:, li, dt:dt + 1]
        b1_ = lambda dt: l1b[:, li, dt:dt + 1]
        y = lp.tile([P, DT, CHUNK], MDT, tag="y")
        yh = lp.tile([P, DT, HALF], MDT, tag="yh")
        krh = lp.tile([P, DT, HALF], MDT, tag="krh")
        # LN1 over own tokens first: halo-independent.
        emit_ln([(lambda dt: x[:, dt, :], lambda dt: y[:, dt, :], CHUNK, s1, b1_, F32R)])

        # --- projections (weights stationary, d-major out), own tokens
        wq_r = load_w(io["wq"].ap()[li].rearrange("p (t m) -> p t m", t=DT), "wr", [P, DT, D])
        qr = lp.tile([P, DT, CHUNK], MDT, tag="qr")
        for do in range(DT):
            pq = ps_a.tile([P, CHUNK], F32, tag="ps_a")
            for dt in range(DT):
                nc.tensor.matmul(out=pq[:], lhsT=wq_r[:, dt, ts(do, P)],
                                 rhs=y[:, dt, :], start=(dt == 0), stop=(dt == DT - 1))
            nc.vector.tensor_copy(out=qr[:, do, :], in_=pq[:])

        wk_r = load_w(io["wk"].ap()[li].rearrange("p (t m) -> p t m", t=DT), "wr", [P, DT, D])
        kro = lp.tile([P, DT, CHUNK], MDT, tag="kro")
        for do in range(DT):
            pk = ps_a.tile([P, CHUNK], F32, tag="ps_a")
            for dt in range(DT):
                nc.tensor.matmul(out=pk[:], lhsT=wk_r[:, dt, ts(do, P)],
                                 rhs=y[:, dt, :], start=(dt == 0), stop=(dt == DT - 1))
            nc.vector.tensor_copy(out=kro[:, do, :], in_=pk[:])

        wv_r = load_w(io["wv"].ap()[li].rearrange("p (t m) -> p t m", t=DT), "wr", [P, DT, D])
        for t in range(2, W // P):
            pv = ps_a.tile([P, D], F32, tag="ps_a")
            for dt in range(DT):
                nc.tensor.matmul(out=pv[:], lhsT=y[:, dt, ts(t - 2, P)], rhs=wv_r[:, dt, :],
                                 start=(dt == 0), stop=(dt == DT - 1))
            vtv = vt[t][:].rearrange("p (h c) -> p h c", c=DH + 1)
            nc.vector.tensor_copy(out=vtv[:, :, 0:DH],
                                  in_=pv[:].rearrange("p (h c) -> p h c", c=DH))

        # --- sliding-window attention, deferred softmax division
        attru = lp.tile([P, DT, CHUNK], F32, tag="attru")
        attr = lp.tile([P, DT, CHUNK], MDT, tag="attr")

        def kslice(kt, r0, dto):
            if kt < 2:
                return krh[ds(r0, DH), dto, ts(kt, P)]
            return kro[ds(r0, DH), dto, ts(kt - 2, P)]

        def attn_qblock(qB):
            for h in range(H):
                r0 = (h % 2) * DH
                dto = h // 2
                pa = ps_c.tile([DH + 1, 256], F32, tag="ps_c")
                for jp in range(2):
                    pscore = ps_b.tile([P, 512], F32, tag="ps_b")
                    for jj in range(2):
                        j = jp * 2 + jj
                        kt = qB * 2 + j
                        nc.tensor.matmul(out=pscore[:, ts(jj, 256)],
                                         lhsT=kslice(kt, r0, dto),
                                         rhs=qr[ds(r0, DH), dto, ds(qB * 256, 256)],
                                         start=True, stop=False)
                        nc.tensor.matmul(out=pscore[:, ts(jj, 256)],
                                         lhsT=identm[:], rhs=masks[:, qB, j, :],
                                         start=False, stop=True)
                    ej = lp3.tile([P, 512], MDT, tag="ej", bufs=3)
                    nc.scalar.activation(ej[:], pscore[:], AF.Exp, bias=negb[:], scale=SCALE)
                    for jj in range(2):
                        j = jp * 2 + jj
                        kt = qB * 2 + j
                        nc.tensor.matmul(out=pa[:], lhsT=vt[kt][:, ds(h * (DH + 1), DH + 1)],
                                         rhs=ej[:, ts(jj, 256)], start=(j == 0), stop=(j == 3))
                nc.vector.tensor_copy(out=attru[ds(r0, DH), dto, ds(qB * 256, 256)],
                                      in_=pa[0:DH, :])
                dent = lp3.tile([1, 256], F32, tag="dent", bufs=2)
                nc.vector.tensor_copy(out=dent[:], in_=pa[DH:DH + 1, :])
                rf = lp3.tile([1, 256], F32, tag="rf", bufs=2)
                nc.vector.reciprocal_approx_fast(out=rf[:], in_=dent[:])
                rfh = lp3.tile([1, 256], MDT, tag="rfh", bufs=2)
                nc.vector.tensor_copy(out=rfh[:], in_=rf[:])
                if l == 0 and "d_rf" in io:
                    nc.sync.dma_start(out=io["d_rf"].ap()[qB * 8 + h:qB * 8 + h + 1, :], in_=rf[:])
                psc = ps_a.tile([DH, 256], F32, tag="ps_a")
                nc.tensor.matmul(out=psc[:], lhsT=ones[0:1, 0:DH], rhs=rfh[:],
                                 start=True, stop=True)
                nc.vector.tensor_mul(out=attr[ds(r0, DH), dto, ds(qB * 256, 256)],
                                     in0=attru[ds(r0, DH), dto, ds(qB * 256, 256)],
                                     in1=psc[:])

        # q-block 1 attends only to own keys: runs while the halo is in flight
        attn_qblock(1)

        # --- LN1 over halo + K/V halo (halo arrives via AllGather; consumed
        # as late as possible so the collective hides under own-token work)
        hx, hdt = (xh, F32R) if l == 0 else (xh_pre, F16)
        emit_ln([(lambda dt: hx[:, dt, :], lambda dt: yh[:, dt, :], HALF, s1, b1_, hdt)])
        for do in range(DT):
            pk = ps_a.tile([P, CHUNK], F32, tag="ps_a")
            for dt in range(DT):
                nc.tensor.matmul(out=pk[:, :HALF], lhsT=wk_r[:, dt, ts(do, P)],
                                 rhs=yh[:, dt, :], start=(dt == 0), stop=(dt == DT - 1))
            nc.vector.tensor_copy(out=krh[:, do, :], in_=pk[:, :HALF])
        for t in range(2):
            pv = ps_a.tile([P, D], F32, tag="ps_a")
            for dt in range(DT):
                nc.tensor.matmul(out=pv[:], lhsT=yh[:, dt, ts(t, P)], rhs=wv_r[:, dt, :],
                                 start=(dt == 0), stop=(dt == DT - 1))
            vtv = vt[t][:].rearrange("p (h c) -> p h c", c=DH + 1)
            nc.vector.tensor_copy(out=vtv[:, :, 0:DH],
                                  in_=pv[:].rearrange("p (h c) -> p h c", c=DH))

        attn_qblock(0)

        if l == 0 and "d_y" in io:
            nc.sync.dma_start(out=io["d_y"].ap().rearrange("(t p) m -> p t m", p=P), in_=y[:])
            nc.sync.dma_start(out=io["d_yh"].ap().rearrange("(t p) m -> p t m", p=P), in_=yh[:])
            nc.sync.dma_start(out=io["d_attr"].ap().rearrange("(t p) m -> p t m", p=P), in_=attr[:])

        # --- output projection + residual
        wo_r = load_w(io["wo"].ap()[li].rearrange("p (t m) -> p t m", t=DT), "wr", [P, DT, D])
        for do in range(DT):
            po = ps_a.tile([P, CHUNK], F32, tag="ps_a")
            for dt in range(DT):
                nc.tensor.matmul(out=po[:], lhsT=wo_r[:, dt, ts(do, P)],
                                 rhs=attr[:, dt, :], start=(dt == 0), stop=(dt == DT - 1))
            nc.vector.tensor_add(out=x[:, do, :], in0=x[:, do, :], in1=po[:])

        # --- LN2 + MLP
        y2 = lp.tile([P, DT, CHUNK], MDT, tag="y2")
        s2 = lambda dt: l2s[:, li, dt:dt + 1]
        b2_ = lambda dt: l2b[:, li, dt:dt + 1]
        emit_ln([(lambda dt: x[:, dt, 0:256], lambda dt: y2[:, dt, 0:256], 256, s2, b2_, F32R),
                 (lambda dt: x[:, dt, 256:512], lambda dt: y2[:, dt, 256:512], 256, s2, b2_, F32R)])

        pb = [ps_b.tile([P, CHUNK], F32, tag="ps_b", name=f"pb{i}") for i in range(DT)]
        w1r = lp.tile([P, DT, MLPD], MDT, tag="w1r")
        nc.sync.dma_start(out=w1r[:], in_=io["w1"].ap()[li].rearrange("p (t m) -> p t m", t=DT))
        w2r = lp.tile([P, MT, D], MDT, tag="w2r")
        nc.sync.dma_start(out=w2r[:], in_=io["w2"].ap()[li].rearrange("p (t m) -> p t m", t=MT))

        def emit_mlp2(m, hm):
            for do in range(DT):
                nc.tensor.matmul(out=pb[do][:], lhsT=w2r[:, m, ts(do, P)],
                                 rhs=hm[:], start=(m == 0), stop=(m == MT - 1))

        hist = []
        for m in range(MT):
            p1 = ps_a.tile([P, CHUNK], F32, tag="ps_a")
            for dt in range(DT):
                nc.tensor.matmul(out=p1[:], lhsT=w1r[:, dt, ts(m, P)],
                                 rhs=y2[:, dt, :], start=(dt == 0), stop=(dt == DT - 1))
            hm = lp3.tile([P, CHUNK], MDT, tag="hm", bufs=3)
            nc.scalar.activation(hm[:], p1[:], AF.Gelu_apprx_tanh,
                                 bias=b1t[:, li, m:m + 1], scale=1.0)
            hist.append((m, hm))
            if len(hist) > 2:
                emit_mlp2(*hist.pop(0))
        for mm_, hh_ in hist:
            emit_mlp2(mm_, hh_)
        # residual (+b2)
        for do in range(DT):
            nc.vector.scalar_tensor_tensor(
                out=x[:, do, :], in0=pb[do][:],
                scalar=b2t[:, li, do:do + 1], in1=x[:, do, :],
                op0=OP.add, op1=OP.add)
        if l == 0 and "d_x1" in io:
            xd = lp3.tile([P, DT, CHUNK], F32, tag="xdump", bufs=1)
            nc.vector.tensor_copy(out=xd[:], in_=x[:])
            nc.sync.dma_start(out=io["d_x1"].ap().rearrange("(t p) m -> p t m", p=P), in_=xd[:])
        if l < NL - 1:
            xhs = lp.tile([P, DT, HALF], F16, tag="xhs")
            nc.vector.tensor_copy(out=xhs[:], in_=x[:, :, ds(HALF, HALF)])
            agin = drp.tile([D, HALF], F16, tag=f"agin{l}")
            agout = drp.tile([len(GROUPS[0]) * D, HALF], F16, tag=f"agout{l}")
            nc.sync.dma_start(out=agin[:].rearrange("(t p) m -> p t m", p=P),
                              in_=xhs[:])
            nc.gpsimd.collective_compute(
                "AllGather", OP.bypass, replica_groups=GROUPS,
                ins=[agin.opt()], outs=[agout.opt()])
            for dt in range(DT):
                nc.gpsimd.indirect_dma_start(
                    out=xh_pre[:, dt, :], out_offset=None, in_=agout[:],
                    in_offset=bass.IndirectOffsetOnAxis(ap=hoffs[:, dt:dt + 1], axis=0))

    # ------------------------------------------------ final LN + logits
    # Each core computes the FULL vocab for its own 512 tokens: no final
    # AllGather; w_out streams tile-by-tile from DRAM, prefetched by the
    # pool double-buffering. Output bias is added host-side.
    yf = lp.tile([P, DT, CHUNK], MDT, tag="y")
    fs_ = lambda dt: lfs[:, dt:dt + 1]
    fb_ = lambda dt: lfb[:, dt:dt + 1]
    emit_ln([(lambda dt: x[:, dt, 0:256], lambda dt: yf[:, dt, 0:256], 256, fs_, fb_, F32R),
             (lambda dt: x[:, dt, 256:512], lambda dt: yf[:, dt, 256:512], 256, fs_, fb_, F32R)])

    vtp.release()
    lp3.release()
    tp.release()

    ps_c.release()
    ps_b.release()
    fps = tc.alloc_tile_pool(name="fps", bufs=4, space="PSUM")
    with tc.tile_pool(name="ftrans", bufs=3) as ftp, \
         tc.tile_pool(name="fout", bufs=3) as fop:
        for g in range(VG):
            fwr = ftp.tile([P, GL, DT, P], MDT, tag="fwr")
            nc.sync.dma_start(out=fwr[:],
                              in_=io["w_tiles"].ap()[g]
                              .rearrange("p (j t q) -> p j t q", j=GL, t=DT))
            ot = fop.tile([P, GL, CHUNK], F16, tag="fot")
            for j in range(GL):
                pf = fps.tile([P, CHUNK], F32, tag="fps")
                for dt in range(DT):
                    nc.tensor.matmul(out=pf[:], lhsT=fwr[:, j, dt, :], rhs=yf[:, dt, :],
                                     start=(dt == 0), stop=(dt == DT - 1))
                if (g * GL + j) % 2 == 0:
                    nc.vector.tensor_copy(out=ot[:, j, :], in_=pf[:])
                else:
                    nc.scalar.activation(ot[:, j, :], pf[:], AF.Copy)
            nc.sync.dma_start(out=io["out"].ap()[g],
                              in_=ot[:].rearrange("p j m -> p (j m)"))

    fps.release()
    lp.release()
    drp.release()
    ps_a.release()
    xpool.release()
    cpool.release()


# ================================================================ host side
def _pe_table():
    pos = np.arange(S, dtype=np.float32)[:, None]
    div = np.exp(np.arange(0, D, 2, dtype=np.float32) * -(np.log(10000.0) / D))
    pe = np.zeros((S, D), dtype=np.float32)
    pe[:, 0::2] = np.sin(pos * div)
    pe[:, 1::2] = np.cos(pos * div)
    return pe


def _in_maps(inputs):
    inp = np.asarray(inputs["inputs"]).astype(np.int32)
    ids = np.pad(inp, ((0, 0), (1, 0)))[:, :-1].astype(np.int32)
    pe = _pe_table()
    wout = np.asarray(inputs["w_out"], dtype=np.float32).astype(np.float16)
    def dmaj(a):
        # [X, (DT_, P)-rows, M] -> [X, P, DT_*M] device layout (d-major tiles)
        a = np.asarray(a)
        nl, dd, m = a.shape
        return np.ascontiguousarray(
            a.reshape(nl, dd // P, P, m).transpose(0, 2, 1, 3).reshape(nl, P, (dd // P) * m))

    def prow(a, tiles):
        # [.., tiles*P] -> [P, .. * tiles] per-partition rows
        a = np.asarray(a, np.float32).reshape(-1, tiles, P)
        return np.ascontiguousarray(a.transpose(2, 0, 1).reshape(P, -1))

    shared = {"embed": np.ascontiguousarray(np.asarray(inputs["embed"], np.float32))}
    for k in ("ln1_s", "ln1_b", "ln2_s", "ln2_b"):
        shared[k] = prow(inputs[k], DT)
    shared["b1"] = prow(inputs["b1"], MT)
    shared["b2"] = prow(inputs["b2"], DT)
    for k in ("wq", "wk", "wv", "wo", "w1"):
        shared[k] = dmaj(np.asarray(inputs[k], np.float32).astype(np.float16))
    shared["w2"] = dmaj(np.asarray(inputs["w2"], np.float32).astype(np.float16))
    shared["lnf_s"] = prow(np.asarray(inputs["lnf_s"], np.float32).reshape(1, D), DT)
    shared["lnf_b"] = prow(np.asarray(inputs["lnf_b"], np.float32).reshape(1, D), DT)
    # w_tiles[g, p, ((j*DT+dt)*P)+q] = w_out[dt*128+p, (g*GL+j)*128+q]
    shared["w_tiles"] = np.ascontiguousarray(
        wout.reshape(DT, P, VG, GL, P).transpose(2, 1, 3, 0, 4)
        .reshape(VG, P, GL * DT * P))

    maps = []
    qi = np.arange(256)[None, :]
    ki = np.arange(P)[:, None]
    for c in range(NCORES):
        b, ch = divmod(c, NCORES // B)
        t0 = ch * CHUNK
        lo = t0 - HALF
        ids768 = np.zeros(W, np.int32)
        pe768 = np.zeros((W, D), np.float32)
        s0 = max(0, lo)
        ids768[s0 - lo:] = ids[b, s0:t0 + CHUNK]
        pe768[s0 - lo:] = pe[s0:t0 + CHUNK]
        m = np.zeros((2, 4, P, 256), np.float16)
        for qB in range(2):
            for j in range(4):
                w = 256 + qi - (j * P + ki)      # u_q - u_k
                ok = (w >= 0) & (w <= HALF)
                if ch == 0:
                    ok = ok & ((lo + qB * 256 + j * P + ki) >= 0)
                m[qB, j] = np.where(ok, 0.0, MASK_BIAS).astype(np.float16)
        src = ch - 1 if ch > 0 else 0
        hoffs = (src * D + np.arange(DT)[None, :] * P
                 + np.arange(P)[:, None]).astype(np.int32)
        mp = dict(shared)
        mp.update(
            idx_in=np.ascontiguousarray(ids768.reshape(W // P, P).T),
            pe_dm=np.ascontiguousarray(
                pe768.T.reshape(DT, P, W).transpose(1, 0, 2).reshape(P, DT * W)),
            masks=np.ascontiguousarray(
                m.transpose(2, 0, 1, 3).reshape(P, 2 * 4 * 256)),
            halo_offs=hoffs)
        maps.append(mp)
    return maps


def kernel(**inputs):
    nc = _CACHE.get("nc")
    if nc is None:
        nc = _build()
        _CACHE["nc"] = nc
    maps = _in_maps(inputs)
    res = run_bass_kernel_spmd(nc, maps, list(range(NCORES))).results
    bout = np.asarray(inputs["b_out"], dtype=np.float32)
    full = np.empty((NTOK, V), np.float32)
    for c in range(NCORES):
        lv = (res[c]["logits_vm"].reshape(VG, P, GL, CHUNK)
              .transpose(0, 2, 1, 3).reshape(V, CHUNK))
        full[c * CHUNK:(c + 1) * CHUNK, :] = lv.T.astype(np.float32) + bout[None, :]
    return full.reshape(B, S, V)
